# revision 31
# baseline (speedup 1.0000x reference)
"""Bass/Trainium2 kernel for nn_MemoryGAT (3-layer GATv2 + MLP head), 8 NeuronCores.

Nodes are degree-balanced into 8x98 tiles of 128 (K edge-tiles per node tile,
K~4). hs rows are written straight into a device-shared hs_full buffer with
batched indirect scatters; a 1-element AllGather acts as the cross-core
barrier (no bulk collective). Edge loop gathers hs[src] in multi-tile batched
indirect DMAs (SWDGE fixed cost amortized), builds the one-hot S / S^T
selection masks on DVE+Pool without PE transposes, accumulates msg in paired
PSUM banks, and keeps LN stats via accum_out. z stays in SBUF end to end.
"""

import sys
import types

sys.path.insert(0, "/opt/trn_rl_repo")

import ml_dtypes
import numpy as np
import orjson

# ---------------------------------------------------------------- shims

_counter = [0]


def _legalize_module(m, maxw=1):
    """This walrus build accepts only ONE sync-wait per instruction; hoist
    overflow waits onto NoOps inserted just before, on the same engine."""
    for f in m.get("functions", []):
        for b in f.get("blocks", []):
            insts = b.get("instructions")
            if not insts:
                continue
            out = []
            for inst in insts:
                si = inst.get("sync_info")
                waits = (si or {}).get("on_wait") or []
                if si is not None and len(waits) > maxw:
                    keep = waits[-maxw:]
                    extra = waits[: len(waits) - maxw]
                    for j in range(0, len(extra), maxw):
                        _counter[0] += 1
                        out.append(
                            {
                                "name": f"ant-wsplit-{_counter[0]}",
                                "opcode": "NoOp",
                                "engine": inst.get("engine"),
                                "ins": [],
                                "outs": [],
                                "sync_info": {
                                    "on_wait": extra[j : j + maxw],
                                    "on_update": [],
                                },
                            }
                        )
                    si["on_wait"] = keep
                out.append(inst)
            b["instructions"] = out
    return m


def _install_shims():
    import antenv

    if "antenv.axon_hooks" not in sys.modules:
        try:
            from trn_agent_boot.trn_boot import _ntff_profile_via_ctypes

            hooks = types.ModuleType("antenv.axon_hooks")
            hook = _ntff_profile_via_ctypes("/opt/axon/libaxon_pjrt.so")
            hooks.get_axon_ntff_profile_hook = lambda: hook
            hooks.set_axon_ntff_profile_hook = lambda h: None
            sys.modules["antenv.axon_hooks"] = hooks
            antenv.axon_hooks = hooks
        except Exception:
            pass

    import concourse.bass as bass
    from concourse import bass_utils

    bass_utils.upload_artifacts = lambda tmpdir: tmpdir

    if not getattr(bass.Bass, "_waitfix_installed", False):
        base = bass.Bass.to_json_bytes

        def patched(self):
            return orjson.dumps(_legalize_module(orjson.loads(base(self))))

        bass.Bass.to_json_bytes = patched
        bass.Bass._waitfix_installed = True


_install_shims()

import concourse.bass as bass
import concourse.tile as tile
from concourse import mybir
from concourse.bass_utils import run_bass_kernel_spmd

F32 = mybir.dt.float32
BF = mybir.dt.bfloat16
AF = mybir.ActivationFunctionType
ALU = mybir.AluOpType

# ---------------------------------------------------------------- sizes
N = 100_000
E = 400_000
FN = 267
DC = 256
H, D = 4, 64
HD = 256
ED = 11
NCORES = 8
P = 128
NT = 98
NPAD = NT * P  # 12544
NFULL = NCORES * NPAD
# AllGather chunk boundaries (in node tiles) and hs_full region bases
CHT = [0, 40, 72, 92, 98]
NCH = len(CHT) - 1
CHROWS = [(CHT[i + 1] - CHT[i]) * P for i in range(NCH)]
CHBASE = [0]
for i in range(NCH - 1):
    CHBASE.append(CHBASE[-1] + NCORES * CHROWS[i])

TRACE = False
DEBUG = False
LAST_RESULT = {}


# ---------------------------------------------------------------- builder
def build_nc(K, bh2_val):
    NTK = NT * K
    ES = NTK * P
    KP = K * P

    nc = bass.Bass()
    dp = nc.declare_dram_parameter

    x_T = dp("x_T", [384, NPAD], BF, isOutput=False)
    src_c = dp("src_c", [P, NTK], mybir.dt.int32, isOutput=False)
    tgt_c = dp("tgt_c", [P, NTK], BF, isOutput=False)
    tgt_r = dp("tgt_r", [1, ES], BF, isOutput=False)
    ea_T = dp("ea_T", [16, ES], BF, isOutput=False)
    wp1 = dp("wp1", [384, 64], BF, isOutput=False)
    iota2d = dp("iota2d", [P, P], BF, isOutput=False)
    iota_col = dp("iota_col", [P, 1], F32, isOutput=False)
    ident = dp("ident", [P, P], BF, isOutput=False)
    wh1 = dp("wh1", [64, 32], F32, isOutput=False)
    bh1_rep = dp("bh1_rep", [P, 32], F32, isOutput=False)
    wh2_rep = dp("wh2_rep", [P, 32], F32, isOutput=False)

    LW = []
    for l, ind in ((0, 65), (1, 256), (2, 256)):
        d = {"ind": ind, "outd": 64 if l == 2 else 256}
        d["wswt"] = dp(f"wswt{l}", [ind, 512], BF, isOutput=False)
        d["we"] = dp(f"we{l}", [16, 256], BF, isOutput=False)
        d["a_rep2"] = dp(f"a_rep2_{l}", [P, 2 * KP * 2], BF, isOutput=False)
        if l != 1:
            d["skw"] = dp(f"skw{l}", [ind, d["outd"]], BF, isOutput=False)
            d["skb_rep"] = dp(f"skb_rep{l}", [P, d["outd"]], BF, isOutput=False)
        d["gn_rep"] = dp(f"gn_rep{l}", [P, d["outd"]], BF, isOutput=False)
        d["bn_rep"] = dp(f"bn_rep{l}", [P, d["outd"]], BF, isOutput=False)
        LW.append(d)

    out = dp("out", [P, NT], F32, isOutput=True)
    if DEBUG:
        dbg_z0 = dp("dbg_z0", [P, NT, 64], BF, isOutput=True)
        dbg_ht = dp("dbg_ht", [P, NT * 256], BF, isOutput=True)
        dbg_z1 = dp("dbg_z1", [P, NT, 256], BF, isOutput=True)
        dbg_lr = dp("dbg_lr", [P, 2, 1024], BF, isOutput=True)
        dbg_st = dp("dbg_st", [P, 512], BF, isOutput=True)

    hs_shard = [nc.dram_tensor(f"hs_shard{l}", [NPAD, 256], BF) for l in range(3)]
    hs_full = [
        nc.dram_tensor(f"hs_full{l}", [NFULL, 256], BF, addr_space="Shared")
        for l in range(3)
    ]
    res0_dram = nc.dram_tensor("res0_dram", [NPAD, 256], BF)
    h1_dram = nc.dram_tensor("h1_dram", [NPAD, 256], BF)

    with tile.TileContext(nc) as tc:
        with (
            tc.tile_pool(name="const", bufs=1) as cpool,
            tc.tile_pool(name="work", bufs=2) as wpool,
            tc.tile_pool(name="small", bufs=2) as spool,
            tc.tile_pool(name="persist", bufs=1) as ppool,
            tc.tile_pool(name="psPair", bufs=4, space="PSUM") as psPair,
            tc.tile_pool(name="psB", bufs=2, space="PSUM") as psB,
            tc.tile_pool(name="psC", bufs=2, space="PSUM") as psC,
        ):
            for v in {1e-5, 1e-8, float(bh2_val)}:
                ct = cpool.tile([P, 1], F32, tag=f"k{v}", name=f"k{_counter[0]}")
                _counter[0] += 1
                nc.vector.memset(ct[:], v)
                nc.const_aps.aps[(F32, float(v))] = ct[:]

            _cn = [0]

            def c_load(ap, shape, dt=F32):
                _cn[0] += 1
                t = cpool.tile(shape, dt, tag=f"c{_cn[0]}", name=f"c{_cn[0]}")
                nc.sync.dma_start(out=t[:], in_=ap[:])
                return t

            def c_load_chunks(ap, kk, ck, n, dt=F32):
                _cn[0] += 1
                t = cpool.tile([kk, ck * n], dt, tag=f"c{_cn[0]}", name=f"c{_cn[0]}")
                for c in range(ck):
                    nc.sync.dma_start(
                        out=t[:, c * n : (c + 1) * n],
                        in_=ap[c * kk : (c + 1) * kk, :],
                    )
                return t

            iota_sb = c_load(iota2d, [P, P], BF)
            idb_sb = c_load(ident, [P, P], BF)
            iotac_sb = c_load(iota_col, [P, 1], F32)
            iotaK_sb = cpool.tile([P, KP], BF, tag="iotaK", name="iotaK")
            for k in range(K):
                nc.vector.tensor_copy(iotaK_sb[:, k * P : (k + 1) * P], iota_sb[:])
            ones1p = cpool.tile([1, P], BF, tag="ones1p", name="ones1p")
            nc.vector.memset(ones1p[:], 1.0)
            wp1_sb = c_load_chunks(wp1, P, 3, 64, BF)
            wh1_sb = c_load(wh1, [64, 32])
            bh1_sb = c_load(bh1_rep, [P, 32])
            wh2_sb = c_load(wh2_rep, [P, 32])
            lws = []
            for l, d in enumerate(LW):
                s = {}
                ck = max(d["ind"] // P, 1)
                kk = min(d["ind"], P)
                s["wswt"] = c_load_chunks(d["wswt"], kk, ck, 512, BF)
                s["we"] = c_load(d["we"], [16, 256], BF)
                if "skw" in d:
                    s["skw"] = c_load_chunks(d["skw"], kk, ck, d["outd"], BF)
                    s["skb"] = c_load(d["skb_rep"], [P, d["outd"]], BF)
                s["gn"] = c_load(d["gn_rep"], [P, d["outd"]], BF)
                s["bn"] = c_load(d["bn_rep"], [P, d["outd"]], BF)
                s["ck"], s["kk"] = ck, kk
                lws.append(s)

            srcs = ppool.tile([P, NTK], mybir.dt.int32)
            nc.sync.dma_start(out=srcs[:], in_=src_c[:])
            tgts = ppool.tile([P, NTK], BF)
            nc.sync.dma_start(out=tgts[:], in_=tgt_c[:])

            ht_all = ppool.tile([P, NT * 256], BF)
            z_all = ppool.tile([P, NT, 256], BF)
            res2_all = ppool.tile([P, NT * 64], BF)
            scores = ppool.tile([P, NT], F32)

            # one shared LN-stat set; stages are strictly phased so WAR
            # deps keep this safe
            _st = {}
            for nm in ("s1", "s2", "m", "va", "rstd"):
                _st[nm] = ppool.tile([P, NT], F32, tag=f"st{nm}", name=f"st{nm}")
            stats = [_st] * 4

            def sqrt_batch(i, dim):
                st = stats[i]
                nc.vector.tensor_scalar_mul(st["m"][:], st["s1"][:], 1.0 / dim)
                nc.vector.tensor_scalar_mul(st["va"][:], st["s2"][:], 1.0 / dim)
                nm2 = spool.tile([P, NT], F32, tag="nm2")
                nc.vector.scalar_tensor_tensor(
                    nm2[:], st["m"][:], -1.0, st["m"][:], op0=ALU.mult, op1=ALU.mult
                )
                nc.vector.tensor_add(st["va"][:], st["va"][:], nm2[:])
                sd = spool.tile([P, NT], F32, tag="sd")
                nc.scalar.activation(sd[:], st["va"][:], AF.Sqrt, bias=1e-5)
                nc.vector.reciprocal(st["rstd"][:], sd[:])
                nc.vector.scalar_tensor_tensor(
                    st["va"][:], st["m"][:], -1.0, st["rstd"][:],
                    op0=ALU.mult, op1=ALU.mult,
                )

            def ag_chunk(l, c):
                nc.gpsimd.collective_compute(
                    "AllGather",
                    ALU.bypass,
                    ins=[hs_shard[l][CHT[c] * P : CHT[c + 1] * P, :]],
                    outs=[
                        hs_full[l][CHBASE[c] : CHBASE[c] + NCORES * CHROWS[c], :]
                    ],
                    replica_groups=[list(range(NCORES))],
                )

            # ---------------- phase 0: u = x@Wp (+ctx/bias via ones-row),
            # z0 = gelu(u), accumulate LN stats -------------------------
            with nc.named_scope("p0"):
                for t in range(NT):
                    if t % 4 == 0:
                        nbt = min(4, NT - t)
                        xt = wpool.tile([P, 3, 4 * P], BF, tag="hsg", bufs=3)
                        for c in range(3):
                            nc.sync.dma_start(
                                out=xt[:, c, : nbt * P],
                                in_=x_T[c * P : (c + 1) * P, t * P : (t + nbt) * P],
                            )
                    xoff = (t % 4) * P
                    h0p = psPair.tile([P, 512], F32, tag="pair", name="h0p")[:, 0:64]
                    for c in range(3):
                        nc.tensor.matmul(
                            out=h0p,
                            lhsT=xt[:, c, xoff : xoff + P],
                            rhs=wp1_sb[:, c * 64 : (c + 1) * 64],
                            start=(c == 0),
                            stop=(c == 2),
                        )
                    zsl = z_all[:, t, 0:64]
                    nc.scalar.activation(
                        zsl, h0p, AF.Gelu, accum_out=stats[0]["s1"][:, t : t + 1]
                    )
                    junk = wpool.tile([P, 256], BF, tag="junk", bufs=1, name="junk0")[:, 0:64]
                    nc.vector.scalar_tensor_tensor(
                        junk, zsl, 1.0, zsl,
                        op0=ALU.mult, op1=ALU.mult,
                        accum_out=stats[0]["s2"][:, t : t + 1],
                    )
                sqrt_batch(0, 64)

            # ---------------- F2A(l): finalize h_l, project, scatter+barrier
            # Software-pipelined: hn(t) | transpose(t-1) | proj+copies(t-2)
            def f2a(l):
                st = stats[l]
                ind = 64 if l == 0 else (256 if l < 3 else 64)
                s = lws[l] if l < 3 else None
                hns = {}
                lhss = {}
                h3Ts = {}
                hshts = {}
                rps = {}
                stag_h1 = None
                stag_hs = None
                stag_res = None
                for step in range(NT + 3):
                    # ---- stage A: produce hn(step)
                    if step < NT:
                        t = step
                        if l == 1 and t % 4 == 0:
                            stag_h1 = wpool.tile(
                                [P, 4, 256], BF, tag="r4x256", bufs=3, name="sh1"
                            )
                        if l == 0:
                            hn = wpool.tile(
                                [P, 256], BF, tag="hn", bufs=4, name="hn0"
                            )[:, :ind]
                            nc.scalar.activation(
                                hn, z_all[:, t, :ind], AF.Identity,
                                bias=st["va"][:, t : t + 1],
                                scale=st["rstd"][:, t : t + 1],
                            )
                        else:
                            if l == 1:
                                hn = stag_h1[:, t % 4, :]
                            else:
                                hn = wpool.tile(
                                    [P, 256], BF, tag="hn", bufs=4, name="hnl"
                                )[:, :ind]
                            g_sb = lws[l - 1]["gn"]
                            b_sb = lws[l - 1]["bn"]
                            t1 = wpool.tile(
                                [P, 256], F32, tag="t1", bufs=1, name="t1"
                            )[:, :ind]
                            nc.vector.scalar_tensor_tensor(
                                t1, z_all[:, t, :ind], st["m"][:, t : t + 1],
                                g_sb[:, :ind], op0=ALU.subtract, op1=ALU.mult,
                            )
                            u = wpool.tile(
                                [P, 256], F32, tag="u", bufs=2, name="u"
                            )[:, :ind]
                            nc.vector.scalar_tensor_tensor(
                                u, t1, st["rstd"][:, t : t + 1], b_sb[:, :ind],
                                op0=ALU.mult, op1=ALU.add,
                            )
                            nc.scalar.activation(hn, u, AF.Gelu)
                        hns[t] = hn
                        if l == 1 and (t % 4 == 3 or t == NT - 1):
                            t0 = t - (t % 4)
                            nc.sync.dma_start(
                                out=h1_dram[:].rearrange("(t p) c -> p t c", p=P)[
                                    :, t0 : t + 1, :
                                ],
                                in_=stag_h1[:, : t - t0 + 1, :],
                            )
                    # ---- stage B: transpose hn(step-1)
                    if 1 <= step <= NT:
                        t = step - 1
                        hn = hns[t]
                        if l == 3:
                            trp = psB.tile([P, 2, P], BF, tag="tr")
                            nc.tensor.transpose(
                                out=trp[0:64, 0, :], in_=hn, identity=idb_sb[:]
                            )
                            h3T = wpool.tile([64, P], F32, tag="h3T", bufs=3)
                            nc.scalar.copy(h3T[:], trp[0:64, 0, :])
                            h3Ts[t] = h3T
                        elif l == 0:
                            h0t = wpool.tile([65, P], BF, tag="h0t", bufs=4)
                            if t < 4:
                                nc.vector.memset(h0t[64:65, :], 1.0)
                            trp = psB.tile([P, 2, P], BF, tag="tr")
                            nc.tensor.transpose(
                                out=trp[0:64, 0, :], in_=hn, identity=idb_sb[:]
                            )
                            nc.vector.tensor_copy(h0t[0:64, :], trp[0:64, 0, :])
                            lhss[t] = [h0t[:, :]]
                        else:
                            trp = psB.tile([P, 2, P], BF, tag="tr")
                            for c in range(2):
                                nc.tensor.transpose(
                                    out=trp[:, c, :],
                                    in_=hn[:, c * P : (c + 1) * P],
                                    identity=idb_sb[:],
                                )
                            hnT = wpool.tile([P, 2, P], BF, tag="hnT", bufs=4)
                            nc.vector.tensor_copy(hnT[:], trp[:])
                            lhss[t] = [hnT[:, c, :] for c in range(2)]
                    # ---- stage C: project for tile step-2
                    if 2 <= step <= NT + 1:
                        t = step - 2
                        if l == 3:
                            pass
                        else:
                            lhs = lhss.pop(t)
                            hns.pop(t, None)
                            ck = s["ck"]
                            hsht = psPair.tile([P, 512], F32, tag="pair", name="hsht")
                            for c in range(ck):
                                nc.tensor.matmul(
                                    out=hsht[:],
                                    lhsT=lhs[c],
                                    rhs=s["wswt"][:, c * 512 : (c + 1) * 512],
                                    start=(c == 0),
                                    stop=(c == ck - 1),
                                )
                            hshts[t] = hsht
                            if l != 1:
                                outd = LW[l]["outd"]
                                rp = psC.tile([P, 512], F32, tag="agg", name="rp")[
                                    :, :outd
                                ]
                                for c in range(ck):
                                    nc.tensor.matmul(
                                        out=rp,
                                        lhsT=lhs[c],
                                        rhs=s["skw"][:, c * outd : (c + 1) * outd],
                                        start=(c == 0),
                                        stop=(c == ck - 1),
                                    )
                                rps[t] = rp
                    # ---- stage D: copies + stores for tile step-3
                    if step < 3:
                        continue
                    t = step - 3
                    if l == 3:
                        h3T = h3Ts.pop(t)
                        sp1 = psC.tile([P, 512], F32, tag="agg", name="sp1")[:, :32]
                        nc.tensor.matmul(
                            out=sp1, lhsT=h3T[:], rhs=wh1_sb[:], start=True, stop=True
                        )
                        u1 = wpool.tile([P, 32], F32, tag="u1", bufs=2)
                        nc.vector.tensor_add(u1[:], sp1, bh1_sb[:])
                        g1 = wpool.tile([P, 32], F32, tag="g1", bufs=2)
                        nc.scalar.activation(g1[:], u1[:], AF.Gelu)
                        j32 = wpool.tile([P, 32], BF, tag="j32", bufs=2)
                        nc.vector.scalar_tensor_tensor(
                            j32[:], g1[:], 1.0, wh2_sb[:],
                            op0=ALU.mult, op1=ALU.mult,
                            accum_out=scores[:, t : t + 1],
                        )
                        hns.pop(t, None)
                        continue
                    if t % 4 == 0:
                        stag_hs = wpool.tile(
                            [P, 4, 256], BF, tag="stag_hs", bufs=2, name="shs"
                        )
                        if l == 0:
                            stag_res = wpool.tile(
                                [P, 4, 256], BF, tag="r4x256", bufs=3, name="sres"
                            )
                    hsht = hshts.pop(t)
                    nc.scalar.copy(stag_hs[:, t % 4, :], hsht[:, 0:256])
                    nc.scalar.copy(
                        ht_all[:, t * 256 : (t + 1) * 256], hsht[:, 256:512]
                    )
                    if l != 1:
                        rp = rps.pop(t)
                        if l == 0:
                            nc.scalar.copy(stag_res[:, t % 4, :], rp)
                        else:
                            nc.vector.scalar_tensor_tensor(
                                res2_all[:, t * 64 : (t + 1) * 64], rp, 1.0,
                                s["skb"][:], op0=ALU.mult, op1=ALU.add,
                            )
                    # batched stores + AG chunks
                    if t % 4 == 3 or t == NT - 1:
                        t0 = t - (t % 4)
                        nbt = t - t0 + 1
                        nc.sync.dma_start(
                            out=hs_shard[l][:].rearrange("(t p) c -> p t c", p=P)[
                                :, t0 : t0 + nbt, :
                            ],
                            in_=stag_hs[:, :nbt, :],
                        )
                        if l == 0:
                            nc.sync.dma_start(
                                out=res0_dram[:].rearrange("(t p) c -> p t c", p=P)[
                                    :, t0 : t0 + nbt, :
                                ],
                                in_=stag_res[:, :nbt, :],
                            )
                    for c in range(NCH):
                        if t == CHT[c + 1] - 1:
                            ag_chunk(l, c)

            # ---------------- edge + F1 loop --------------------------------
            # Deep pipeline: every cross-engine dep is >=1 tile old.
            #  front(t):   loads, 4 gathers(t) [Pool], ST/S masks(t) [V]
            #  msg(t-1):   10 matmuls [PE] + 2 Prelu(t-2) [S]
            #  alpha(j):   at t=2j+4: scr2/alph2 [V], exp [S]
            #  mid(j):     at t=2j+5: w4 [V], w4col [S]
            #  tail_pe(j): at t=2j+6: agg [PE], den [S]
            #  tail_vs(j): at t=2j+7: rden/gat/z [V], square [S]
            def edge_f1(l):
                s = lws[l]
                outd = LW[l]["outd"]
                st = stats[l + 1]
                a2_sb = wpool.tile(
                    [P, 2 * KP * 2], BF, tag="arep", bufs=1, name=f"arep{l}"
                )
                nc.sync.dma_start(out=a2_sb[:], in_=LW[l]["a_rep2"][:])
                info = {}
                tinfo = {}
                res_sb = [None]

                def front(t):
                    j = t // 2
                    if t % 2 == 0:
                        d = {"S": {}, "res": {}, "msgp": {}}
                        info[j] = d
                        d["hsg"] = wpool.tile(
                            [P, 2 * K * 256], BF, tag="hsg", bufs=3, name="hsg"
                        )
                        d["lr2"] = wpool.tile(
                            [P, 2, 1024], BF, tag="lr2", bufs=2, name="lr2"
                        )
                        ea_sb = wpool.tile([16, 2 * KP], BF, tag="ea", bufs=2)
                        nc.sync.dma_start(
                            out=ea_sb[:], in_=ea_T[:, t * KP : (t + 2) * KP]
                        )
                        tr_sb = wpool.tile([P, 2 * KP], BF, tag="tgtr", bufs=2)
                        nc.sync.dma_start(
                            out=tr_sb[:],
                            in_=tgt_r[0:1, t * KP : (t + 2) * KP].to_broadcast(
                                (P, 2 * KP)
                            ),
                        )
                        d["ea"], d["tr"] = ea_sb, tr_sb
                    d = info[j]
                    if l < 2:
                        if t % 4 == 0:
                            nbr = min(4, NT - t)
                            res_sb[0] = wpool.tile(
                                [P, 4, 256], BF, tag="r4x256", bufs=3, name="res_sb"
                            )
                            rdram = res0_dram if l == 0 else h1_dram
                            nc.sync.dma_start(
                                out=res_sb[0][:, :nbr, :],
                                in_=rdram[:].rearrange("(t p) c -> p t c", p=P)[
                                    :, t : t + nbr, :
                                ],
                            )
                        d["res"][t] = res_sb[0][:, t % 4, :]
                    else:
                        d["res"][t] = res2_all[:, t * 64 : (t + 1) * 64]
                    for k in range(K):
                        nc.gpsimd.indirect_dma_start(
                            out=d["hsg"][
                                :, ((t % 2) * K + k) * 256 : ((t % 2) * K + k + 1) * 256
                            ],
                            out_offset=None,
                            in_=hs_full[l][:],
                            in_offset=bass.IndirectOffsetOnAxis(
                                ap=srcs[:, t * K + k : t * K + k + 1], axis=0
                            ),
                        )
                    eoff = (t % 2) * KP
                    ST_all = wpool.tile([P, KP], BF, tag="ST", bufs=3)
                    nc.vector.tensor_scalar(
                        ST_all[:], d["tr"][:, eoff : eoff + KP], iotac_sb[:, 0:1],
                        None, op0=ALU.is_equal,
                    )
                    S_all = wpool.tile([P, KP], BF, tag="S", bufs=7)
                    nc.vector.tensor_tensor(
                        out=S_all[:].rearrange("p (k c) -> p k c", k=K),
                        in0=iotaK_sb[:].rearrange("p (k c) -> p k c", k=K),
                        in1=tgts[:, t * K : (t + 1) * K].to_broadcast((P, K, P)),
                        op=ALU.is_equal,
                    )
                    d["S"][t] = S_all
                    tinfo[t] = (ST_all, d)

                def msg(t):
                    ST_all, d = tinfo.pop(t)
                    j = t // 2
                    eoff = (t % 2) * KP
                    d["msgp"][t] = []
                    for jj in range(2):
                        msgp = psPair.tile([P, 512], F32, tag="pair", name="msgp")
                        cb = ((t % 2) * K + 2 * jj) * 256
                        nc.tensor.matmul(
                            out=msgp[:], lhsT=idb_sb[:],
                            rhs=d["hsg"][:, cb : cb + 512],
                            start=True, stop=False, skip_group_check=True,
                        )
                        for c in range(2):
                            k = 2 * jj + c
                            nc.tensor.matmul(
                                out=msgp[:, c * 256 : (c + 1) * 256],
                                lhsT=d["ea"][:, eoff + k * P : eoff + (k + 1) * P],
                                rhs=s["we"][:],
                                start=False, stop=False, skip_group_check=True,
                            )
                        for c in range(2):
                            k = 2 * jj + c
                            nc.tensor.matmul(
                                out=msgp[:, c * 256 : (c + 1) * 256],
                                lhsT=ST_all[:, k * P : (k + 1) * P],
                                rhs=ht_all[:, t * 256 : (t + 1) * 256],
                                start=False, stop=(c == 1), skip_group_check=True,
                            )
                        d["msgp"][t].append(msgp)

                def prelu(t):
                    j = t // 2
                    d = info[j]
                    for jj in range(2):
                        nc.scalar.activation(
                            d["lr2"][:, t % 2, jj * 512 : (jj + 1) * 512],
                            d["msgp"][t][jj][:],
                            AF.Prelu, alpha=0.2,
                        )
                    del d["msgp"][t]

                def alpha(j):
                    d = info[j]
                    scr2 = wpool.tile([P, 2048], BF, tag="scr2", bufs=1)
                    nc.vector.tensor_tensor(
                        out=scr2[:],
                        in0=d["lr2"][:].rearrange("p a b -> p (a b)"),
                        in1=a2_sb[:],
                        op=ALU.mult,
                    )
                    alph2 = spool.tile([P, 32], F32, tag="alph", bufs=2)
                    nc.vector.tensor_reduce(
                        out=alph2[:],
                        in_=scr2[:].rearrange("p (g d) -> p g d", d=64),
                        axis=mybir.AxisListType.X,
                        op=ALU.add,
                    )
                    expa2 = spool.tile([P, 32], F32, tag="expa", bufs=2)
                    nc.scalar.activation(expa2[:], alph2[:], AF.Exp)
                    d["expa"] = expa2

                def mid(j):
                    d = info[j]
                    d["waug"] = []
                    for i in range(2):
                        w_aug = wpool.tile(
                            [P, K * 260], BF, tag="waug", bufs=2, name="waug"
                        )
                        w4 = w_aug[:].rearrange("p (k h c) -> p k h c", k=K, c=65)
                        hs4 = d["hsg"][
                            :, i * K * 256 : (i + 1) * K * 256
                        ].rearrange("p (k h dd) -> p k h dd", k=K, dd=64)
                        e4 = d["expa"][:, i * 16 : (i + 1) * 16].rearrange(
                            "p (k h) -> p k h", k=K
                        )
                        nc.vector.tensor_tensor(
                            out=w4[:, :, :, 0:64],
                            in0=hs4[:],
                            in1=e4[:].to_broadcast((P, K, H, 64)),
                            op=ALU.mult,
                        )
                        nc.scalar.copy(w4[:, :, :, 64], e4)
                        d["waug"].append(w_aug)

                def tail_pe(j):
                    d = info[j]
                    d["agg"] = []
                    d["den"] = []
                    for i in range(2):
                        agg = psC.tile([P, 512], F32, tag="agg", name="agg")[:, 0:260]
                        for k in range(K):
                            nc.tensor.matmul(
                                out=agg,
                                lhsT=d["S"][2 * j + i][:, k * P : (k + 1) * P],
                                rhs=d["waug"][i][:, k * 260 : (k + 1) * 260],
                                start=(k == 0),
                                stop=(k == K - 1),
                            )
                        d["agg"].append(agg)
                    for i in range(2):
                        aggv = d["agg"][i].rearrange("p (h c) -> p h c", c=65)
                        den = spool.tile([P, 4], F32, tag="den", bufs=4)
                        nc.scalar.activation(den[:], aggv[:, :, 64], AF.Copy, bias=1e-8)
                        d["den"].append(den)

                def tail_vs(j):
                    d = info.pop(j)
                    for i in range(2):
                        tt = 2 * j + i
                        aggv = d["agg"][i].rearrange("p (h c) -> p h c", c=65)
                        rden = spool.tile([P, 4], F32, tag="rden", bufs=2)
                        nc.vector.reciprocal(rden[:], d["den"][i][:])
                        gat = wpool.tile([P, 256], F32, tag="gat", bufs=1)
                        nc.vector.scalar_tensor_tensor(
                            gat[:].rearrange("p (h dd) -> p h dd", h=4),
                            aggv[:, :, 0:64],
                            0.25 if l == 2 else 1.0,
                            rden[:].to_broadcast((P, 4, 64)),
                            op0=ALU.mult,
                            op1=ALU.mult,
                        )
                        if l == 2:
                            g64 = wpool.tile([P, 64], F32, tag="g64", bufs=2)
                            nc.vector.tensor_reduce(
                                out=g64[:],
                                in_=gat[:].rearrange("p (h dd) -> p dd h", h=4),
                                axis=mybir.AxisListType.X,
                                op=ALU.add,
                            )
                            zin = g64[:]
                        else:
                            zin = gat[:]
                        zslot = z_all[:, tt, :outd]
                        nc.vector.scalar_tensor_tensor(
                            zslot, zin, 1.0, d["res"][tt],
                            op0=ALU.mult, op1=ALU.add,
                            accum_out=st["s1"][:, tt : tt + 1],
                        )
                        junk = wpool.tile(
                            [P, 256], BF, tag="junk", bufs=1, name="junke"
                        )[:, :outd]
                        nc.scalar.activation(
                            junk, zslot, AF.Square,
                            accum_out=st["s2"][:, tt : tt + 1],
                        )

                for t in range(NT + 2):
                    if t % 2 == 0:
                        if t >= 4:
                            alpha(t // 2 - 2)
                        if t >= 6:
                            tail_pe(t // 2 - 3)
                    else:
                        if t >= 5:
                            mid(t // 2 - 2)
                        if t >= 7:
                            tail_vs(t // 2 - 3)
                    if t < NT:
                        front(t)
                    if 1 <= t <= NT:
                        msg(t - 1)
                    if 2 <= t <= NT + 1:
                        prelu(t - 2)
                NP = NT // 2
                alpha(NP - 1)
                tail_pe(NP - 2)
                mid(NP - 1)
                tail_vs(NP - 2)
                tail_pe(NP - 1)
                tail_vs(NP - 1)

            if DEBUG:
                nc.sync.dma_start(out=dbg_z0[:], in_=z_all[:, :, 0:64])
            with nc.named_scope("f2a0"):
                f2a(0)
            if DEBUG:
                nc.sync.dma_start(out=dbg_ht[:], in_=ht_all[:])
            for l in range(3):
                with nc.named_scope(f"edge{l}"):
                    edge_f1(l)
                    sqrt_batch(l + 1, LW[l]["outd"])
                if DEBUG and l == 0:
                    nc.sync.dma_start(out=dbg_z1[:], in_=z_all[:])
                with nc.named_scope(f"f2a{l + 1}"):
                    f2a(l + 1)

            sig = ppool.tile([P, NT], F32)
            nc.scalar.activation(sig[:], scores[:], AF.Sigmoid, bias=bh2_val)
            nc.sync.dma_start(out=out[:], in_=sig[:])
    return nc


# ---------------------------------------------------------------- host prep
def _balance_nodes(tgt):
    """Degree-balanced assignment of nodes to NCORES*NT tiles of <=128 slots.
    Returns (gtile[node], slot[node], K)."""
    import heapq

    NTILES = NCORES * NT
    deg = np.bincount(tgt, minlength=N)
    order = np.argsort(-deg, kind="stable")
    gtile = np.empty(N, np.int32)
    slot = np.empty(N, np.int32)
    count = np.zeros(NTILES, np.int32)
    load = np.zeros(NTILES, np.int64)
    heap = [(0, t) for t in range(NTILES)]
    heapq.heapify(heap)
    for node in order:
        while True:
            ld, t = heapq.heappop(heap)
            if count[t] < P and ld == load[t]:
                break
        gtile[node] = t
        slot[node] = count[t]
        count[t] += 1
        load[t] += deg[node]
        if count[t] < P:
            heapq.heappush(heap, (int(load[t]), t))
    K = int(np.ceil(load.max() / P))
    return gtile, slot, K


def _prep(inputs):
    ei = np.asarray(inputs["edge_index"]).astype(np.int64)
    src, tgt = ei[0], ei[1]
    ea = np.asarray(inputs["edge_attr"], np.float32)

    gtile, slot, K = _balance_nodes(tgt)
    core_of = gtile // NT
    lt_of = gtile % NT

    lt = lt_of.astype(np.int64)
    chunk = np.searchsorted(np.array(CHT[1:-1]), lt, side="right")
    chrows = np.array(CHROWS)[chunk]
    chbase = np.array(CHBASE)[chunk]
    chtile0 = np.array(CHT[:-1])[chunk]
    row_id = chbase + core_of.astype(np.int64) * chrows + (lt - chtile0) * P + slot

    NTK = NT * K
    ES = NTK * P

    e_core = core_of[tgt]
    e_lt = lt_of[tgt]
    e_p = slot[tgt]  # target's slot within its tile
    order = np.lexsort((e_lt, e_core))
    src_s = src[order]
    ea_s = ea[order]
    e_core_s, e_lt_s, e_p_s = e_core[order], e_lt[order], e_p[order]

    grp = e_core_s * NT + e_lt_s
    idx_in_grp = np.zeros(len(grp), np.int64)
    _, first_pos, cnt = np.unique(grp, return_index=True, return_counts=True)
    for fp, c in zip(first_pos, cnt):
        idx_in_grp[fp : fp + c] = np.arange(c)
    assert cnt.max() <= K * P, (cnt.max(), K)

    src_cols = np.zeros((NCORES, P, NTK), np.int32)
    tgt_cols = np.full((NCORES, P, NTK), -1.0, np.float32)
    tgt_rows = np.full((NCORES, 1, ES), -1.0, np.float32)
    ea_T = np.zeros((NCORES, 16, ES), np.float32)
    eslot = e_lt_s * (K * P) + idx_in_grp
    col = eslot // P
    row = eslot % P
    src_cols[e_core_s, row, col] = row_id[src_s].astype(np.int32)
    tgt_cols[e_core_s, row, col] = e_p_s.astype(np.float32)
    tgt_rows[e_core_s, 0, eslot] = e_p_s.astype(np.float32)
    ea_T[e_core_s[:, None], np.arange(ED)[None, :], eslot[:, None]] = ea_s

    x = np.asarray(inputs["x"], np.float32)
    x_T = np.zeros((NCORES, 384, NPAD), np.float32)
    pos = lt * P + slot  # position within core [0, NPAD)
    x_T[core_of, :FN, pos] = x
    x_T[core_of, FN, pos] = 1.0  # ones-row carries ctx@Wp+bp via wp1

    rep = lambda v: np.broadcast_to(
        np.asarray(v, np.float32)[None, :], (P, len(np.asarray(v)))
    ).copy()
    bf = lambda a: np.asarray(a).astype(ml_dtypes.bfloat16)

    Wp = np.asarray(inputs["Wp"], np.float32)
    cb = (
        np.asarray(inputs["context_vector"], np.float32) @ Wp[FN:]
        + np.asarray(inputs["bp"], np.float32)
    )
    wp1 = np.zeros((384, 64), np.float32)
    wp1[:FN] = Wp[:FN]
    wp1[FN] = cb
    wp1 = wp1.astype(ml_dtypes.bfloat16)

    common = {
        "wp1": wp1,
        "iota2d": np.broadcast_to(
            np.arange(P, dtype=np.float32)[None, :], (P, P)
        ).astype(ml_dtypes.bfloat16),
        "iota_col": np.arange(P, dtype=np.float32)[:, None].copy(),
        "ident": np.eye(P, dtype=np.float32).astype(ml_dtypes.bfloat16),
        "wh1": np.asarray(inputs["Wh1"], np.float32),
        "bh1_rep": rep(inputs["bh1"]),
        "wh2_rep": rep(np.asarray(inputs["Wh2"], np.float32)[:, 0]),
    }
    g_in = np.asarray(inputs["g_in"], np.float32)
    b_in = np.asarray(inputs["b_in"], np.float32)
    for l in range(3):
        sfx = str(l)
        ws = np.asarray(inputs["Ws" + sfx], np.float32)
        wt = np.asarray(inputs["Wt" + sfx], np.float32)
        wswt = np.concatenate([ws, wt], axis=1)
        if l == 0:
            wswt = np.concatenate(
                [g_in[:, None] * wswt, (b_in @ wswt)[None, :]], axis=0
            )
        common[f"wswt{l}"] = bf(wswt)
        we = np.zeros((16, 256), np.float32)
        we[:ED] = np.asarray(inputs["We" + sfx], np.float32)
        common[f"we{l}"] = bf(we)
        a1 = np.asarray(inputs["A" + sfx], np.float32).reshape(-1)
        common[f"a_rep2_{l}"] = bf(rep(np.tile(a1, 2 * K)))
        if l != 1:
            skw = np.asarray(inputs[f"Sk{l}W"], np.float32)
            if l == 0:
                skw = np.concatenate(
                    [
                        g_in[:, None] * skw,
                        (b_in @ skw + np.asarray(inputs["Sk0b"], np.float32))[
                            None, :
                        ],
                    ],
                    axis=0,
                )
            common[f"skw{l}"] = bf(skw)
            common[f"skb_rep{l}"] = bf(rep(inputs[f"Sk{l}b"]))
        common[f"gn_rep{l}"] = bf(rep(inputs["gn" + sfx]))
        common[f"bn_rep{l}"] = bf(rep(inputs["bn" + sfx]))

    in_maps = []
    for c in range(NCORES):
        m = dict(common)
        m["x_T"] = x_T[c].astype(ml_dtypes.bfloat16)
        m["src_c"] = src_cols[c]
        m["tgt_c"] = tgt_cols[c].astype(ml_dtypes.bfloat16)
        m["tgt_r"] = tgt_rows[c].astype(ml_dtypes.bfloat16)
        m["ea_T"] = ea_T[c].astype(ml_dtypes.bfloat16)
        in_maps.append(m)
    bh2_val = float(np.asarray(inputs["bh2"]).reshape(-1)[0])
    return in_maps, K, bh2_val, (core_of, lt_of, slot)


def kernel(**inputs):
    in_maps, K, bh2_val, (core_of, lt_of, slot) = _prep(inputs)
    nc = build_nc(K, bh2_val)
    res = run_bass_kernel_spmd(
        nc, in_maps, core_ids=list(range(NCORES)), trace=TRACE
    )
    LAST_RESULT["exec_time_ns"] = res.exec_time_ns
    LAST_RESULT["res"] = res
    if DEBUG:
        LAST_RESULT["dbg"] = res.results
        LAST_RESULT["layout"] = (core_of, lt_of, slot)
    outs = np.stack([res.results[c]["out"] for c in range(NCORES)])  # [8, P, NT]
    return outs[core_of, slot, lt_of].astype(np.float32)


# revision 33
# speedup vs baseline: 1.0044x; 1.0044x over previous
"""Bass/Trainium2 kernel for nn_MemoryGAT (3-layer GATv2 + MLP head), 8 NeuronCores.

Nodes are degree-balanced into 8x98 tiles of 128 (K edge-tiles per node tile,
K~4). hs rows are written straight into a device-shared hs_full buffer with
batched indirect scatters; a 1-element AllGather acts as the cross-core
barrier (no bulk collective). Edge loop gathers hs[src] in multi-tile batched
indirect DMAs (SWDGE fixed cost amortized), builds the one-hot S / S^T
selection masks on DVE+Pool without PE transposes, accumulates msg in paired
PSUM banks, and keeps LN stats via accum_out. z stays in SBUF end to end.
"""

import sys
import types

sys.path.insert(0, "/opt/trn_rl_repo")

import ml_dtypes
import numpy as np
import orjson

# ---------------------------------------------------------------- shims

_counter = [0]


def _legalize_module(m, maxw=1):
    """This walrus build accepts only ONE sync-wait per instruction; hoist
    overflow waits onto NoOps inserted just before, on the same engine."""
    for f in m.get("functions", []):
        for b in f.get("blocks", []):
            insts = b.get("instructions")
            if not insts:
                continue
            out = []
            for inst in insts:
                si = inst.get("sync_info")
                waits = (si or {}).get("on_wait") or []
                if si is not None and len(waits) > maxw:
                    keep = waits[-maxw:]
                    extra = waits[: len(waits) - maxw]
                    for j in range(0, len(extra), maxw):
                        _counter[0] += 1
                        out.append(
                            {
                                "name": f"ant-wsplit-{_counter[0]}",
                                "opcode": "NoOp",
                                "engine": inst.get("engine"),
                                "ins": [],
                                "outs": [],
                                "sync_info": {
                                    "on_wait": extra[j : j + maxw],
                                    "on_update": [],
                                },
                            }
                        )
                    si["on_wait"] = keep
                out.append(inst)
            b["instructions"] = out
    return m


def _install_shims():
    import antenv

    if "antenv.axon_hooks" not in sys.modules:
        try:
            from trn_agent_boot.trn_boot import _ntff_profile_via_ctypes

            hooks = types.ModuleType("antenv.axon_hooks")
            hook = _ntff_profile_via_ctypes("/opt/axon/libaxon_pjrt.so")
            hooks.get_axon_ntff_profile_hook = lambda: hook
            hooks.set_axon_ntff_profile_hook = lambda h: None
            sys.modules["antenv.axon_hooks"] = hooks
            antenv.axon_hooks = hooks
        except Exception:
            pass

    import concourse.bass as bass
    from concourse import bass_utils

    bass_utils.upload_artifacts = lambda tmpdir: tmpdir

    if not getattr(bass.Bass, "_waitfix_installed", False):
        base = bass.Bass.to_json_bytes

        def patched(self):
            return orjson.dumps(_legalize_module(orjson.loads(base(self))))

        bass.Bass.to_json_bytes = patched
        bass.Bass._waitfix_installed = True


_install_shims()

import concourse.bass as bass
import concourse.tile as tile
from concourse import mybir
from concourse.bass_utils import run_bass_kernel_spmd

F32 = mybir.dt.float32
BF = mybir.dt.bfloat16
AF = mybir.ActivationFunctionType
ALU = mybir.AluOpType

# ---------------------------------------------------------------- sizes
N = 100_000
E = 400_000
FN = 267
DC = 256
H, D = 4, 64
HD = 256
ED = 11
NCORES = 8
P = 128
NT = 98
NPAD = NT * P  # 12544
NFULL = NCORES * NPAD
# AllGather chunk boundaries (in node tiles) and hs_full region bases
CHT = [0, 40, 72, 92, 98]
NCH = len(CHT) - 1
CHROWS = [(CHT[i + 1] - CHT[i]) * P for i in range(NCH)]
CHBASE = [0]
for i in range(NCH - 1):
    CHBASE.append(CHBASE[-1] + NCORES * CHROWS[i])

TRACE = False
DEBUG = False
LAST_RESULT = {}


# ---------------------------------------------------------------- builder
def build_nc(K, bh2_val):
    NTK = NT * K
    ES = NTK * P
    KP = K * P

    nc = bass.Bass()
    dp = nc.declare_dram_parameter

    x_T = dp("x_T", [384, NPAD], BF, isOutput=False)
    src_c = dp("src_c", [P, NTK], mybir.dt.int32, isOutput=False)
    tgt_c = dp("tgt_c", [P, NTK], BF, isOutput=False)
    tgt_r = dp("tgt_r", [1, ES], BF, isOutput=False)
    ea_T = dp("ea_T", [16, ES], BF, isOutput=False)
    wp1 = dp("wp1", [384, 64], BF, isOutput=False)
    iota2d = dp("iota2d", [P, P], BF, isOutput=False)
    iota_col = dp("iota_col", [P, 1], F32, isOutput=False)
    ident = dp("ident", [P, P], BF, isOutput=False)
    wh1 = dp("wh1", [64, 32], F32, isOutput=False)
    bh1_rep = dp("bh1_rep", [P, 32], F32, isOutput=False)
    wh2_rep = dp("wh2_rep", [P, 32], F32, isOutput=False)

    LW = []
    for l, ind in ((0, 65), (1, 256), (2, 256)):
        d = {"ind": ind, "outd": 64 if l == 2 else 256}
        d["wswt"] = dp(f"wswt{l}", [ind, 512], BF, isOutput=False)
        d["we"] = dp(f"we{l}", [16, 256], BF, isOutput=False)
        d["a_rep2"] = dp(f"a_rep2_{l}", [P, 2 * KP * 2], BF, isOutput=False)
        if l != 1:
            d["skw"] = dp(f"skw{l}", [ind, d["outd"]], BF, isOutput=False)
            d["skb_rep"] = dp(f"skb_rep{l}", [P, d["outd"]], BF, isOutput=False)
        d["gn_rep"] = dp(f"gn_rep{l}", [P, d["outd"]], BF, isOutput=False)
        d["bn_rep"] = dp(f"bn_rep{l}", [P, d["outd"]], BF, isOutput=False)
        LW.append(d)

    out = dp("out", [P, NT], F32, isOutput=True)
    if DEBUG:
        dbg_z0 = dp("dbg_z0", [P, NT, 64], BF, isOutput=True)
        dbg_ht = dp("dbg_ht", [P, NT * 256], BF, isOutput=True)
        dbg_z1 = dp("dbg_z1", [P, NT, 256], BF, isOutput=True)
        dbg_lr = dp("dbg_lr", [P, 2, 1024], BF, isOutput=True)
        dbg_st = dp("dbg_st", [P, 512], BF, isOutput=True)

    hs_shard = [nc.dram_tensor(f"hs_shard{l}", [NPAD, 256], BF) for l in range(3)]
    hs_full = [
        nc.dram_tensor(f"hs_full{l}", [NFULL, 256], BF, addr_space="Shared")
        for l in range(3)
    ]
    res0_dram = nc.dram_tensor("res0_dram", [NPAD, 256], BF)
    h1_dram = nc.dram_tensor("h1_dram", [NPAD, 256], BF)

    with tile.TileContext(nc) as tc:
        with (
            tc.tile_pool(name="const", bufs=1) as cpool,
            tc.tile_pool(name="work", bufs=2) as wpool,
            tc.tile_pool(name="small", bufs=2) as spool,
            tc.tile_pool(name="persist", bufs=1) as ppool,
            tc.tile_pool(name="psPair", bufs=4, space="PSUM") as psPair,
            tc.tile_pool(name="psB", bufs=2, space="PSUM") as psB,
            tc.tile_pool(name="psC", bufs=2, space="PSUM") as psC,
        ):
            for v in {1e-5, 1e-8, float(bh2_val)}:
                ct = cpool.tile([P, 1], F32, tag=f"k{v}", name=f"k{_counter[0]}")
                _counter[0] += 1
                nc.vector.memset(ct[:], v)
                nc.const_aps.aps[(F32, float(v))] = ct[:]

            _cn = [0]

            def c_load(ap, shape, dt=F32):
                _cn[0] += 1
                t = cpool.tile(shape, dt, tag=f"c{_cn[0]}", name=f"c{_cn[0]}")
                nc.sync.dma_start(out=t[:], in_=ap[:])
                return t

            def c_load_chunks(ap, kk, ck, n, dt=F32):
                _cn[0] += 1
                t = cpool.tile([kk, ck * n], dt, tag=f"c{_cn[0]}", name=f"c{_cn[0]}")
                for c in range(ck):
                    nc.sync.dma_start(
                        out=t[:, c * n : (c + 1) * n],
                        in_=ap[c * kk : (c + 1) * kk, :],
                    )
                return t

            iota_sb = c_load(iota2d, [P, P], BF)
            idb_sb = c_load(ident, [P, P], BF)
            iotac_sb = c_load(iota_col, [P, 1], F32)
            iotaK_sb = cpool.tile([P, KP], BF, tag="iotaK", name="iotaK")
            for k in range(K):
                nc.vector.tensor_copy(iotaK_sb[:, k * P : (k + 1) * P], iota_sb[:])
            ones1p = cpool.tile([1, P], BF, tag="ones1p", name="ones1p")
            nc.vector.memset(ones1p[:], 1.0)
            wp1_sb = c_load_chunks(wp1, P, 3, 64, BF)
            wh1_sb = c_load(wh1, [64, 32])
            bh1_sb = c_load(bh1_rep, [P, 32])
            wh2_sb = c_load(wh2_rep, [P, 32])
            lws = []
            for l, d in enumerate(LW):
                s = {}
                ck = max(d["ind"] // P, 1)
                kk = min(d["ind"], P)
                s["wswt"] = c_load_chunks(d["wswt"], kk, ck, 512, BF)
                s["we"] = c_load(d["we"], [16, 256], BF)
                if "skw" in d:
                    s["skw"] = c_load_chunks(d["skw"], kk, ck, d["outd"], BF)
                    s["skb"] = c_load(d["skb_rep"], [P, d["outd"]], BF)
                s["gn"] = c_load(d["gn_rep"], [P, d["outd"]], BF)
                s["bn"] = c_load(d["bn_rep"], [P, d["outd"]], BF)
                s["ck"], s["kk"] = ck, kk
                lws.append(s)

            srcs = ppool.tile([P, NTK], mybir.dt.int32)
            nc.sync.dma_start(out=srcs[:], in_=src_c[:])
            tgts = ppool.tile([P, NTK], BF)
            nc.sync.dma_start(out=tgts[:], in_=tgt_c[:])

            ht_all = ppool.tile([P, NT * 256], BF)
            z_all = ppool.tile([P, NT, 256], BF)
            res2_all = ppool.tile([P, NT * 64], BF)
            scores = ppool.tile([P, NT], F32)

            # one shared LN-stat set; stages are strictly phased so WAR
            # deps keep this safe
            _st = {}
            for nm in ("s1", "s2", "m", "va", "rstd"):
                _st[nm] = ppool.tile([P, NT], F32, tag=f"st{nm}", name=f"st{nm}")
            stats = [_st] * 4

            def sqrt_batch(i, dim, t0=0, t1=NT):
                st = stats[i]
                sl = slice(t0, t1)
                nc.vector.tensor_scalar_mul(st["m"][:, sl], st["s1"][:, sl], 1.0 / dim)
                nc.vector.tensor_scalar_mul(st["va"][:, sl], st["s2"][:, sl], 1.0 / dim)
                nm2 = spool.tile([P, NT], F32, tag="nm2", name="nm2")[:, sl]
                nc.vector.scalar_tensor_tensor(
                    nm2, st["m"][:, sl], -1.0, st["m"][:, sl],
                    op0=ALU.mult, op1=ALU.mult,
                )
                nc.vector.tensor_add(st["va"][:, sl], st["va"][:, sl], nm2)
                sd = spool.tile([P, NT], F32, tag="sd", name="sd")[:, sl]
                nc.scalar.activation(sd, st["va"][:, sl], AF.Sqrt, bias=1e-5)
                nc.vector.reciprocal(st["rstd"][:, sl], sd)
                nc.vector.scalar_tensor_tensor(
                    st["va"][:, sl], st["m"][:, sl], -1.0, st["rstd"][:, sl],
                    op0=ALU.mult, op1=ALU.mult,
                )

            def ag_chunk(l, c):
                nc.gpsimd.collective_compute(
                    "AllGather",
                    ALU.bypass,
                    ins=[hs_shard[l][CHT[c] * P : CHT[c + 1] * P, :]],
                    outs=[
                        hs_full[l][CHBASE[c] : CHBASE[c] + NCORES * CHROWS[c], :]
                    ],
                    replica_groups=[list(range(NCORES))],
                )

            # ---------------- fused phase 0 + f2a0 pipeline -------------------
            # p0(t): x@Wp -> gelu -> z0, stats; sqrt per 4-block;
            # f2a0 stages trail: hn(t-6) | transpose(t-7) | proj(t-8) | copies(t-9)
            def p0_f2a0():
                st = stats[0]
                s = lws[0]
                hns = {}
                lhss = {}
                hshts = {}
                rps = {}
                stag_hs = [None]
                stag_res = [None]
                LAG = 6
                for step in range(NT + LAG + 3):
                    if step < NT:
                        t = step
                        if t % 4 == 0:
                            nbt = min(4, NT - t)
                            xt = wpool.tile([P, 3, 4 * P], BF, tag="hsg", bufs=3)
                            for c in range(3):
                                nc.sync.dma_start(
                                    out=xt[:, c, : nbt * P],
                                    in_=x_T[
                                        c * P : (c + 1) * P, t * P : (t + nbt) * P
                                    ],
                                )
                        xoff = (t % 4) * P
                        h0p = psPair.tile([P, 512], F32, tag="pair", name="h0p")[:, 0:64]
                        for c in range(3):
                            nc.tensor.matmul(
                                out=h0p,
                                lhsT=xt[:, c, xoff : xoff + P],
                                rhs=wp1_sb[:, c * 64 : (c + 1) * 64],
                                start=(c == 0),
                                stop=(c == 2),
                            )
                        zsl = z_all[:, t, 0:64]
                        nc.scalar.activation(
                            zsl, h0p, AF.Gelu, accum_out=st["s1"][:, t : t + 1]
                        )
                        junk = wpool.tile([P, 256], BF, tag="junk", bufs=1, name="junk0")[:, 0:64]
                        nc.vector.scalar_tensor_tensor(
                            junk, zsl, 1.0, zsl,
                            op0=ALU.mult, op1=ALU.mult,
                            accum_out=st["s2"][:, t : t + 1],
                        )
                        if t % 4 == 3 or t == NT - 1:
                            sqrt_batch(0, 64, t - (t % 4), t + 1)
                    # stage A: hn(t) via identity
                    tA = step - LAG
                    if 0 <= tA < NT:
                        hn = wpool.tile([P, 256], BF, tag="hn", bufs=4, name="hn0")[:, 0:64]
                        nc.scalar.activation(
                            hn, z_all[:, tA, :64], AF.Identity,
                            bias=st["va"][:, tA : tA + 1],
                            scale=st["rstd"][:, tA : tA + 1],
                        )
                        hns[tA] = hn
                    # stage B: transpose(t-LAG-1)
                    tB = step - LAG - 1
                    if 0 <= tB < NT:
                        hn = hns.pop(tB)
                        h0t = wpool.tile([65, P], BF, tag="h0t", bufs=4)
                        if tB < 4:
                            nc.vector.memset(h0t[64:65, :], 1.0)
                        trp = psB.tile([P, 2, P], BF, tag="tr")
                        nc.tensor.transpose(
                            out=trp[0:64, 0, :], in_=hn, identity=idb_sb[:]
                        )
                        nc.vector.tensor_copy(h0t[0:64, :], trp[0:64, 0, :])
                        lhss[tB] = h0t
                    # stage C: projections(t-LAG-2)
                    tC = step - LAG - 2
                    if 0 <= tC < NT:
                        h0t = lhss.pop(tC)
                        hsht = psPair.tile([P, 512], F32, tag="pair", name="hsht")
                        nc.tensor.matmul(
                            out=hsht[:], lhsT=h0t[:, :], rhs=s["wswt"][:, 0:512],
                            start=True, stop=True,
                        )
                        hshts[tC] = hsht
                        rp = psC.tile([P, 512], F32, tag="agg", name="rp")[:, 0:256]
                        nc.tensor.matmul(
                            out=rp, lhsT=h0t[:, :], rhs=s["skw"][:, 0:256],
                            start=True, stop=True,
                        )
                        rps[tC] = rp
                    # stage D: copies + stores(t-LAG-3)
                    tD = step - LAG - 3
                    if 0 <= tD < NT:
                        t = tD
                        if t % 4 == 0:
                            stag_hs[0] = wpool.tile(
                                [P, 4, 256], BF, tag="stag_hs", bufs=2, name="shs"
                            )
                            stag_res[0] = wpool.tile(
                                [P, 4, 256], BF, tag="r4x256", bufs=3, name="sres"
                            )
                        hsht = hshts.pop(t)
                        nc.scalar.copy(stag_hs[0][:, t % 4, :], hsht[:, 0:256])
                        nc.scalar.copy(
                            ht_all[:, t * 256 : (t + 1) * 256], hsht[:, 256:512]
                        )
                        rp = rps.pop(t)
                        nc.scalar.copy(stag_res[0][:, t % 4, :], rp)
                        if t % 4 == 3 or t == NT - 1:
                            t0 = t - (t % 4)
                            nbt = t - t0 + 1
                            nc.sync.dma_start(
                                out=hs_shard[0][:].rearrange("(t p) c -> p t c", p=P)[
                                    :, t0 : t0 + nbt, :
                                ],
                                in_=stag_hs[0][:, :nbt, :],
                            )
                            nc.sync.dma_start(
                                out=res0_dram[:].rearrange("(t p) c -> p t c", p=P)[
                                    :, t0 : t0 + nbt, :
                                ],
                                in_=stag_res[0][:, :nbt, :],
                            )
                        for c in range(NCH):
                            if t == CHT[c + 1] - 1:
                                ag_chunk(0, c)

            with nc.named_scope("p0"):
                p0_f2a0()

            # ---------------- F2A(l): finalize h_l, project, scatter+barrier
            # Software-pipelined: hn(t) | transpose(t-1) | proj+copies(t-2)
            def f2a(l):
                st = stats[l]
                ind = 64 if l == 0 else (256 if l < 3 else 64)
                s = lws[l] if l < 3 else None
                hns = {}
                lhss = {}
                h3Ts = {}
                hshts = {}
                rps = {}
                stag_h1 = None
                stag_hs = None
                stag_res = None
                for step in range(NT + 3):
                    # ---- stage A: produce hn(step)
                    if step < NT:
                        t = step
                        if l == 1 and t % 4 == 0:
                            stag_h1 = wpool.tile(
                                [P, 4, 256], BF, tag="r4x256", bufs=3, name="sh1"
                            )
                        if l == 0:
                            hn = wpool.tile(
                                [P, 256], BF, tag="hn", bufs=4, name="hn0"
                            )[:, :ind]
                            nc.scalar.activation(
                                hn, z_all[:, t, :ind], AF.Identity,
                                bias=st["va"][:, t : t + 1],
                                scale=st["rstd"][:, t : t + 1],
                            )
                        else:
                            if l == 1:
                                hn = stag_h1[:, t % 4, :]
                            else:
                                hn = wpool.tile(
                                    [P, 256], BF, tag="hn", bufs=4, name="hnl"
                                )[:, :ind]
                            g_sb = lws[l - 1]["gn"]
                            b_sb = lws[l - 1]["bn"]
                            t1 = wpool.tile(
                                [P, 256], F32, tag="t1", bufs=1, name="t1"
                            )[:, :ind]
                            nc.vector.scalar_tensor_tensor(
                                t1, z_all[:, t, :ind], st["m"][:, t : t + 1],
                                g_sb[:, :ind], op0=ALU.subtract, op1=ALU.mult,
                            )
                            u = wpool.tile(
                                [P, 256], F32, tag="u", bufs=2, name="u"
                            )[:, :ind]
                            nc.vector.scalar_tensor_tensor(
                                u, t1, st["rstd"][:, t : t + 1], b_sb[:, :ind],
                                op0=ALU.mult, op1=ALU.add,
                            )
                            nc.scalar.activation(hn, u, AF.Gelu)
                        hns[t] = hn
                        if l == 1 and (t % 4 == 3 or t == NT - 1):
                            t0 = t - (t % 4)
                            nc.sync.dma_start(
                                out=h1_dram[:].rearrange("(t p) c -> p t c", p=P)[
                                    :, t0 : t + 1, :
                                ],
                                in_=stag_h1[:, : t - t0 + 1, :],
                            )
                    # ---- stage B: transpose hn(step-1)
                    if 1 <= step <= NT:
                        t = step - 1
                        hn = hns[t]
                        if l == 3:
                            trp = psB.tile([P, 2, P], BF, tag="tr")
                            nc.tensor.transpose(
                                out=trp[0:64, 0, :], in_=hn, identity=idb_sb[:]
                            )
                            h3T = wpool.tile([64, P], F32, tag="h3T", bufs=3)
                            nc.scalar.copy(h3T[:], trp[0:64, 0, :])
                            h3Ts[t] = h3T
                        elif l == 0:
                            h0t = wpool.tile([65, P], BF, tag="h0t", bufs=4)
                            if t < 4:
                                nc.vector.memset(h0t[64:65, :], 1.0)
                            trp = psB.tile([P, 2, P], BF, tag="tr")
                            nc.tensor.transpose(
                                out=trp[0:64, 0, :], in_=hn, identity=idb_sb[:]
                            )
                            nc.vector.tensor_copy(h0t[0:64, :], trp[0:64, 0, :])
                            lhss[t] = [h0t[:, :]]
                        else:
                            trp = psB.tile([P, 2, P], BF, tag="tr")
                            for c in range(2):
                                nc.tensor.transpose(
                                    out=trp[:, c, :],
                                    in_=hn[:, c * P : (c + 1) * P],
                                    identity=idb_sb[:],
                                )
                            hnT = wpool.tile([P, 2, P], BF, tag="hnT", bufs=4)
                            nc.vector.tensor_copy(hnT[:], trp[:])
                            lhss[t] = [hnT[:, c, :] for c in range(2)]
                    # ---- stage C: project for tile step-2
                    if 2 <= step <= NT + 1:
                        t = step - 2
                        if l == 3:
                            pass
                        else:
                            lhs = lhss.pop(t)
                            hns.pop(t, None)
                            ck = s["ck"]
                            hsht = psPair.tile([P, 512], F32, tag="pair", name="hsht")
                            for c in range(ck):
                                nc.tensor.matmul(
                                    out=hsht[:],
                                    lhsT=lhs[c],
                                    rhs=s["wswt"][:, c * 512 : (c + 1) * 512],
                                    start=(c == 0),
                                    stop=(c == ck - 1),
                                )
                            hshts[t] = hsht
                            if l != 1:
                                outd = LW[l]["outd"]
                                rp = psC.tile([P, 512], F32, tag="agg", name="rp")[
                                    :, :outd
                                ]
                                for c in range(ck):
                                    nc.tensor.matmul(
                                        out=rp,
                                        lhsT=lhs[c],
                                        rhs=s["skw"][:, c * outd : (c + 1) * outd],
                                        start=(c == 0),
                                        stop=(c == ck - 1),
                                    )
                                rps[t] = rp
                    # ---- stage D: copies + stores for tile step-3
                    if step < 3:
                        continue
                    t = step - 3
                    if l == 3:
                        h3T = h3Ts.pop(t)
                        sp1 = psC.tile([P, 512], F32, tag="agg", name="sp1")[:, :32]
                        nc.tensor.matmul(
                            out=sp1, lhsT=h3T[:], rhs=wh1_sb[:], start=True, stop=True
                        )
                        u1 = wpool.tile([P, 32], F32, tag="u1", bufs=2)
                        nc.vector.tensor_add(u1[:], sp1, bh1_sb[:])
                        g1 = wpool.tile([P, 32], F32, tag="g1", bufs=2)
                        nc.scalar.activation(g1[:], u1[:], AF.Gelu)
                        j32 = wpool.tile([P, 32], BF, tag="j32", bufs=2)
                        nc.vector.scalar_tensor_tensor(
                            j32[:], g1[:], 1.0, wh2_sb[:],
                            op0=ALU.mult, op1=ALU.mult,
                            accum_out=scores[:, t : t + 1],
                        )
                        hns.pop(t, None)
                        continue
                    if t % 4 == 0:
                        stag_hs = wpool.tile(
                            [P, 4, 256], BF, tag="stag_hs", bufs=2, name="shs"
                        )
                        if l == 0:
                            stag_res = wpool.tile(
                                [P, 4, 256], BF, tag="r4x256", bufs=3, name="sres"
                            )
                    hsht = hshts.pop(t)
                    nc.scalar.copy(stag_hs[:, t % 4, :], hsht[:, 0:256])
                    nc.scalar.copy(
                        ht_all[:, t * 256 : (t + 1) * 256], hsht[:, 256:512]
                    )
                    if l != 1:
                        rp = rps.pop(t)
                        if l == 0:
                            nc.scalar.copy(stag_res[:, t % 4, :], rp)
                        else:
                            nc.vector.scalar_tensor_tensor(
                                res2_all[:, t * 64 : (t + 1) * 64], rp, 1.0,
                                s["skb"][:], op0=ALU.mult, op1=ALU.add,
                            )
                    # batched stores + AG chunks
                    if t % 4 == 3 or t == NT - 1:
                        t0 = t - (t % 4)
                        nbt = t - t0 + 1
                        nc.sync.dma_start(
                            out=hs_shard[l][:].rearrange("(t p) c -> p t c", p=P)[
                                :, t0 : t0 + nbt, :
                            ],
                            in_=stag_hs[:, :nbt, :],
                        )
                        if l == 0:
                            nc.sync.dma_start(
                                out=res0_dram[:].rearrange("(t p) c -> p t c", p=P)[
                                    :, t0 : t0 + nbt, :
                                ],
                                in_=stag_res[:, :nbt, :],
                            )
                    for c in range(NCH):
                        if t == CHT[c + 1] - 1:
                            ag_chunk(l, c)

            # ---------------- edge + F1 loop --------------------------------
            # Deep pipeline: every cross-engine dep is >=1 tile old.
            #  front(t):   loads, 4 gathers(t) [Pool], ST/S masks(t) [V]
            #  msg(t-1):   10 matmuls [PE] + 2 Prelu(t-2) [S]
            #  alpha(j):   at t=2j+4: scr2/alph2 [V], exp [S]
            #  mid(j):     at t=2j+5: w4 [V], w4col [S]
            #  tail_pe(j): at t=2j+6: agg [PE], den [S]
            #  tail_vs(j): at t=2j+7: rden/gat/z [V], square [S]
            def edge_f1(l):
                s = lws[l]
                outd = LW[l]["outd"]
                st = stats[l + 1]
                a2_sb = wpool.tile(
                    [P, 2 * KP * 2], BF, tag="arep", bufs=1, name=f"arep{l}"
                )
                nc.sync.dma_start(out=a2_sb[:], in_=LW[l]["a_rep2"][:])
                info = {}
                tinfo = {}
                res_sb = [None]

                def front(t):
                    j = t // 2
                    if t % 2 == 0:
                        d = {"S": {}, "res": {}, "msgp": {}}
                        info[j] = d
                        d["hsg"] = wpool.tile(
                            [P, 2 * K * 256], BF, tag="hsg", bufs=3, name="hsg"
                        )
                        d["lr2"] = wpool.tile(
                            [P, 2, 1024], BF, tag="lr2", bufs=2, name="lr2"
                        )
                        ea_sb = wpool.tile([16, 2 * KP], BF, tag="ea", bufs=2)
                        nc.sync.dma_start(
                            out=ea_sb[:], in_=ea_T[:, t * KP : (t + 2) * KP]
                        )
                        tr_sb = wpool.tile([P, 2 * KP], BF, tag="tgtr", bufs=2)
                        nc.sync.dma_start(
                            out=tr_sb[:],
                            in_=tgt_r[0:1, t * KP : (t + 2) * KP].to_broadcast(
                                (P, 2 * KP)
                            ),
                        )
                        d["ea"], d["tr"] = ea_sb, tr_sb
                    d = info[j]
                    if l < 2:
                        if t % 4 == 0:
                            nbr = min(4, NT - t)
                            res_sb[0] = wpool.tile(
                                [P, 4, 256], BF, tag="r4x256", bufs=3, name="res_sb"
                            )
                            rdram = res0_dram if l == 0 else h1_dram
                            nc.sync.dma_start(
                                out=res_sb[0][:, :nbr, :],
                                in_=rdram[:].rearrange("(t p) c -> p t c", p=P)[
                                    :, t : t + nbr, :
                                ],
                            )
                        d["res"][t] = res_sb[0][:, t % 4, :]
                    else:
                        d["res"][t] = res2_all[:, t * 64 : (t + 1) * 64]
                    for k in range(K):
                        nc.gpsimd.indirect_dma_start(
                            out=d["hsg"][
                                :, ((t % 2) * K + k) * 256 : ((t % 2) * K + k + 1) * 256
                            ],
                            out_offset=None,
                            in_=hs_full[l][:],
                            in_offset=bass.IndirectOffsetOnAxis(
                                ap=srcs[:, t * K + k : t * K + k + 1], axis=0
                            ),
                        )
                    eoff = (t % 2) * KP
                    ST_all = wpool.tile([P, KP], BF, tag="ST", bufs=3)
                    nc.vector.tensor_scalar(
                        ST_all[:], d["tr"][:, eoff : eoff + KP], iotac_sb[:, 0:1],
                        None, op0=ALU.is_equal,
                    )
                    S_all = wpool.tile([P, KP], BF, tag="S", bufs=7)
                    nc.vector.tensor_tensor(
                        out=S_all[:].rearrange("p (k c) -> p k c", k=K),
                        in0=iotaK_sb[:].rearrange("p (k c) -> p k c", k=K),
                        in1=tgts[:, t * K : (t + 1) * K].to_broadcast((P, K, P)),
                        op=ALU.is_equal,
                    )
                    d["S"][t] = S_all
                    tinfo[t] = (ST_all, d)

                def msg(t):
                    ST_all, d = tinfo.pop(t)
                    j = t // 2
                    eoff = (t % 2) * KP
                    d["msgp"][t] = []
                    for jj in range(2):
                        msgp = psPair.tile([P, 512], F32, tag="pair", name="msgp")
                        cb = ((t % 2) * K + 2 * jj) * 256
                        nc.tensor.matmul(
                            out=msgp[:], lhsT=idb_sb[:],
                            rhs=d["hsg"][:, cb : cb + 512],
                            start=True, stop=False, skip_group_check=True,
                        )
                        for c in range(2):
                            k = 2 * jj + c
                            nc.tensor.matmul(
                                out=msgp[:, c * 256 : (c + 1) * 256],
                                lhsT=d["ea"][:, eoff + k * P : eoff + (k + 1) * P],
                                rhs=s["we"][:],
                                start=False, stop=False, skip_group_check=True,
                            )
                        for c in range(2):
                            k = 2 * jj + c
                            nc.tensor.matmul(
                                out=msgp[:, c * 256 : (c + 1) * 256],
                                lhsT=ST_all[:, k * P : (k + 1) * P],
                                rhs=ht_all[:, t * 256 : (t + 1) * 256],
                                start=False, stop=(c == 1), skip_group_check=True,
                            )
                        d["msgp"][t].append(msgp)

                def prelu(t):
                    j = t // 2
                    d = info[j]
                    for jj in range(2):
                        nc.scalar.activation(
                            d["lr2"][:, t % 2, jj * 512 : (jj + 1) * 512],
                            d["msgp"][t][jj][:],
                            AF.Prelu, alpha=0.2,
                        )
                    del d["msgp"][t]

                def alpha(j):
                    d = info[j]
                    scr2 = wpool.tile([P, 2048], BF, tag="scr2", bufs=1)
                    nc.vector.tensor_tensor(
                        out=scr2[:],
                        in0=d["lr2"][:].rearrange("p a b -> p (a b)"),
                        in1=a2_sb[:],
                        op=ALU.mult,
                    )
                    alph2 = spool.tile([P, 32], F32, tag="alph", bufs=2)
                    nc.vector.tensor_reduce(
                        out=alph2[:],
                        in_=scr2[:].rearrange("p (g d) -> p g d", d=64),
                        axis=mybir.AxisListType.X,
                        op=ALU.add,
                    )
                    expa2 = spool.tile([P, 32], F32, tag="expa", bufs=2)
                    nc.scalar.activation(expa2[:], alph2[:], AF.Exp)
                    d["expa"] = expa2

                def mid(j):
                    d = info[j]
                    d["waug"] = []
                    for i in range(2):
                        w_aug = wpool.tile(
                            [P, K * 260], BF, tag="waug", bufs=2, name="waug"
                        )
                        w4 = w_aug[:].rearrange("p (k h c) -> p k h c", k=K, c=65)
                        hs4 = d["hsg"][
                            :, i * K * 256 : (i + 1) * K * 256
                        ].rearrange("p (k h dd) -> p k h dd", k=K, dd=64)
                        e4 = d["expa"][:, i * 16 : (i + 1) * 16].rearrange(
                            "p (k h) -> p k h", k=K
                        )
                        nc.vector.tensor_tensor(
                            out=w4[:, :, :, 0:64],
                            in0=hs4[:],
                            in1=e4[:].to_broadcast((P, K, H, 64)),
                            op=ALU.mult,
                        )
                        nc.scalar.copy(w4[:, :, :, 64], e4)
                        d["waug"].append(w_aug)

                def tail_pe(j):
                    d = info[j]
                    d["agg"] = []
                    d["den"] = []
                    for i in range(2):
                        agg = psC.tile([P, 512], F32, tag="agg", name="agg")[:, 0:260]
                        for k in range(K):
                            nc.tensor.matmul(
                                out=agg,
                                lhsT=d["S"][2 * j + i][:, k * P : (k + 1) * P],
                                rhs=d["waug"][i][:, k * 260 : (k + 1) * 260],
                                start=(k == 0),
                                stop=(k == K - 1),
                            )
                        d["agg"].append(agg)
                    for i in range(2):
                        aggv = d["agg"][i].rearrange("p (h c) -> p h c", c=65)
                        den = spool.tile([P, 4], F32, tag="den", bufs=4)
                        nc.scalar.activation(den[:], aggv[:, :, 64], AF.Copy, bias=1e-8)
                        d["den"].append(den)

                def tail_vs(j):
                    d = info.pop(j)
                    for i in range(2):
                        tt = 2 * j + i
                        aggv = d["agg"][i].rearrange("p (h c) -> p h c", c=65)
                        rden = spool.tile([P, 4], F32, tag="rden", bufs=2)
                        nc.vector.reciprocal(rden[:], d["den"][i][:])
                        gat = wpool.tile([P, 256], F32, tag="gat", bufs=1)
                        nc.vector.scalar_tensor_tensor(
                            gat[:].rearrange("p (h dd) -> p h dd", h=4),
                            aggv[:, :, 0:64],
                            0.25 if l == 2 else 1.0,
                            rden[:].to_broadcast((P, 4, 64)),
                            op0=ALU.mult,
                            op1=ALU.mult,
                        )
                        if l == 2:
                            g64 = wpool.tile([P, 64], F32, tag="g64", bufs=2)
                            nc.vector.tensor_reduce(
                                out=g64[:],
                                in_=gat[:].rearrange("p (h dd) -> p dd h", h=4),
                                axis=mybir.AxisListType.X,
                                op=ALU.add,
                            )
                            zin = g64[:]
                        else:
                            zin = gat[:]
                        zslot = z_all[:, tt, :outd]
                        nc.vector.scalar_tensor_tensor(
                            zslot, zin, 1.0, d["res"][tt],
                            op0=ALU.mult, op1=ALU.add,
                            accum_out=st["s1"][:, tt : tt + 1],
                        )
                        junk = wpool.tile(
                            [P, 256], BF, tag="junk", bufs=1, name="junke"
                        )[:, :outd]
                        nc.scalar.activation(
                            junk, zslot, AF.Square,
                            accum_out=st["s2"][:, tt : tt + 1],
                        )

                for t in range(NT + 2):
                    if t % 2 == 0:
                        if t >= 4:
                            alpha(t // 2 - 2)
                        if t >= 6:
                            tail_pe(t // 2 - 3)
                    else:
                        if t >= 5:
                            mid(t // 2 - 2)
                        if t >= 7:
                            tail_vs(t // 2 - 3)
                    if t < NT:
                        front(t)
                    if 1 <= t <= NT:
                        msg(t - 1)
                    if 2 <= t <= NT + 1:
                        prelu(t - 2)
                NP = NT // 2
                alpha(NP - 1)
                tail_pe(NP - 2)
                mid(NP - 1)
                tail_vs(NP - 2)
                tail_pe(NP - 1)
                tail_vs(NP - 1)

            if DEBUG:
                nc.sync.dma_start(out=dbg_z0[:], in_=z_all[:, :, 0:64])
            if DEBUG:
                nc.sync.dma_start(out=dbg_ht[:], in_=ht_all[:])
            for l in range(3):
                with nc.named_scope(f"edge{l}"):
                    edge_f1(l)
                    sqrt_batch(l + 1, LW[l]["outd"])
                if DEBUG and l == 0:
                    nc.sync.dma_start(out=dbg_z1[:], in_=z_all[:])
                with nc.named_scope(f"f2a{l + 1}"):
                    f2a(l + 1)

            sig = ppool.tile([P, NT], F32)
            nc.scalar.activation(sig[:], scores[:], AF.Sigmoid, bias=bh2_val)
            nc.sync.dma_start(out=out[:], in_=sig[:])
    return nc


# ---------------------------------------------------------------- host prep
def _balance_nodes(tgt):
    """Degree-balanced assignment of nodes to NCORES*NT tiles of <=128 slots.
    Returns (gtile[node], slot[node], K)."""
    import heapq

    NTILES = NCORES * NT
    deg = np.bincount(tgt, minlength=N)
    order = np.argsort(-deg, kind="stable")
    gtile = np.empty(N, np.int32)
    slot = np.empty(N, np.int32)
    count = np.zeros(NTILES, np.int32)
    load = np.zeros(NTILES, np.int64)
    heap = [(0, t) for t in range(NTILES)]
    heapq.heapify(heap)
    for node in order:
        while True:
            ld, t = heapq.heappop(heap)
            if count[t] < P and ld == load[t]:
                break
        gtile[node] = t
        slot[node] = count[t]
        count[t] += 1
        load[t] += deg[node]
        if count[t] < P:
            heapq.heappush(heap, (int(load[t]), t))
    K = int(np.ceil(load.max() / P))
    return gtile, slot, K


def _prep(inputs):
    ei = np.asarray(inputs["edge_index"]).astype(np.int64)
    src, tgt = ei[0], ei[1]
    ea = np.asarray(inputs["edge_attr"], np.float32)

    gtile, slot, K = _balance_nodes(tgt)
    core_of = gtile // NT
    lt_of = gtile % NT

    lt = lt_of.astype(np.int64)
    chunk = np.searchsorted(np.array(CHT[1:-1]), lt, side="right")
    chrows = np.array(CHROWS)[chunk]
    chbase = np.array(CHBASE)[chunk]
    chtile0 = np.array(CHT[:-1])[chunk]
    row_id = chbase + core_of.astype(np.int64) * chrows + (lt - chtile0) * P + slot

    NTK = NT * K
    ES = NTK * P

    e_core = core_of[tgt]
    e_lt = lt_of[tgt]
    e_p = slot[tgt]  # target's slot within its tile
    order = np.lexsort((e_lt, e_core))
    src_s = src[order]
    ea_s = ea[order]
    e_core_s, e_lt_s, e_p_s = e_core[order], e_lt[order], e_p[order]

    grp = e_core_s * NT + e_lt_s
    idx_in_grp = np.zeros(len(grp), np.int64)
    _, first_pos, cnt = np.unique(grp, return_index=True, return_counts=True)
    for fp, c in zip(first_pos, cnt):
        idx_in_grp[fp : fp + c] = np.arange(c)
    assert cnt.max() <= K * P, (cnt.max(), K)

    src_cols = np.zeros((NCORES, P, NTK), np.int32)
    tgt_cols = np.full((NCORES, P, NTK), -1.0, np.float32)
    tgt_rows = np.full((NCORES, 1, ES), -1.0, np.float32)
    ea_T = np.zeros((NCORES, 16, ES), np.float32)
    eslot = e_lt_s * (K * P) + idx_in_grp
    col = eslot // P
    row = eslot % P
    src_cols[e_core_s, row, col] = row_id[src_s].astype(np.int32)
    tgt_cols[e_core_s, row, col] = e_p_s.astype(np.float32)
    tgt_rows[e_core_s, 0, eslot] = e_p_s.astype(np.float32)
    ea_T[e_core_s[:, None], np.arange(ED)[None, :], eslot[:, None]] = ea_s

    x = np.asarray(inputs["x"], np.float32)
    x_T = np.zeros((NCORES, 384, NPAD), np.float32)
    pos = lt * P + slot  # position within core [0, NPAD)
    x_T[core_of, :FN, pos] = x
    x_T[core_of, FN, pos] = 1.0  # ones-row carries ctx@Wp+bp via wp1

    rep = lambda v: np.broadcast_to(
        np.asarray(v, np.float32)[None, :], (P, len(np.asarray(v)))
    ).copy()
    bf = lambda a: np.asarray(a).astype(ml_dtypes.bfloat16)

    Wp = np.asarray(inputs["Wp"], np.float32)
    cb = (
        np.asarray(inputs["context_vector"], np.float32) @ Wp[FN:]
        + np.asarray(inputs["bp"], np.float32)
    )
    wp1 = np.zeros((384, 64), np.float32)
    wp1[:FN] = Wp[:FN]
    wp1[FN] = cb
    wp1 = wp1.astype(ml_dtypes.bfloat16)

    common = {
        "wp1": wp1,
        "iota2d": np.broadcast_to(
            np.arange(P, dtype=np.float32)[None, :], (P, P)
        ).astype(ml_dtypes.bfloat16),
        "iota_col": np.arange(P, dtype=np.float32)[:, None].copy(),
        "ident": np.eye(P, dtype=np.float32).astype(ml_dtypes.bfloat16),
        "wh1": np.asarray(inputs["Wh1"], np.float32),
        "bh1_rep": rep(inputs["bh1"]),
        "wh2_rep": rep(np.asarray(inputs["Wh2"], np.float32)[:, 0]),
    }
    g_in = np.asarray(inputs["g_in"], np.float32)
    b_in = np.asarray(inputs["b_in"], np.float32)
    for l in range(3):
        sfx = str(l)
        ws = np.asarray(inputs["Ws" + sfx], np.float32)
        wt = np.asarray(inputs["Wt" + sfx], np.float32)
        wswt = np.concatenate([ws, wt], axis=1)
        if l == 0:
            wswt = np.concatenate(
                [g_in[:, None] * wswt, (b_in @ wswt)[None, :]], axis=0
            )
        common[f"wswt{l}"] = bf(wswt)
        we = np.zeros((16, 256), np.float32)
        we[:ED] = np.asarray(inputs["We" + sfx], np.float32)
        common[f"we{l}"] = bf(we)
        a1 = np.asarray(inputs["A" + sfx], np.float32).reshape(-1)
        common[f"a_rep2_{l}"] = bf(rep(np.tile(a1, 2 * K)))
        if l != 1:
            skw = np.asarray(inputs[f"Sk{l}W"], np.float32)
            if l == 0:
                skw = np.concatenate(
                    [
                        g_in[:, None] * skw,
                        (b_in @ skw + np.asarray(inputs["Sk0b"], np.float32))[
                            None, :
                        ],
                    ],
                    axis=0,
                )
            common[f"skw{l}"] = bf(skw)
            common[f"skb_rep{l}"] = bf(rep(inputs[f"Sk{l}b"]))
        common[f"gn_rep{l}"] = bf(rep(inputs["gn" + sfx]))
        common[f"bn_rep{l}"] = bf(rep(inputs["bn" + sfx]))

    in_maps = []
    for c in range(NCORES):
        m = dict(common)
        m["x_T"] = x_T[c].astype(ml_dtypes.bfloat16)
        m["src_c"] = src_cols[c]
        m["tgt_c"] = tgt_cols[c].astype(ml_dtypes.bfloat16)
        m["tgt_r"] = tgt_rows[c].astype(ml_dtypes.bfloat16)
        m["ea_T"] = ea_T[c].astype(ml_dtypes.bfloat16)
        in_maps.append(m)
    bh2_val = float(np.asarray(inputs["bh2"]).reshape(-1)[0])
    return in_maps, K, bh2_val, (core_of, lt_of, slot)


def kernel(**inputs):
    in_maps, K, bh2_val, (core_of, lt_of, slot) = _prep(inputs)
    nc = build_nc(K, bh2_val)
    res = run_bass_kernel_spmd(
        nc, in_maps, core_ids=list(range(NCORES)), trace=TRACE
    )
    LAST_RESULT["exec_time_ns"] = res.exec_time_ns
    LAST_RESULT["res"] = res
    if DEBUG:
        LAST_RESULT["dbg"] = res.results
        LAST_RESULT["layout"] = (core_of, lt_of, slot)
    outs = np.stack([res.results[c]["out"] for c in range(NCORES)])  # [8, P, NT]
    return outs[core_of, slot, lt_of].astype(np.float32)


# revision 39
# speedup vs baseline: 1.0368x; 1.0322x over previous
"""Bass/Trainium2 kernel for nn_MemoryGAT (3-layer GATv2 + MLP head), 8 NeuronCores.

Nodes are degree-balanced into 8x98 tiles of 128 (K edge-tiles per node tile,
K~4). hs rows are written straight into a device-shared hs_full buffer with
batched indirect scatters; a 1-element AllGather acts as the cross-core
barrier (no bulk collective). Edge loop gathers hs[src] in multi-tile batched
indirect DMAs (SWDGE fixed cost amortized), builds the one-hot S / S^T
selection masks on DVE+Pool without PE transposes, accumulates msg in paired
PSUM banks, and keeps LN stats via accum_out. z stays in SBUF end to end.
"""

import sys
import types

sys.path.insert(0, "/opt/trn_rl_repo")

import ml_dtypes
import numpy as np
import orjson

# ---------------------------------------------------------------- shims

_counter = [0]


def _legalize_module(m, maxw=1):
    """This walrus build accepts only ONE sync-wait per instruction; hoist
    overflow waits onto NoOps inserted just before, on the same engine."""
    for f in m.get("functions", []):
        for b in f.get("blocks", []):
            insts = b.get("instructions")
            if not insts:
                continue
            out = []
            for inst in insts:
                si = inst.get("sync_info")
                waits = (si or {}).get("on_wait") or []
                if si is not None and len(waits) > maxw:
                    keep = waits[-maxw:]
                    extra = waits[: len(waits) - maxw]
                    for j in range(0, len(extra), maxw):
                        _counter[0] += 1
                        out.append(
                            {
                                "name": f"ant-wsplit-{_counter[0]}",
                                "opcode": "NoOp",
                                "engine": inst.get("engine"),
                                "ins": [],
                                "outs": [],
                                "sync_info": {
                                    "on_wait": extra[j : j + maxw],
                                    "on_update": [],
                                },
                            }
                        )
                    si["on_wait"] = keep
                out.append(inst)
            b["instructions"] = out
    return m


def _install_shims():
    import antenv

    if "antenv.axon_hooks" not in sys.modules:
        try:
            from trn_agent_boot.trn_boot import _ntff_profile_via_ctypes

            hooks = types.ModuleType("antenv.axon_hooks")
            hook = _ntff_profile_via_ctypes("/opt/axon/libaxon_pjrt.so")
            hooks.get_axon_ntff_profile_hook = lambda: hook
            hooks.set_axon_ntff_profile_hook = lambda h: None
            sys.modules["antenv.axon_hooks"] = hooks
            antenv.axon_hooks = hooks
        except Exception:
            pass

    import concourse.bass as bass
    from concourse import bass_utils

    bass_utils.upload_artifacts = lambda tmpdir: tmpdir

    if not getattr(bass.Bass, "_waitfix_installed", False):
        base = bass.Bass.to_json_bytes

        def patched(self):
            return orjson.dumps(_legalize_module(orjson.loads(base(self))))

        bass.Bass.to_json_bytes = patched
        bass.Bass._waitfix_installed = True


_install_shims()

import concourse.bass as bass
import concourse.tile as tile
from concourse import mybir
from concourse.bass_utils import run_bass_kernel_spmd

F32 = mybir.dt.float32
BF = mybir.dt.bfloat16
AF = mybir.ActivationFunctionType
ALU = mybir.AluOpType

# ---------------------------------------------------------------- sizes
N = 100_000
E = 400_000
FN = 267
DC = 256
H, D = 4, 64
HD = 256
ED = 11
NCORES = 8
P = 128
NT = 98
NPAD = NT * P  # 12544
NFULL = NCORES * NPAD
# AllGather chunk boundaries (in node tiles) and hs_full region bases
CHT = [0, 40, 72, 92, 98]
NCH = len(CHT) - 1
CHROWS = [(CHT[i + 1] - CHT[i]) * P for i in range(NCH)]
CHBASE = [0]
for i in range(NCH - 1):
    CHBASE.append(CHBASE[-1] + NCORES * CHROWS[i])

TRACE = False
DEBUG = False
LAST_RESULT = {}


# ---------------------------------------------------------------- builder
def build_nc(K, bh2_val):
    NTK = NT * K
    ES = NTK * P
    KP = K * P

    nc = bass.Bass()
    dp = nc.declare_dram_parameter

    x_T = dp("x_T", [384, NPAD], BF, isOutput=False)
    src_c = dp("src_c", [P, NTK], mybir.dt.int32, isOutput=False)
    tgt_c = dp("tgt_c", [P, NTK], BF, isOutput=False)
    tgt_r = dp("tgt_r", [1, ES], BF, isOutput=False)
    ea_T = dp("ea_T", [16, ES], BF, isOutput=False)
    wp1 = dp("wp1", [384, 64], BF, isOutput=False)
    iota2d = dp("iota2d", [P, P], BF, isOutput=False)
    iota_col = dp("iota_col", [P, 1], F32, isOutput=False)
    ident = dp("ident", [P, P], BF, isOutput=False)
    wh1 = dp("wh1", [64, 32], F32, isOutput=False)
    bh1_rep = dp("bh1_rep", [P, 32], F32, isOutput=False)
    wh2_rep = dp("wh2_rep", [P, 32], F32, isOutput=False)

    LW = []
    for l, ind in ((0, 65), (1, 256), (2, 256)):
        d = {"ind": ind, "outd": 64 if l == 2 else 256}
        d["wswt"] = dp(f"wswt{l}", [ind, 512], BF, isOutput=False)
        d["we"] = dp(f"we{l}", [16, 256], BF, isOutput=False)
        d["a_rep2"] = dp(f"a_rep2_{l}", [P, 2 * KP * 2], BF, isOutput=False)
        if l != 1:
            d["skw"] = dp(f"skw{l}", [ind, d["outd"]], BF, isOutput=False)
            d["skb_rep"] = dp(f"skb_rep{l}", [P, d["outd"]], BF, isOutput=False)
        d["gn_rep"] = dp(f"gn_rep{l}", [P, d["outd"]], BF, isOutput=False)
        d["bn_rep"] = dp(f"bn_rep{l}", [P, d["outd"]], BF, isOutput=False)
        LW.append(d)

    out = dp("out", [P, NT], F32, isOutput=True)
    if DEBUG:
        dbg_z0 = dp("dbg_z0", [P, NT, 64], BF, isOutput=True)
        dbg_ht = dp("dbg_ht", [P, NT * 256], BF, isOutput=True)
        dbg_z1 = dp("dbg_z1", [P, NT, 256], BF, isOutput=True)
        dbg_lr = dp("dbg_lr", [P, 2, 1024], BF, isOutput=True)
        dbg_st = dp("dbg_st", [P, 512], BF, isOutput=True)

    hs_shard = [nc.dram_tensor(f"hs_shard{l}", [NPAD, 256], BF) for l in range(3)]
    hs_full = [
        nc.dram_tensor(f"hs_full{l}", [NFULL, 256], BF, addr_space="Shared")
        for l in range(3)
    ]
    res0_dram = nc.dram_tensor("res0_dram", [NPAD, 256], BF)
    h1_dram = nc.dram_tensor("h1_dram", [NPAD, 256], BF)

    with tile.TileContext(nc) as tc:
        with (
            tc.tile_pool(name="const", bufs=1) as cpool,
            tc.tile_pool(name="work", bufs=2) as wpool,
            tc.tile_pool(name="small", bufs=2) as spool,
            tc.tile_pool(name="persist", bufs=1) as ppool,
            tc.tile_pool(name="psPair", bufs=4, space="PSUM") as psPair,
            tc.tile_pool(name="psB", bufs=2, space="PSUM") as psB,
            tc.tile_pool(name="psC", bufs=2, space="PSUM") as psC,
        ):
            for v in {1e-5, 1e-8, float(bh2_val)}:
                ct = cpool.tile([P, 1], F32, tag=f"k{v}", name=f"k{_counter[0]}")
                _counter[0] += 1
                nc.vector.memset(ct[:], v)
                nc.const_aps.aps[(F32, float(v))] = ct[:]

            _cn = [0]

            def c_load(ap, shape, dt=F32):
                _cn[0] += 1
                t = cpool.tile(shape, dt, tag=f"c{_cn[0]}", name=f"c{_cn[0]}")
                nc.sync.dma_start(out=t[:], in_=ap[:])
                return t

            def c_load_chunks(ap, kk, ck, n, dt=F32):
                _cn[0] += 1
                t = cpool.tile([kk, ck * n], dt, tag=f"c{_cn[0]}", name=f"c{_cn[0]}")
                for c in range(ck):
                    nc.sync.dma_start(
                        out=t[:, c * n : (c + 1) * n],
                        in_=ap[c * kk : (c + 1) * kk, :],
                    )
                return t

            iota_sb = c_load(iota2d, [P, P], BF)
            idb_sb = c_load(ident, [P, P], BF)
            iotac_sb = c_load(iota_col, [P, 1], F32)
            iotaK_sb = cpool.tile([P, KP], BF, tag="iotaK", name="iotaK")
            for k in range(K):
                nc.vector.tensor_copy(iotaK_sb[:, k * P : (k + 1) * P], iota_sb[:])
            ones1p = cpool.tile([1, P], BF, tag="ones1p", name="ones1p")
            nc.vector.memset(ones1p[:], 1.0)
            wp1_sb = c_load_chunks(wp1, P, 3, 64, BF)
            wh1_sb = c_load(wh1, [64, 32])
            bh1_sb = c_load(bh1_rep, [P, 32])
            wh2_sb = c_load(wh2_rep, [P, 32])
            lws = []
            for l, d in enumerate(LW):
                s = {}
                ck = max(d["ind"] // P, 1)
                kk = min(d["ind"], P)
                s["wswt"] = c_load_chunks(d["wswt"], kk, ck, 512, BF)
                s["we"] = c_load(d["we"], [16, 256], BF)
                if "skw" in d:
                    s["skw"] = c_load_chunks(d["skw"], kk, ck, d["outd"], BF)
                    s["skb"] = c_load(d["skb_rep"], [P, d["outd"]], BF)
                s["gn"] = c_load(d["gn_rep"], [P, d["outd"]], BF)
                s["bn"] = c_load(d["bn_rep"], [P, d["outd"]], BF)
                s["ck"], s["kk"] = ck, kk
                lws.append(s)

            srcs = ppool.tile([P, NTK], mybir.dt.int32)
            nc.sync.dma_start(out=srcs[:], in_=src_c[:])
            tgts = ppool.tile([P, NTK], BF)
            nc.sync.dma_start(out=tgts[:], in_=tgt_c[:])

            ht_all = ppool.tile([P, NT * 256], BF)
            z_all = ppool.tile([P, NT, 256], BF)
            res2_all = ppool.tile([P, NT * 64], BF)
            scores = ppool.tile([P, NT], F32)

            # one shared LN-stat set; stages are strictly phased so WAR
            # deps keep this safe
            _st = {}
            for nm in ("s1", "s2", "m", "va", "rstd"):
                _st[nm] = ppool.tile([P, NT], F32, tag=f"st{nm}", name=f"st{nm}")
            stats = [_st] * 4

            def sqrt_batch(i, dim, t0=0, t1=NT):
                st = stats[i]
                sl = slice(t0, t1)
                nc.vector.tensor_scalar_mul(st["m"][:, sl], st["s1"][:, sl], 1.0 / dim)
                nc.vector.tensor_scalar_mul(st["va"][:, sl], st["s2"][:, sl], 1.0 / dim)
                nm2 = spool.tile([P, NT], F32, tag="nm2", name="nm2")[:, sl]
                nc.vector.scalar_tensor_tensor(
                    nm2, st["m"][:, sl], -1.0, st["m"][:, sl],
                    op0=ALU.mult, op1=ALU.mult,
                )
                nc.vector.tensor_add(st["va"][:, sl], st["va"][:, sl], nm2)
                sd = spool.tile([P, NT], F32, tag="sd", name="sd")[:, sl]
                nc.scalar.activation(sd, st["va"][:, sl], AF.Sqrt, bias=1e-5)
                nc.vector.reciprocal(st["rstd"][:, sl], sd)
                nc.vector.scalar_tensor_tensor(
                    st["va"][:, sl], st["m"][:, sl], -1.0, st["rstd"][:, sl],
                    op0=ALU.mult, op1=ALU.mult,
                )

            def ag_chunk(l, c):
                nc.gpsimd.collective_compute(
                    "AllGather",
                    ALU.bypass,
                    ins=[hs_shard[l][CHT[c] * P : CHT[c + 1] * P, :]],
                    outs=[
                        hs_full[l][CHBASE[c] : CHBASE[c] + NCORES * CHROWS[c], :]
                    ],
                    replica_groups=[list(range(NCORES))],
                )

            # ---------------- fused phase 0 + f2a0 pipeline -------------------
            # p0(t): x@Wp -> gelu -> z0, stats; sqrt per 4-block;
            # f2a0 stages trail: hn(t-6) | transpose(t-7) | proj(t-8) | copies(t-9)
            def p0_f2a0():
                st = stats[0]
                s = lws[0]
                hns = {}
                lhss = {}
                hshts = {}
                rps = {}
                stag_hs = [None]
                stag_res = [None]
                LAG = 6
                for step in range(NT + LAG + 3):
                    if step < NT:
                        t = step
                        if t % 4 == 0:
                            nbt = min(4, NT - t)
                            xt = wpool.tile([P, 3, 4 * P], BF, tag="hsg", bufs=3)
                            for c in range(3):
                                nc.sync.dma_start(
                                    out=xt[:, c, : nbt * P],
                                    in_=x_T[
                                        c * P : (c + 1) * P, t * P : (t + nbt) * P
                                    ],
                                )
                        xoff = (t % 4) * P
                        h0p = psPair.tile([P, 512], F32, tag="pair", name="h0p")[:, 0:64]
                        for c in range(3):
                            nc.tensor.matmul(
                                out=h0p,
                                lhsT=xt[:, c, xoff : xoff + P],
                                rhs=wp1_sb[:, c * 64 : (c + 1) * 64],
                                start=(c == 0),
                                stop=(c == 2),
                            )
                        zsl = z_all[:, t, 0:64]
                        nc.scalar.activation(
                            zsl, h0p, AF.Gelu, accum_out=st["s1"][:, t : t + 1]
                        )
                        junk = wpool.tile([P, 256], BF, tag="junk", bufs=1, name="junk0")[:, 0:64]
                        nc.vector.scalar_tensor_tensor(
                            junk, zsl, 1.0, zsl,
                            op0=ALU.mult, op1=ALU.mult,
                            accum_out=st["s2"][:, t : t + 1],
                        )
                        if t % 4 == 3 or t == NT - 1:
                            sqrt_batch(0, 64, t - (t % 4), t + 1)
                    # stage A: hn(t) via identity
                    tA = step - LAG
                    if 0 <= tA < NT:
                        hn = wpool.tile([P, 256], BF, tag="hn", bufs=6, name="hn0")[:, 0:64]
                        nc.scalar.activation(
                            hn, z_all[:, tA, :64], AF.Identity,
                            bias=st["va"][:, tA : tA + 1],
                            scale=st["rstd"][:, tA : tA + 1],
                        )
                        hns[tA] = hn
                    # stage B: transpose(t-LAG-1)
                    tB = step - LAG - 1
                    if 0 <= tB < NT:
                        hn = hns.pop(tB)
                        h0t = wpool.tile([65, P], BF, tag="h0t", bufs=4)
                        if tB < 4:
                            nc.vector.memset(h0t[64:65, :], 1.0)
                        trp = psB.tile([P, 2, P], BF, tag="tr", bufs=2)
                        nc.tensor.transpose(
                            out=trp[0:64, 0, :], in_=hn, identity=idb_sb[:]
                        )
                        nc.vector.tensor_copy(h0t[0:64, :], trp[0:64, 0, :])
                        lhss[tB] = h0t
                    # stage C: projections(t-LAG-2)
                    tC = step - LAG - 2
                    if 0 <= tC < NT:
                        h0t = lhss.pop(tC)
                        hsht = psPair.tile([P, 512], F32, tag="pair", name="hsht")
                        nc.tensor.matmul(
                            out=hsht[:], lhsT=h0t[:, :], rhs=s["wswt"][:, 0:512],
                            start=True, stop=True,
                        )
                        hshts[tC] = hsht
                        rp = psC.tile([P, 512], F32, tag="agg", name="rp")[:, 0:256]
                        nc.tensor.matmul(
                            out=rp, lhsT=h0t[:, :], rhs=s["skw"][:, 0:256],
                            start=True, stop=True,
                        )
                        rps[tC] = rp
                    # stage D: copies + stores(t-LAG-3)
                    tD = step - LAG - 3
                    if 0 <= tD < NT:
                        t = tD
                        if t % 4 == 0:
                            stag_hs[0] = wpool.tile(
                                [P, 4, 256], BF, tag="stag_hs", bufs=2, name="shs"
                            )
                            stag_res[0] = wpool.tile(
                                [P, 4, 256], BF, tag="r4x256", bufs=3, name="sres"
                            )
                        hsht = hshts.pop(t)
                        nc.scalar.copy(stag_hs[0][:, t % 4, :], hsht[:, 0:256])
                        nc.scalar.copy(
                            ht_all[:, t * 256 : (t + 1) * 256], hsht[:, 256:512]
                        )
                        rp = rps.pop(t)
                        nc.scalar.copy(stag_res[0][:, t % 4, :], rp)
                        if t % 4 == 3 or t == NT - 1:
                            t0 = t - (t % 4)
                            nbt = t - t0 + 1
                            nc.sync.dma_start(
                                out=hs_shard[0][:].rearrange("(t p) c -> p t c", p=P)[
                                    :, t0 : t0 + nbt, :
                                ],
                                in_=stag_hs[0][:, :nbt, :],
                            )
                            nc.sync.dma_start(
                                out=res0_dram[:].rearrange("(t p) c -> p t c", p=P)[
                                    :, t0 : t0 + nbt, :
                                ],
                                in_=stag_res[0][:, :nbt, :],
                            )
                        for c in range(NCH):
                            if t == CHT[c + 1] - 1:
                                ag_chunk(0, c)

            with nc.named_scope("p0"):
                p0_f2a0()

            # ---------------- F2A(l): finalize h_l, project, scatter+barrier
            # Software-pipelined: hn(t) | transpose(t-1) | proj+copies(t-2)
            def f2a(l):
                st = stats[l]
                ind = 64 if l == 0 else (256 if l < 3 else 64)
                s = lws[l] if l < 3 else None
                hns = {}
                lhss = {}
                h3Ts = {}
                hshts = {}
                rps = {}
                stag_h1 = None
                stag_hs = None
                stag_res = None
                for step in range(NT + 3):
                    # ---- stage A: produce hn(step)
                    if step < NT:
                        t = step
                        if l == 1 and t % 4 == 0:
                            stag_h1 = wpool.tile(
                                [P, 4, 256], BF, tag="sh1", bufs=2, name="sh1"
                            )
                        if l == 0:
                            hn = wpool.tile(
                                [P, 256], BF, tag="hn", bufs=6, name="hn0"
                            )[:, :ind]
                            nc.scalar.activation(
                                hn, z_all[:, t, :ind], AF.Identity,
                                bias=st["va"][:, t : t + 1],
                                scale=st["rstd"][:, t : t + 1],
                            )
                        else:
                            if l == 1:
                                hn = stag_h1[:, t % 4, :]
                            else:
                                hn = wpool.tile(
                                    [P, 256], BF, tag="hn", bufs=6, name="hnl"
                                )[:, :ind]
                            g_sb = lws[l - 1]["gn"]
                            b_sb = lws[l - 1]["bn"]
                            t1 = wpool.tile(
                                [P, 256], F32, tag="t1", bufs=1, name="t1"
                            )[:, :ind]
                            nc.vector.scalar_tensor_tensor(
                                t1, z_all[:, t, :ind], st["m"][:, t : t + 1],
                                g_sb[:, :ind], op0=ALU.subtract, op1=ALU.mult,
                            )
                            u = wpool.tile(
                                [P, 256], BF, tag="u", bufs=6, name="u"
                            )[:, :ind]
                            nc.vector.scalar_tensor_tensor(
                                u, t1, st["rstd"][:, t : t + 1], b_sb[:, :ind],
                                op0=ALU.mult, op1=ALU.add,
                            )
                            nc.scalar.activation(hn, u, AF.Gelu)
                        hns[t] = hn
                        if l == 1 and (t % 4 == 3 or t == NT - 1):
                            t0 = t - (t % 4)
                            nc.sync.dma_start(
                                out=h1_dram[:].rearrange("(t p) c -> p t c", p=P)[
                                    :, t0 : t + 1, :
                                ],
                                in_=stag_h1[:, : t - t0 + 1, :],
                            )
                    # ---- stage B: transpose hn(step-1)
                    if 1 <= step <= NT:
                        t = step - 1
                        hn = hns[t]
                        if l == 3:
                            trp = psB.tile([P, 2, P], BF, tag="tr", bufs=2)
                            nc.tensor.transpose(
                                out=trp[0:64, 0, :], in_=hn, identity=idb_sb[:]
                            )
                            h3T = wpool.tile([64, P], F32, tag="h3T", bufs=3)
                            nc.scalar.copy(h3T[:], trp[0:64, 0, :])
                            h3Ts[t] = h3T
                        elif l == 0:
                            h0t = wpool.tile([65, P], BF, tag="h0t", bufs=4)
                            if t < 4:
                                nc.vector.memset(h0t[64:65, :], 1.0)
                            trp = psB.tile([P, 2, P], BF, tag="tr", bufs=2)
                            nc.tensor.transpose(
                                out=trp[0:64, 0, :], in_=hn, identity=idb_sb[:]
                            )
                            nc.vector.tensor_copy(h0t[0:64, :], trp[0:64, 0, :])
                            lhss[t] = [h0t[:, :]]
                        else:
                            trp = psB.tile([P, 2, P], BF, tag="tr", bufs=2)
                            for c in range(2):
                                nc.tensor.transpose(
                                    out=trp[:, c, :],
                                    in_=hn[:, c * P : (c + 1) * P],
                                    identity=idb_sb[:],
                                )
                            hnT = wpool.tile([P, 2, P], BF, tag="hnT", bufs=6)
                            nc.vector.tensor_copy(hnT[:], trp[:])
                            lhss[t] = [hnT[:, c, :] for c in range(2)]
                    # ---- stage C: project for tile step-2
                    if 2 <= step <= NT + 1:
                        t = step - 2
                        if l == 3:
                            pass
                        else:
                            lhs = lhss.pop(t)
                            hns.pop(t, None)
                            ck = s["ck"]
                            hsht = psPair.tile([P, 512], F32, tag="pair", name="hsht")
                            for c in range(ck):
                                nc.tensor.matmul(
                                    out=hsht[:],
                                    lhsT=lhs[c],
                                    rhs=s["wswt"][:, c * 512 : (c + 1) * 512],
                                    start=(c == 0),
                                    stop=(c == ck - 1),
                                )
                            hshts[t] = hsht
                            if l != 1:
                                outd = LW[l]["outd"]
                                rp = psC.tile([P, 512], F32, tag="agg", name="rp")[
                                    :, :outd
                                ]
                                for c in range(ck):
                                    nc.tensor.matmul(
                                        out=rp,
                                        lhsT=lhs[c],
                                        rhs=s["skw"][:, c * outd : (c + 1) * outd],
                                        start=(c == 0),
                                        stop=(c == ck - 1),
                                    )
                                rps[t] = rp
                    # ---- stage D: copies + stores for tile step-3
                    if step < 3:
                        continue
                    t = step - 3
                    if l == 3:
                        h3T = h3Ts.pop(t)
                        sp1 = psC.tile([P, 512], F32, tag="agg", name="sp1")[:, :32]
                        nc.tensor.matmul(
                            out=sp1, lhsT=h3T[:], rhs=wh1_sb[:], start=True, stop=True
                        )
                        u1 = wpool.tile([P, 32], F32, tag="u1", bufs=2)
                        nc.vector.tensor_add(u1[:], sp1, bh1_sb[:])
                        g1 = wpool.tile([P, 32], F32, tag="g1", bufs=2)
                        nc.scalar.activation(g1[:], u1[:], AF.Gelu)
                        j32 = wpool.tile([P, 32], BF, tag="j32", bufs=2)
                        nc.vector.scalar_tensor_tensor(
                            j32[:], g1[:], 1.0, wh2_sb[:],
                            op0=ALU.mult, op1=ALU.mult,
                            accum_out=scores[:, t : t + 1],
                        )
                        hns.pop(t, None)
                        continue
                    if t % 4 == 0:
                        stag_hs = wpool.tile(
                            [P, 4, 256], BF, tag="stag_hs", bufs=2, name="shs"
                        )
                        if l == 0:
                            stag_res = wpool.tile(
                                [P, 4, 256], BF, tag="r4x256", bufs=3, name="sres"
                            )
                    hsht = hshts.pop(t)
                    nc.scalar.copy(stag_hs[:, t % 4, :], hsht[:, 0:256])
                    nc.scalar.copy(
                        ht_all[:, t * 256 : (t + 1) * 256], hsht[:, 256:512]
                    )
                    if l != 1:
                        rp = rps.pop(t)
                        if l == 0:
                            nc.scalar.copy(stag_res[:, t % 4, :], rp)
                        else:
                            nc.vector.scalar_tensor_tensor(
                                res2_all[:, t * 64 : (t + 1) * 64], rp, 1.0,
                                s["skb"][:], op0=ALU.mult, op1=ALU.add,
                            )
                    # batched stores + AG chunks
                    if t % 4 == 3 or t == NT - 1:
                        t0 = t - (t % 4)
                        nbt = t - t0 + 1
                        nc.sync.dma_start(
                            out=hs_shard[l][:].rearrange("(t p) c -> p t c", p=P)[
                                :, t0 : t0 + nbt, :
                            ],
                            in_=stag_hs[:, :nbt, :],
                        )
                        if l == 0:
                            nc.sync.dma_start(
                                out=res0_dram[:].rearrange("(t p) c -> p t c", p=P)[
                                    :, t0 : t0 + nbt, :
                                ],
                                in_=stag_res[:, :nbt, :],
                            )
                    for c in range(NCH):
                        if t == CHT[c + 1] - 1:
                            ag_chunk(l, c)

            # ---------------- edge + F1 loop --------------------------------
            # Deep pipeline: every cross-engine dep is >=1 tile old.
            #  front(t):   loads, 4 gathers(t) [Pool], ST/S masks(t) [V]
            #  msg(t-1):   10 matmuls [PE] + 2 Prelu(t-2) [S]
            #  alpha(j):   at t=2j+4: scr2/alph2 [V], exp [S]
            #  mid(j):     at t=2j+5: w4 [V], w4col [S]
            #  tail_pe(j): at t=2j+6: agg [PE], den [S]
            #  tail_vs(j): at t=2j+7: rden/gat/z [V], square [S]
            def edge_f1(l):
                s = lws[l]
                outd = LW[l]["outd"]
                st = stats[l + 1]
                a2_sb = wpool.tile(
                    [P, 2 * KP * 2], BF, tag="arep", bufs=1, name=f"arep{l}"
                )
                nc.sync.dma_start(out=a2_sb[:], in_=LW[l]["a_rep2"][:])
                info = {}
                tinfo = {}
                res_sb = [None]
                # ---- fused next-layer f2a state (l2 = l+1, only for l < 2) ----
                l2 = l + 1
                s2 = lws[l2] if l2 < 3 else None
                fstate = {"tf": 0, "sq": 0, "stag_hs": None, "stag_h1": None}

                def f2a_block(zready, nmax):
                    # finalize LN stats in 4-blocks as they become available
                    while fstate["sq"] + 4 <= zready + 1 or (
                        zready == NT - 1 and fstate["sq"] < NT
                    ):
                        b0 = fstate["sq"]
                        b1 = min(b0 + 4, NT)
                        sqrt_batch(l2, LW[l]["outd"], b0, b1)
                        fstate["sq"] = b1
                    n = min(nmax, fstate["sq"] - fstate["tf"])
                    if n <= 0:
                        return
                    t0f = fstate["tf"]
                    tiles = range(t0f, t0f + n)
                    stf = stats[l2]
                    hnb = {}
                    sh1_map = {}
                    # V: LN finalize
                    for tf in tiles:
                        if l2 == 1 and tf % 4 == 0:
                            fstate["stag_h1"] = wpool.tile(
                                [P, 4, 256], BF, tag="sh1", bufs=2, name="sh1"
                            )
                        if l2 == 1:
                            sh1_map[tf] = fstate["stag_h1"]
                        t1 = wpool.tile([P, 256], F32, tag="t1", bufs=1, name="t1")
                        nc.vector.scalar_tensor_tensor(
                            t1[:], z_all[:, tf, :], stf["m"][:, tf : tf + 1],
                            lws[l2 - 1]["gn"][:], op0=ALU.subtract, op1=ALU.mult,
                        )
                        u = wpool.tile([P, 256], BF, tag="u", bufs=6, name="u")
                        nc.vector.scalar_tensor_tensor(
                            u[:], t1[:], stf["rstd"][:, tf : tf + 1],
                            lws[l2 - 1]["bn"][:], op0=ALU.mult, op1=ALU.add,
                        )
                        hnb[tf] = u
                    # S: gelu cluster (gelu table segment)
                    for tf in tiles:
                        if l2 == 1:
                            hn = sh1_map[tf][:, tf % 4, :]
                        else:
                            hn = wpool.tile(
                                [P, 256], BF, tag="hn", bufs=6, name="hnf"
                            )
                            hn = hn[:]
                        nc.scalar.activation(hn, hnb[tf][:], AF.Gelu)
                        hnb[tf] = hn
                        if l2 == 1 and (tf % 4 == 3 or tf == NT - 1):
                            g0 = tf - (tf % 4)
                            nc.sync.dma_start(
                                out=h1_dram[:].rearrange("(t p) c -> p t c", p=P)[
                                    :, g0 : tf + 1, :
                                ],
                                in_=sh1_map[tf][:, : tf - g0 + 1, :],
                            )
                    # PE/V: transpose + hnT copies
                    lhsb = {}
                    for tf in tiles:
                        trp = psB.tile([P, 2, P], BF, tag="tr", bufs=2)
                        for c in range(2):
                            nc.tensor.transpose(
                                out=trp[:, c, :],
                                in_=hnb[tf][:, c * P : (c + 1) * P],
                                identity=idb_sb[:],
                            )
                        hnT = wpool.tile([P, 2, P], BF, tag="hnT", bufs=6)
                        nc.vector.tensor_copy(hnT[:], trp[:])
                        lhsb[tf] = hnT
                    # PE: projections; S: copies (still gelu/copy table)
                    for tf in tiles:
                        if tf % 4 == 0:
                            fstate["stag_hs"] = wpool.tile(
                                [P, 4, 256], BF, tag="stag_hs", bufs=2, name="shs"
                            )
                        hsht = psPair.tile([P, 512], F32, tag="pair", name="hshtf")
                        for c in range(2):
                            nc.tensor.matmul(
                                out=hsht[:],
                                lhsT=lhsb[tf][:, c, :],
                                rhs=s2["wswt"][:, c * 512 : (c + 1) * 512],
                                start=(c == 0),
                                stop=(c == 1),
                            )
                        nc.scalar.copy(fstate["stag_hs"][:, tf % 4, :], hsht[:, 0:256])
                        nc.scalar.copy(
                            ht_all[:, tf * 256 : (tf + 1) * 256], hsht[:, 256:512]
                        )
                        if l2 == 2:
                            rp = psC.tile([P, 512], F32, tag="agg", name="rpf")[:, 0:64]
                            for c in range(2):
                                nc.tensor.matmul(
                                    out=rp,
                                    lhsT=lhsb[tf][:, c, :],
                                    rhs=s2["skw"][:, c * 64 : (c + 1) * 64],
                                    start=(c == 0),
                                    stop=(c == 1),
                                )
                            nc.vector.scalar_tensor_tensor(
                                res2_all[:, tf * 64 : (tf + 1) * 64], rp, 1.0,
                                s2["skb"][:], op0=ALU.mult, op1=ALU.add,
                            )
                        if tf % 4 == 3 or tf == NT - 1:
                            g0 = tf - (tf % 4)
                            nc.sync.dma_start(
                                out=hs_shard[l2][:].rearrange("(t p) c -> p t c", p=P)[
                                    :, g0 : tf + 1, :
                                ],
                                in_=fstate["stag_hs"][:, : tf - g0 + 1, :],
                            )
                        for c in range(NCH):
                            if tf == CHT[c + 1] - 1:
                                ag_chunk(l2, c)
                    fstate["tf"] = t0f + n

                def front(t):
                    j = t // 2
                    if t % 2 == 0:
                        d = {"S": {}, "res": {}, "msgp": {}}
                        info[j] = d
                        d["hsg"] = wpool.tile(
                            [P, 2 * K * 256], BF, tag="hsg", bufs=3, name="hsg"
                        )
                        d["lr2"] = wpool.tile(
                            [P, 2, 1024], BF, tag="lr2", bufs=2, name="lr2"
                        )
                        ea_sb = wpool.tile([16, 2 * KP], BF, tag="ea", bufs=2)
                        nc.sync.dma_start(
                            out=ea_sb[:], in_=ea_T[:, t * KP : (t + 2) * KP]
                        )
                        tr_sb = wpool.tile([P, 2 * KP], BF, tag="tgtr", bufs=1)
                        nc.sync.dma_start(
                            out=tr_sb[:],
                            in_=tgt_r[0:1, t * KP : (t + 2) * KP].to_broadcast(
                                (P, 2 * KP)
                            ),
                        )
                        d["ea"], d["tr"] = ea_sb, tr_sb
                    d = info[j]
                    if l < 2:
                        if t % 4 == 0:
                            nbr = min(4, NT - t)
                            res_sb[0] = wpool.tile(
                                [P, 4, 256], BF, tag="r4x256", bufs=3, name="res_sb"
                            )
                            rdram = res0_dram if l == 0 else h1_dram
                            nc.sync.dma_start(
                                out=res_sb[0][:, :nbr, :],
                                in_=rdram[:].rearrange("(t p) c -> p t c", p=P)[
                                    :, t : t + nbr, :
                                ],
                            )
                        d["res"][t] = res_sb[0][:, t % 4, :]
                    else:
                        d["res"][t] = res2_all[:, t * 64 : (t + 1) * 64]
                    for k in range(K):
                        nc.gpsimd.indirect_dma_start(
                            out=d["hsg"][
                                :, ((t % 2) * K + k) * 256 : ((t % 2) * K + k + 1) * 256
                            ],
                            out_offset=None,
                            in_=hs_full[l][:],
                            in_offset=bass.IndirectOffsetOnAxis(
                                ap=srcs[:, t * K + k : t * K + k + 1], axis=0
                            ),
                        )
                    eoff = (t % 2) * KP
                    ST_all = wpool.tile([P, KP], BF, tag="ST", bufs=3)
                    nc.vector.tensor_scalar(
                        ST_all[:], d["tr"][:, eoff : eoff + KP], iotac_sb[:, 0:1],
                        None, op0=ALU.is_equal,
                    )
                    S_all = wpool.tile([P, KP], BF, tag="S", bufs=7)
                    nc.vector.tensor_tensor(
                        out=S_all[:].rearrange("p (k c) -> p k c", k=K),
                        in0=iotaK_sb[:].rearrange("p (k c) -> p k c", k=K),
                        in1=tgts[:, t * K : (t + 1) * K].to_broadcast((P, K, P)),
                        op=ALU.is_equal,
                    )
                    d["S"][t] = S_all
                    tinfo[t] = (ST_all, d)

                def msg(t):
                    ST_all, d = tinfo.pop(t)
                    j = t // 2
                    eoff = (t % 2) * KP
                    d["msgp"][t] = []
                    for jj in range(2):
                        msgp = psPair.tile([P, 512], F32, tag="pair", name="msgp")
                        cb = ((t % 2) * K + 2 * jj) * 256
                        nc.tensor.matmul(
                            out=msgp[:], lhsT=idb_sb[:],
                            rhs=d["hsg"][:, cb : cb + 512],
                            start=True, stop=False, skip_group_check=True,
                        )
                        for c in range(2):
                            k = 2 * jj + c
                            nc.tensor.matmul(
                                out=msgp[:, c * 256 : (c + 1) * 256],
                                lhsT=d["ea"][:, eoff + k * P : eoff + (k + 1) * P],
                                rhs=s["we"][:],
                                start=False, stop=False, skip_group_check=True,
                            )
                        for c in range(2):
                            k = 2 * jj + c
                            nc.tensor.matmul(
                                out=msgp[:, c * 256 : (c + 1) * 256],
                                lhsT=ST_all[:, k * P : (k + 1) * P],
                                rhs=ht_all[:, t * 256 : (t + 1) * 256],
                                start=False, stop=(c == 1), skip_group_check=True,
                            )
                        d["msgp"][t].append(msgp)

                def prelu(t):
                    j = t // 2
                    d = info[j]
                    for jj in range(2):
                        nc.scalar.activation(
                            d["lr2"][:, t % 2, jj * 512 : (jj + 1) * 512],
                            d["msgp"][t][jj][:],
                            AF.Prelu, alpha=0.2,
                        )
                    del d["msgp"][t]

                def alpha(j):
                    d = info[j]
                    scr2 = wpool.tile([P, 2048], BF, tag="scr2", bufs=1)
                    nc.vector.tensor_tensor(
                        out=scr2[:],
                        in0=d["lr2"][:].rearrange("p a b -> p (a b)"),
                        in1=a2_sb[:],
                        op=ALU.mult,
                    )
                    alph2 = spool.tile([P, 32], F32, tag="alph", bufs=2)
                    nc.vector.tensor_reduce(
                        out=alph2[:],
                        in_=scr2[:].rearrange("p (g d) -> p g d", d=64),
                        axis=mybir.AxisListType.X,
                        op=ALU.add,
                    )
                    expa2 = spool.tile([P, 32], F32, tag="expa", bufs=2)
                    nc.scalar.activation(expa2[:], alph2[:], AF.Exp)
                    d["expa"] = expa2

                def mid(j):
                    d = info[j]
                    d["waug"] = []
                    for i in range(2):
                        w_aug = wpool.tile(
                            [P, K * 260], BF, tag="waug", bufs=2, name="waug"
                        )
                        w4 = w_aug[:].rearrange("p (k h c) -> p k h c", k=K, c=65)
                        hs4 = d["hsg"][
                            :, i * K * 256 : (i + 1) * K * 256
                        ].rearrange("p (k h dd) -> p k h dd", k=K, dd=64)
                        e4 = d["expa"][:, i * 16 : (i + 1) * 16].rearrange(
                            "p (k h) -> p k h", k=K
                        )
                        nc.vector.tensor_tensor(
                            out=w4[:, :, :, 0:64],
                            in0=hs4[:],
                            in1=e4[:].to_broadcast((P, K, H, 64)),
                            op=ALU.mult,
                        )
                        nc.scalar.copy(w4[:, :, :, 64], e4)
                        d["waug"].append(w_aug)

                def tail_pe(j):
                    d = info[j]
                    d["agg"] = []
                    d["den"] = []
                    for i in range(2):
                        agg = psC.tile([P, 512], F32, tag="agg", name="agg")[:, 0:260]
                        for k in range(K):
                            nc.tensor.matmul(
                                out=agg,
                                lhsT=d["S"][2 * j + i][:, k * P : (k + 1) * P],
                                rhs=d["waug"][i][:, k * 260 : (k + 1) * 260],
                                start=(k == 0),
                                stop=(k == K - 1),
                            )
                        d["agg"].append(agg)
                    for i in range(2):
                        aggv = d["agg"][i].rearrange("p (h c) -> p h c", c=65)
                        den = spool.tile([P, 4], F32, tag="den", bufs=4)
                        nc.scalar.activation(den[:], aggv[:, :, 64], AF.Copy, bias=1e-8)
                        d["den"].append(den)

                def tail_vs(j):
                    d = info.pop(j)
                    for i in range(2):
                        tt = 2 * j + i
                        aggv = d["agg"][i].rearrange("p (h c) -> p h c", c=65)
                        rden = spool.tile([P, 4], F32, tag="rden", bufs=2)
                        nc.vector.reciprocal(rden[:], d["den"][i][:])
                        gat = wpool.tile([P, 256], F32, tag="gat", bufs=1)
                        nc.vector.scalar_tensor_tensor(
                            gat[:].rearrange("p (h dd) -> p h dd", h=4),
                            aggv[:, :, 0:64],
                            0.25 if l == 2 else 1.0,
                            rden[:].to_broadcast((P, 4, 64)),
                            op0=ALU.mult,
                            op1=ALU.mult,
                        )
                        if l == 2:
                            g64 = wpool.tile([P, 64], F32, tag="g64", bufs=2)
                            nc.vector.tensor_reduce(
                                out=g64[:],
                                in_=gat[:].rearrange("p (h dd) -> p dd h", h=4),
                                axis=mybir.AxisListType.X,
                                op=ALU.add,
                            )
                            zin = g64[:]
                        else:
                            zin = gat[:]
                        zslot = z_all[:, tt, :outd]
                        nc.vector.scalar_tensor_tensor(
                            zslot, zin, 1.0, d["res"][tt],
                            op0=ALU.mult, op1=ALU.add,
                            accum_out=st["s1"][:, tt : tt + 1],
                        )
                        junk = wpool.tile(
                            [P, 256], BF, tag="junk", bufs=1, name="junke"
                        )[:, :outd]
                        nc.scalar.activation(
                            junk, zslot, AF.Square,
                            accum_out=st["s2"][:, tt : tt + 1],
                        )

                for t in range(NT + 2):
                    if t % 2 == 0:
                        if t >= 4:
                            alpha(t // 2 - 2)
                        if t >= 6:
                            tail_pe(t // 2 - 3)
                    else:
                        if t >= 5:
                            mid(t // 2 - 2)
                        if t >= 7:
                            tail_vs(t // 2 - 3)
                    if t < NT:
                        front(t)
                    if 1 <= t <= NT:
                        msg(t - 1)
                    if 2 <= t <= NT + 1:
                        prelu(t - 2)
                    # fused next-layer f2a in 6-tile clusters
                    if l < 2 and t % 2 == 1 and (t // 2) % 3 == 2 and t >= 13:
                        f2a_block(2 * (t // 2 - 3) + 1, 6)
                NP = NT // 2
                alpha(NP - 1)
                tail_pe(NP - 2)
                mid(NP - 1)
                tail_vs(NP - 2)
                tail_pe(NP - 1)
                tail_vs(NP - 1)
                if l < 2:
                    while fstate["tf"] < NT:
                        f2a_block(NT - 1, 8)

            if DEBUG:
                nc.sync.dma_start(out=dbg_z0[:], in_=z_all[:, :, 0:64])
            if DEBUG:
                nc.sync.dma_start(out=dbg_ht[:], in_=ht_all[:])
            for l in range(3):
                with nc.named_scope(f"edge{l}"):
                    edge_f1(l)
                    if l == 2:
                        sqrt_batch(3, LW[2]["outd"])
                if DEBUG and l == 0:
                    nc.sync.dma_start(out=dbg_z1[:], in_=z_all[:])
                if l == 2:
                    with nc.named_scope("f2a3"):
                        f2a(3)

            sig = ppool.tile([P, NT], F32)
            nc.scalar.activation(sig[:], scores[:], AF.Sigmoid, bias=bh2_val)
            nc.sync.dma_start(out=out[:], in_=sig[:])
    return nc


# ---------------------------------------------------------------- host prep
def _balance_nodes(tgt):
    """Degree-balanced assignment of nodes to NCORES*NT tiles of <=128 slots.
    Returns (gtile[node], slot[node], K)."""
    import heapq

    NTILES = NCORES * NT
    deg = np.bincount(tgt, minlength=N)
    order = np.argsort(-deg, kind="stable")
    gtile = np.empty(N, np.int32)
    slot = np.empty(N, np.int32)
    count = np.zeros(NTILES, np.int32)
    load = np.zeros(NTILES, np.int64)
    heap = [(0, t) for t in range(NTILES)]
    heapq.heapify(heap)
    for node in order:
        while True:
            ld, t = heapq.heappop(heap)
            if count[t] < P and ld == load[t]:
                break
        gtile[node] = t
        slot[node] = count[t]
        count[t] += 1
        load[t] += deg[node]
        if count[t] < P:
            heapq.heappush(heap, (int(load[t]), t))
    K = int(np.ceil(load.max() / P))
    return gtile, slot, K


def _prep(inputs):
    ei = np.asarray(inputs["edge_index"]).astype(np.int64)
    src, tgt = ei[0], ei[1]
    ea = np.asarray(inputs["edge_attr"], np.float32)

    gtile, slot, K = _balance_nodes(tgt)
    core_of = gtile // NT
    lt_of = gtile % NT

    lt = lt_of.astype(np.int64)
    chunk = np.searchsorted(np.array(CHT[1:-1]), lt, side="right")
    chrows = np.array(CHROWS)[chunk]
    chbase = np.array(CHBASE)[chunk]
    chtile0 = np.array(CHT[:-1])[chunk]
    row_id = chbase + core_of.astype(np.int64) * chrows + (lt - chtile0) * P + slot

    NTK = NT * K
    ES = NTK * P

    e_core = core_of[tgt]
    e_lt = lt_of[tgt]
    e_p = slot[tgt]  # target's slot within its tile
    order = np.lexsort((e_lt, e_core))
    src_s = src[order]
    ea_s = ea[order]
    e_core_s, e_lt_s, e_p_s = e_core[order], e_lt[order], e_p[order]

    grp = e_core_s * NT + e_lt_s
    idx_in_grp = np.zeros(len(grp), np.int64)
    _, first_pos, cnt = np.unique(grp, return_index=True, return_counts=True)
    for fp, c in zip(first_pos, cnt):
        idx_in_grp[fp : fp + c] = np.arange(c)
    assert cnt.max() <= K * P, (cnt.max(), K)

    src_cols = np.zeros((NCORES, P, NTK), np.int32)
    tgt_cols = np.full((NCORES, P, NTK), -1.0, np.float32)
    tgt_rows = np.full((NCORES, 1, ES), -1.0, np.float32)
    ea_T = np.zeros((NCORES, 16, ES), np.float32)
    eslot = e_lt_s * (K * P) + idx_in_grp
    col = eslot // P
    row = eslot % P
    src_cols[e_core_s, row, col] = row_id[src_s].astype(np.int32)
    tgt_cols[e_core_s, row, col] = e_p_s.astype(np.float32)
    tgt_rows[e_core_s, 0, eslot] = e_p_s.astype(np.float32)
    ea_T[e_core_s[:, None], np.arange(ED)[None, :], eslot[:, None]] = ea_s

    x = np.asarray(inputs["x"], np.float32)
    x_T = np.zeros((NCORES, 384, NPAD), np.float32)
    pos = lt * P + slot  # position within core [0, NPAD)
    x_T[core_of, :FN, pos] = x
    x_T[core_of, FN, pos] = 1.0  # ones-row carries ctx@Wp+bp via wp1

    rep = lambda v: np.broadcast_to(
        np.asarray(v, np.float32)[None, :], (P, len(np.asarray(v)))
    ).copy()
    bf = lambda a: np.asarray(a).astype(ml_dtypes.bfloat16)

    Wp = np.asarray(inputs["Wp"], np.float32)
    cb = (
        np.asarray(inputs["context_vector"], np.float32) @ Wp[FN:]
        + np.asarray(inputs["bp"], np.float32)
    )
    wp1 = np.zeros((384, 64), np.float32)
    wp1[:FN] = Wp[:FN]
    wp1[FN] = cb
    wp1 = wp1.astype(ml_dtypes.bfloat16)

    common = {
        "wp1": wp1,
        "iota2d": np.broadcast_to(
            np.arange(P, dtype=np.float32)[None, :], (P, P)
        ).astype(ml_dtypes.bfloat16),
        "iota_col": np.arange(P, dtype=np.float32)[:, None].copy(),
        "ident": np.eye(P, dtype=np.float32).astype(ml_dtypes.bfloat16),
        "wh1": np.asarray(inputs["Wh1"], np.float32),
        "bh1_rep": rep(inputs["bh1"]),
        "wh2_rep": rep(np.asarray(inputs["Wh2"], np.float32)[:, 0]),
    }
    g_in = np.asarray(inputs["g_in"], np.float32)
    b_in = np.asarray(inputs["b_in"], np.float32)
    for l in range(3):
        sfx = str(l)
        ws = np.asarray(inputs["Ws" + sfx], np.float32)
        wt = np.asarray(inputs["Wt" + sfx], np.float32)
        wswt = np.concatenate([ws, wt], axis=1)
        if l == 0:
            wswt = np.concatenate(
                [g_in[:, None] * wswt, (b_in @ wswt)[None, :]], axis=0
            )
        common[f"wswt{l}"] = bf(wswt)
        we = np.zeros((16, 256), np.float32)
        we[:ED] = np.asarray(inputs["We" + sfx], np.float32)
        common[f"we{l}"] = bf(we)
        a1 = np.asarray(inputs["A" + sfx], np.float32).reshape(-1)
        common[f"a_rep2_{l}"] = bf(rep(np.tile(a1, 2 * K)))
        if l != 1:
            skw = np.asarray(inputs[f"Sk{l}W"], np.float32)
            if l == 0:
                skw = np.concatenate(
                    [
                        g_in[:, None] * skw,
                        (b_in @ skw + np.asarray(inputs["Sk0b"], np.float32))[
                            None, :
                        ],
                    ],
                    axis=0,
                )
            common[f"skw{l}"] = bf(skw)
            common[f"skb_rep{l}"] = bf(rep(inputs[f"Sk{l}b"]))
        common[f"gn_rep{l}"] = bf(rep(inputs["gn" + sfx]))
        common[f"bn_rep{l}"] = bf(rep(inputs["bn" + sfx]))

    in_maps = []
    for c in range(NCORES):
        m = dict(common)
        m["x_T"] = x_T[c].astype(ml_dtypes.bfloat16)
        m["src_c"] = src_cols[c]
        m["tgt_c"] = tgt_cols[c].astype(ml_dtypes.bfloat16)
        m["tgt_r"] = tgt_rows[c].astype(ml_dtypes.bfloat16)
        m["ea_T"] = ea_T[c].astype(ml_dtypes.bfloat16)
        in_maps.append(m)
    bh2_val = float(np.asarray(inputs["bh2"]).reshape(-1)[0])
    return in_maps, K, bh2_val, (core_of, lt_of, slot)


def kernel(**inputs):
    in_maps, K, bh2_val, (core_of, lt_of, slot) = _prep(inputs)
    nc = build_nc(K, bh2_val)
    res = run_bass_kernel_spmd(
        nc, in_maps, core_ids=list(range(NCORES)), trace=TRACE
    )
    LAST_RESULT["exec_time_ns"] = res.exec_time_ns
    LAST_RESULT["res"] = res
    if DEBUG:
        LAST_RESULT["dbg"] = res.results
        LAST_RESULT["layout"] = (core_of, lt_of, slot)
    outs = np.stack([res.results[c]["out"] for c in range(NCORES)])  # [8, P, NT]
    return outs[core_of, slot, lt_of].astype(np.float32)


# revision 40
# speedup vs baseline: 1.0424x; 1.0054x over previous
"""Bass/Trainium2 kernel for nn_MemoryGAT (3-layer GATv2 + MLP head), 8 NeuronCores.

Nodes are degree-balanced into 8x98 tiles of 128 (K edge-tiles per node tile,
K~4). hs rows are written straight into a device-shared hs_full buffer with
batched indirect scatters; a 1-element AllGather acts as the cross-core
barrier (no bulk collective). Edge loop gathers hs[src] in multi-tile batched
indirect DMAs (SWDGE fixed cost amortized), builds the one-hot S / S^T
selection masks on DVE+Pool without PE transposes, accumulates msg in paired
PSUM banks, and keeps LN stats via accum_out. z stays in SBUF end to end.
"""

import sys
import types

sys.path.insert(0, "/opt/trn_rl_repo")

import ml_dtypes
import numpy as np
import orjson

# ---------------------------------------------------------------- shims

_counter = [0]


def _legalize_module(m, maxw=1):
    """This walrus build accepts only ONE sync-wait per instruction; hoist
    overflow waits onto NoOps inserted just before, on the same engine."""
    for f in m.get("functions", []):
        for b in f.get("blocks", []):
            insts = b.get("instructions")
            if not insts:
                continue
            out = []
            for inst in insts:
                si = inst.get("sync_info")
                waits = (si or {}).get("on_wait") or []
                if si is not None and len(waits) > maxw:
                    keep = waits[-maxw:]
                    extra = waits[: len(waits) - maxw]
                    for j in range(0, len(extra), maxw):
                        _counter[0] += 1
                        out.append(
                            {
                                "name": f"ant-wsplit-{_counter[0]}",
                                "opcode": "NoOp",
                                "engine": inst.get("engine"),
                                "ins": [],
                                "outs": [],
                                "sync_info": {
                                    "on_wait": extra[j : j + maxw],
                                    "on_update": [],
                                },
                            }
                        )
                    si["on_wait"] = keep
                out.append(inst)
            b["instructions"] = out
    return m


def _install_shims():
    import antenv

    if "antenv.axon_hooks" not in sys.modules:
        try:
            from trn_agent_boot.trn_boot import _ntff_profile_via_ctypes

            hooks = types.ModuleType("antenv.axon_hooks")
            hook = _ntff_profile_via_ctypes("/opt/axon/libaxon_pjrt.so")
            hooks.get_axon_ntff_profile_hook = lambda: hook
            hooks.set_axon_ntff_profile_hook = lambda h: None
            sys.modules["antenv.axon_hooks"] = hooks
            antenv.axon_hooks = hooks
        except Exception:
            pass

    import concourse.bass as bass
    from concourse import bass_utils

    bass_utils.upload_artifacts = lambda tmpdir: tmpdir

    if not getattr(bass.Bass, "_waitfix_installed", False):
        base = bass.Bass.to_json_bytes

        def patched(self):
            return orjson.dumps(_legalize_module(orjson.loads(base(self))))

        bass.Bass.to_json_bytes = patched
        bass.Bass._waitfix_installed = True


_install_shims()

import concourse.bass as bass
import concourse.tile as tile
from concourse import mybir
from concourse.bass_utils import run_bass_kernel_spmd

F32 = mybir.dt.float32
BF = mybir.dt.bfloat16
AF = mybir.ActivationFunctionType
ALU = mybir.AluOpType

# ---------------------------------------------------------------- sizes
N = 100_000
E = 400_000
FN = 267
DC = 256
H, D = 4, 64
HD = 256
ED = 11
NCORES = 8
P = 128
NT = 98
NPAD = NT * P  # 12544
NFULL = NCORES * NPAD
# AllGather chunk boundaries (in node tiles) and hs_full region bases
CHT = [0, 40, 72, 92, 98]
NCH = len(CHT) - 1
CHROWS = [(CHT[i + 1] - CHT[i]) * P for i in range(NCH)]
CHBASE = [0]
for i in range(NCH - 1):
    CHBASE.append(CHBASE[-1] + NCORES * CHROWS[i])

TRACE = False
DEBUG = False
LAST_RESULT = {}


# ---------------------------------------------------------------- builder
def build_nc(K, bh2_val):
    NTK = NT * K
    ES = NTK * P
    KP = K * P

    nc = bass.Bass()
    dp = nc.declare_dram_parameter

    x_T = dp("x_T", [384, NPAD], BF, isOutput=False)
    src_c = dp("src_c", [P, NTK], mybir.dt.int32, isOutput=False)
    tgt_c = dp("tgt_c", [P, NTK], BF, isOutput=False)
    tgt_r = dp("tgt_r", [1, ES], BF, isOutput=False)
    ea_T = dp("ea_T", [16, ES], BF, isOutput=False)
    wp1 = dp("wp1", [384, 64], BF, isOutput=False)
    iota2d = dp("iota2d", [P, P], BF, isOutput=False)
    iota_col = dp("iota_col", [P, 1], F32, isOutput=False)
    ident = dp("ident", [P, P], BF, isOutput=False)
    wh1 = dp("wh1", [64, 32], F32, isOutput=False)
    bh1_rep = dp("bh1_rep", [P, 32], F32, isOutput=False)
    wh2_rep = dp("wh2_rep", [P, 32], F32, isOutput=False)

    LW = []
    for l, ind in ((0, 65), (1, 256), (2, 256)):
        d = {"ind": ind, "outd": 64 if l == 2 else 256}
        d["wswt"] = dp(f"wswt{l}", [ind, 512], BF, isOutput=False)
        d["we"] = dp(f"we{l}", [16, 256], BF, isOutput=False)
        d["a_rep2"] = dp(f"a_rep2_{l}", [P, 2 * KP * 2], BF, isOutput=False)
        if l != 1:
            d["skw"] = dp(f"skw{l}", [ind, d["outd"]], BF, isOutput=False)
            d["skb_rep"] = dp(f"skb_rep{l}", [P, d["outd"]], BF, isOutput=False)
        d["gn_rep"] = dp(f"gn_rep{l}", [P, d["outd"]], BF, isOutput=False)
        d["bn_rep"] = dp(f"bn_rep{l}", [P, d["outd"]], BF, isOutput=False)
        LW.append(d)

    out = dp("out", [P, NT], F32, isOutput=True)
    if DEBUG:
        dbg_z0 = dp("dbg_z0", [P, NT, 64], BF, isOutput=True)
        dbg_ht = dp("dbg_ht", [P, NT * 256], BF, isOutput=True)
        dbg_z1 = dp("dbg_z1", [P, NT, 256], BF, isOutput=True)
        dbg_lr = dp("dbg_lr", [P, 2, 1024], BF, isOutput=True)
        dbg_st = dp("dbg_st", [P, 512], BF, isOutput=True)

    hs_shard = [nc.dram_tensor(f"hs_shard{l}", [NPAD, 256], BF) for l in range(3)]
    hs_full = [
        nc.dram_tensor(f"hs_full{l}", [NFULL, 256], BF, addr_space="Shared")
        for l in range(3)
    ]
    res0_dram = nc.dram_tensor("res0_dram", [NPAD, 256], BF)
    h1_dram = nc.dram_tensor("h1_dram", [NPAD, 256], BF)

    with tile.TileContext(nc) as tc:
        with (
            tc.tile_pool(name="const", bufs=1) as cpool,
            tc.tile_pool(name="work", bufs=2) as wpool,
            tc.tile_pool(name="small", bufs=2) as spool,
            tc.tile_pool(name="persist", bufs=1) as ppool,
            tc.tile_pool(name="psPair", bufs=4, space="PSUM") as psPair,
            tc.tile_pool(name="psB", bufs=2, space="PSUM") as psB,
            tc.tile_pool(name="psC", bufs=2, space="PSUM") as psC,
        ):
            for v in {1e-5, 1e-8, float(bh2_val)}:
                ct = cpool.tile([P, 1], F32, tag=f"k{v}", name=f"k{_counter[0]}")
                _counter[0] += 1
                nc.vector.memset(ct[:], v)
                nc.const_aps.aps[(F32, float(v))] = ct[:]

            _cn = [0]

            def c_load(ap, shape, dt=F32):
                _cn[0] += 1
                t = cpool.tile(shape, dt, tag=f"c{_cn[0]}", name=f"c{_cn[0]}")
                nc.sync.dma_start(out=t[:], in_=ap[:])
                return t

            def c_load_chunks(ap, kk, ck, n, dt=F32):
                _cn[0] += 1
                t = cpool.tile([kk, ck * n], dt, tag=f"c{_cn[0]}", name=f"c{_cn[0]}")
                for c in range(ck):
                    nc.sync.dma_start(
                        out=t[:, c * n : (c + 1) * n],
                        in_=ap[c * kk : (c + 1) * kk, :],
                    )
                return t

            iota_sb = c_load(iota2d, [P, P], BF)
            idb_sb = c_load(ident, [P, P], BF)
            iotac_sb = c_load(iota_col, [P, 1], F32)
            iotaK_sb = cpool.tile([P, KP], BF, tag="iotaK", name="iotaK")
            for k in range(K):
                nc.vector.tensor_copy(iotaK_sb[:, k * P : (k + 1) * P], iota_sb[:])
            ones1p = cpool.tile([1, P], BF, tag="ones1p", name="ones1p")
            nc.vector.memset(ones1p[:], 1.0)
            wp1_sb = c_load_chunks(wp1, P, 3, 64, BF)
            wh1_sb = c_load(wh1, [64, 32])
            bh1_sb = c_load(bh1_rep, [P, 32])
            wh2_sb = c_load(wh2_rep, [P, 32])
            lws = []
            for l, d in enumerate(LW):
                s = {}
                ck = max(d["ind"] // P, 1)
                kk = min(d["ind"], P)
                s["wswt"] = c_load_chunks(d["wswt"], kk, ck, 512, BF)
                s["we"] = c_load(d["we"], [16, 256], BF)
                if "skw" in d:
                    s["skw"] = c_load_chunks(d["skw"], kk, ck, d["outd"], BF)
                    s["skb"] = c_load(d["skb_rep"], [P, d["outd"]], BF)
                s["gn"] = c_load(d["gn_rep"], [P, d["outd"]], BF)
                s["bn"] = c_load(d["bn_rep"], [P, d["outd"]], BF)
                s["ck"], s["kk"] = ck, kk
                lws.append(s)

            srcs = ppool.tile([P, NTK], mybir.dt.int32)
            nc.sync.dma_start(out=srcs[:], in_=src_c[:])
            tgts = ppool.tile([P, NTK], BF)
            nc.sync.dma_start(out=tgts[:], in_=tgt_c[:])

            ht_all = ppool.tile([P, NT * 256], BF)
            z_all = ppool.tile([P, NT, 256], BF)
            res2_all = ppool.tile([P, NT * 64], BF)
            scores = ppool.tile([P, NT], F32)

            # one shared LN-stat set; stages are strictly phased so WAR
            # deps keep this safe
            _st = {}
            for nm in ("s1", "s2", "m", "va", "rstd"):
                _st[nm] = ppool.tile([P, NT], F32, tag=f"st{nm}", name=f"st{nm}")
            stats = [_st] * 4

            def sqrt_batch(i, dim, t0=0, t1=NT):
                st = stats[i]
                sl = slice(t0, t1)
                nc.vector.tensor_scalar_mul(st["m"][:, sl], st["s1"][:, sl], 1.0 / dim)
                nc.vector.tensor_scalar_mul(st["va"][:, sl], st["s2"][:, sl], 1.0 / dim)
                nm2 = spool.tile([P, NT], F32, tag="nm2", name="nm2")[:, sl]
                nc.vector.scalar_tensor_tensor(
                    nm2, st["m"][:, sl], -1.0, st["m"][:, sl],
                    op0=ALU.mult, op1=ALU.mult,
                )
                nc.vector.tensor_add(st["va"][:, sl], st["va"][:, sl], nm2)
                sd = spool.tile([P, NT], F32, tag="sd", name="sd")[:, sl]
                nc.scalar.activation(sd, st["va"][:, sl], AF.Sqrt, bias=1e-5)
                nc.vector.reciprocal(st["rstd"][:, sl], sd)
                nc.vector.scalar_tensor_tensor(
                    st["va"][:, sl], st["m"][:, sl], -1.0, st["rstd"][:, sl],
                    op0=ALU.mult, op1=ALU.mult,
                )

            def ag_chunk(l, c):
                nc.gpsimd.collective_compute(
                    "AllGather",
                    ALU.bypass,
                    ins=[hs_shard[l][CHT[c] * P : CHT[c + 1] * P, :]],
                    outs=[
                        hs_full[l][CHBASE[c] : CHBASE[c] + NCORES * CHROWS[c], :]
                    ],
                    replica_groups=[list(range(NCORES))],
                )

            # ---------------- fused phase 0 + f2a0 pipeline -------------------
            # p0(t): x@Wp -> gelu -> z0, stats; sqrt per 4-block;
            # f2a0 stages trail: hn(t-6) | transpose(t-7) | proj(t-8) | copies(t-9)
            def p0_f2a0():
                st = stats[0]
                s = lws[0]
                hns = {}
                lhss = {}
                hshts = {}
                rps = {}
                stag_hs = [None]
                stag_res = [None]
                LAG = 6
                for step in range(NT + LAG + 3):
                    if step < NT:
                        t = step
                        if t % 4 == 0:
                            nbt = min(4, NT - t)
                            xt = wpool.tile([P, 3, 4 * P], BF, tag="hsg", bufs=3)
                            for c in range(3):
                                nc.sync.dma_start(
                                    out=xt[:, c, : nbt * P],
                                    in_=x_T[
                                        c * P : (c + 1) * P, t * P : (t + nbt) * P
                                    ],
                                )
                        xoff = (t % 4) * P
                        h0p = psPair.tile([P, 512], F32, tag="pair", name="h0p")[:, 0:64]
                        for c in range(3):
                            nc.tensor.matmul(
                                out=h0p,
                                lhsT=xt[:, c, xoff : xoff + P],
                                rhs=wp1_sb[:, c * 64 : (c + 1) * 64],
                                start=(c == 0),
                                stop=(c == 2),
                            )
                        zsl = z_all[:, t, 0:64]
                        nc.scalar.activation(
                            zsl, h0p, AF.Gelu, accum_out=st["s1"][:, t : t + 1]
                        )
                        junk = wpool.tile([P, 256], BF, tag="junk", bufs=1, name="junk0")[:, 0:64]
                        nc.vector.scalar_tensor_tensor(
                            junk, zsl, 1.0, zsl,
                            op0=ALU.mult, op1=ALU.mult,
                            accum_out=st["s2"][:, t : t + 1],
                        )
                        if t % 4 == 3 or t == NT - 1:
                            sqrt_batch(0, 64, t - (t % 4), t + 1)
                    # stage A: hn(t) via identity
                    tA = step - LAG
                    if 0 <= tA < NT:
                        hn = wpool.tile([P, 256], BF, tag="hn", bufs=6, name="hn0")[:, 0:64]
                        nc.scalar.activation(
                            hn, z_all[:, tA, :64], AF.Identity,
                            bias=st["va"][:, tA : tA + 1],
                            scale=st["rstd"][:, tA : tA + 1],
                        )
                        hns[tA] = hn
                    # stage B: transpose(t-LAG-1)
                    tB = step - LAG - 1
                    if 0 <= tB < NT:
                        hn = hns.pop(tB)
                        h0t = wpool.tile([65, P], BF, tag="h0t", bufs=4)
                        if tB < 4:
                            nc.vector.memset(h0t[64:65, :], 1.0)
                        trp = psB.tile([P, 2, P], BF, tag="tr", bufs=2)
                        nc.tensor.transpose(
                            out=trp[0:64, 0, :], in_=hn, identity=idb_sb[:]
                        )
                        nc.vector.tensor_copy(h0t[0:64, :], trp[0:64, 0, :])
                        lhss[tB] = h0t
                    # stage C: projections(t-LAG-2)
                    tC = step - LAG - 2
                    if 0 <= tC < NT:
                        h0t = lhss.pop(tC)
                        hsht = psPair.tile([P, 512], F32, tag="pair", name="hsht")
                        nc.tensor.matmul(
                            out=hsht[:], lhsT=h0t[:, :], rhs=s["wswt"][:, 0:512],
                            start=True, stop=True,
                        )
                        hshts[tC] = hsht
                        rp = psC.tile([P, 512], F32, tag="agg", name="rp")[:, 0:256]
                        nc.tensor.matmul(
                            out=rp, lhsT=h0t[:, :], rhs=s["skw"][:, 0:256],
                            start=True, stop=True,
                        )
                        rps[tC] = rp
                    # stage D: copies + stores(t-LAG-3)
                    tD = step - LAG - 3
                    if 0 <= tD < NT:
                        t = tD
                        if t % 4 == 0:
                            stag_hs[0] = wpool.tile(
                                [P, 4, 256], BF, tag="stag_hs", bufs=2, name="shs"
                            )
                            stag_res[0] = wpool.tile(
                                [P, 4, 256], BF, tag="r4x256", bufs=3, name="sres"
                            )
                        hsht = hshts.pop(t)
                        nc.scalar.copy(stag_hs[0][:, t % 4, :], hsht[:, 0:256])
                        nc.scalar.copy(
                            ht_all[:, t * 256 : (t + 1) * 256], hsht[:, 256:512]
                        )
                        rp = rps.pop(t)
                        nc.scalar.copy(stag_res[0][:, t % 4, :], rp)
                        if t % 4 == 3 or t == NT - 1:
                            t0 = t - (t % 4)
                            nbt = t - t0 + 1
                            nc.sync.dma_start(
                                out=hs_shard[0][:].rearrange("(t p) c -> p t c", p=P)[
                                    :, t0 : t0 + nbt, :
                                ],
                                in_=stag_hs[0][:, :nbt, :],
                            )
                            nc.sync.dma_start(
                                out=res0_dram[:].rearrange("(t p) c -> p t c", p=P)[
                                    :, t0 : t0 + nbt, :
                                ],
                                in_=stag_res[0][:, :nbt, :],
                            )
                        for c in range(NCH):
                            if t == CHT[c + 1] - 1:
                                ag_chunk(0, c)

            with nc.named_scope("p0"):
                p0_f2a0()

            # ---------------- F2A(l): finalize h_l, project, scatter+barrier
            # Software-pipelined: hn(t) | transpose(t-1) | proj+copies(t-2)
            def f2a(l):
                st = stats[l]
                ind = 64 if l == 0 else (256 if l < 3 else 64)
                s = lws[l] if l < 3 else None
                hns = {}
                lhss = {}
                h3Ts = {}
                hshts = {}
                rps = {}
                stag_h1 = None
                stag_hs = None
                stag_res = None
                for step in range(NT + 3):
                    # ---- stage A: produce hn(step)
                    if step < NT:
                        t = step
                        if l == 1 and t % 4 == 0:
                            stag_h1 = wpool.tile(
                                [P, 4, 256], BF, tag="sh1", bufs=2, name="sh1"
                            )
                        if l == 0:
                            hn = wpool.tile(
                                [P, 256], BF, tag="hn", bufs=6, name="hn0"
                            )[:, :ind]
                            nc.scalar.activation(
                                hn, z_all[:, t, :ind], AF.Identity,
                                bias=st["va"][:, t : t + 1],
                                scale=st["rstd"][:, t : t + 1],
                            )
                        else:
                            if l == 1:
                                hn = stag_h1[:, t % 4, :]
                            else:
                                hn = wpool.tile(
                                    [P, 256], BF, tag="hn", bufs=6, name="hnl"
                                )[:, :ind]
                            g_sb = lws[l - 1]["gn"]
                            b_sb = lws[l - 1]["bn"]
                            t1 = wpool.tile(
                                [P, 256], F32, tag="t1", bufs=1, name="t1"
                            )[:, :ind]
                            nc.vector.scalar_tensor_tensor(
                                t1, z_all[:, t, :ind], st["m"][:, t : t + 1],
                                g_sb[:, :ind], op0=ALU.subtract, op1=ALU.mult,
                            )
                            u = wpool.tile(
                                [P, 256], BF, tag="u", bufs=6, name="u"
                            )[:, :ind]
                            nc.vector.scalar_tensor_tensor(
                                u, t1, st["rstd"][:, t : t + 1], b_sb[:, :ind],
                                op0=ALU.mult, op1=ALU.add,
                            )
                            nc.scalar.activation(hn, u, AF.Gelu)
                        hns[t] = hn
                        if l == 1 and (t % 4 == 3 or t == NT - 1):
                            t0 = t - (t % 4)
                            nc.sync.dma_start(
                                out=h1_dram[:].rearrange("(t p) c -> p t c", p=P)[
                                    :, t0 : t + 1, :
                                ],
                                in_=stag_h1[:, : t - t0 + 1, :],
                            )
                    # ---- stage B: transpose hn(step-1)
                    if 1 <= step <= NT:
                        t = step - 1
                        hn = hns[t]
                        if l == 3:
                            trp = psB.tile([P, 2, P], BF, tag="tr", bufs=2)
                            nc.tensor.transpose(
                                out=trp[0:64, 0, :], in_=hn, identity=idb_sb[:]
                            )
                            h3T = wpool.tile([64, P], F32, tag="h3T", bufs=3)
                            nc.scalar.copy(h3T[:], trp[0:64, 0, :])
                            h3Ts[t] = h3T
                        elif l == 0:
                            h0t = wpool.tile([65, P], BF, tag="h0t", bufs=4)
                            if t < 4:
                                nc.vector.memset(h0t[64:65, :], 1.0)
                            trp = psB.tile([P, 2, P], BF, tag="tr", bufs=2)
                            nc.tensor.transpose(
                                out=trp[0:64, 0, :], in_=hn, identity=idb_sb[:]
                            )
                            nc.vector.tensor_copy(h0t[0:64, :], trp[0:64, 0, :])
                            lhss[t] = [h0t[:, :]]
                        else:
                            trp = psB.tile([P, 2, P], BF, tag="tr", bufs=2)
                            for c in range(2):
                                nc.tensor.transpose(
                                    out=trp[:, c, :],
                                    in_=hn[:, c * P : (c + 1) * P],
                                    identity=idb_sb[:],
                                )
                            hnT = wpool.tile([P, 2, P], BF, tag="hnT", bufs=6)
                            nc.vector.tensor_copy(hnT[:], trp[:])
                            lhss[t] = [hnT[:, c, :] for c in range(2)]
                    # ---- stage C: project for tile step-2
                    if 2 <= step <= NT + 1:
                        t = step - 2
                        if l == 3:
                            pass
                        else:
                            lhs = lhss.pop(t)
                            hns.pop(t, None)
                            ck = s["ck"]
                            hsht = psPair.tile([P, 512], F32, tag="pair", name="hsht")
                            for c in range(ck):
                                nc.tensor.matmul(
                                    out=hsht[:],
                                    lhsT=lhs[c],
                                    rhs=s["wswt"][:, c * 512 : (c + 1) * 512],
                                    start=(c == 0),
                                    stop=(c == ck - 1),
                                )
                            hshts[t] = hsht
                            if l != 1:
                                outd = LW[l]["outd"]
                                rp = psC.tile([P, 512], F32, tag="agg", name="rp")[
                                    :, :outd
                                ]
                                for c in range(ck):
                                    nc.tensor.matmul(
                                        out=rp,
                                        lhsT=lhs[c],
                                        rhs=s["skw"][:, c * outd : (c + 1) * outd],
                                        start=(c == 0),
                                        stop=(c == ck - 1),
                                    )
                                rps[t] = rp
                    # ---- stage D: copies + stores for tile step-3
                    if step < 3:
                        continue
                    t = step - 3
                    if l == 3:
                        h3T = h3Ts.pop(t)
                        sp1 = psC.tile([P, 512], F32, tag="agg", name="sp1")[:, :32]
                        nc.tensor.matmul(
                            out=sp1, lhsT=h3T[:], rhs=wh1_sb[:], start=True, stop=True
                        )
                        u1 = wpool.tile([P, 32], F32, tag="u1", bufs=2)
                        nc.vector.tensor_add(u1[:], sp1, bh1_sb[:])
                        g1 = wpool.tile([P, 32], F32, tag="g1", bufs=2)
                        nc.scalar.activation(g1[:], u1[:], AF.Gelu)
                        j32 = wpool.tile([P, 32], BF, tag="j32", bufs=2)
                        nc.vector.scalar_tensor_tensor(
                            j32[:], g1[:], 1.0, wh2_sb[:],
                            op0=ALU.mult, op1=ALU.mult,
                            accum_out=scores[:, t : t + 1],
                        )
                        hns.pop(t, None)
                        continue
                    if t % 4 == 0:
                        stag_hs = wpool.tile(
                            [P, 4, 256], BF, tag="stag_hs", bufs=2, name="shs"
                        )
                        if l == 0:
                            stag_res = wpool.tile(
                                [P, 4, 256], BF, tag="r4x256", bufs=3, name="sres"
                            )
                    hsht = hshts.pop(t)
                    nc.scalar.copy(stag_hs[:, t % 4, :], hsht[:, 0:256])
                    nc.scalar.copy(
                        ht_all[:, t * 256 : (t + 1) * 256], hsht[:, 256:512]
                    )
                    if l != 1:
                        rp = rps.pop(t)
                        if l == 0:
                            nc.scalar.copy(stag_res[:, t % 4, :], rp)
                        else:
                            nc.vector.scalar_tensor_tensor(
                                res2_all[:, t * 64 : (t + 1) * 64], rp, 1.0,
                                s["skb"][:], op0=ALU.mult, op1=ALU.add,
                            )
                    # batched stores + AG chunks
                    if t % 4 == 3 or t == NT - 1:
                        t0 = t - (t % 4)
                        nbt = t - t0 + 1
                        nc.sync.dma_start(
                            out=hs_shard[l][:].rearrange("(t p) c -> p t c", p=P)[
                                :, t0 : t0 + nbt, :
                            ],
                            in_=stag_hs[:, :nbt, :],
                        )
                        if l == 0:
                            nc.sync.dma_start(
                                out=res0_dram[:].rearrange("(t p) c -> p t c", p=P)[
                                    :, t0 : t0 + nbt, :
                                ],
                                in_=stag_res[:, :nbt, :],
                            )
                    for c in range(NCH):
                        if t == CHT[c + 1] - 1:
                            ag_chunk(l, c)

            # ---------------- edge + F1 loop --------------------------------
            # Deep pipeline: every cross-engine dep is >=1 tile old.
            #  front(t):   loads, 4 gathers(t) [Pool], ST/S masks(t) [V]
            #  msg(t-1):   10 matmuls [PE] + 2 Prelu(t-2) [S]
            #  alpha(j):   at t=2j+4: scr2/alph2 [V], exp [S]
            #  mid(j):     at t=2j+5: w4 [V], w4col [S]
            #  tail_pe(j): at t=2j+6: agg [PE], den [S]
            #  tail_vs(j): at t=2j+7: rden/gat/z [V], square [S]
            def edge_f1(l):
                s = lws[l]
                outd = LW[l]["outd"]
                st = stats[l + 1]
                a2_sb = wpool.tile(
                    [P, 2 * KP * 2], BF, tag="arep", bufs=1, name=f"arep{l}"
                )
                nc.sync.dma_start(out=a2_sb[:], in_=LW[l]["a_rep2"][:])
                info = {}
                tinfo = {}
                res_sb = [None]
                # ---- fused next-layer f2a state (l2 = l+1, only for l < 2) ----
                l2 = l + 1
                s2 = lws[l2] if l2 < 3 else None
                fstate = {"tf": 0, "sq": 0, "stag_hs": None, "stag_h1": None}

                def f2a_block(zready, nmax):
                    # finalize LN stats in 4-blocks as they become available
                    while fstate["sq"] + 4 <= zready + 1 or (
                        zready == NT - 1 and fstate["sq"] < NT
                    ):
                        b0 = fstate["sq"]
                        b1 = min(b0 + 4, NT)
                        sqrt_batch(l2, LW[l]["outd"], b0, b1)
                        fstate["sq"] = b1
                    n = min(nmax, fstate["sq"] - fstate["tf"])
                    if n <= 0:
                        return
                    t0f = fstate["tf"]
                    tiles = range(t0f, t0f + n)
                    stf = stats[l2]
                    hnb = {}
                    sh1_map = {}
                    # V: LN finalize
                    for tf in tiles:
                        if l2 == 1 and tf % 4 == 0:
                            fstate["stag_h1"] = wpool.tile(
                                [P, 4, 256], BF, tag="sh1", bufs=2, name="sh1"
                            )
                        if l2 == 1:
                            sh1_map[tf] = fstate["stag_h1"]
                        t1 = wpool.tile([P, 256], F32, tag="t1", bufs=1, name="t1")
                        nc.vector.scalar_tensor_tensor(
                            t1[:], z_all[:, tf, :], stf["m"][:, tf : tf + 1],
                            lws[l2 - 1]["gn"][:], op0=ALU.subtract, op1=ALU.mult,
                        )
                        u = wpool.tile([P, 256], BF, tag="u", bufs=6, name="u")
                        nc.vector.scalar_tensor_tensor(
                            u[:], t1[:], stf["rstd"][:, tf : tf + 1],
                            lws[l2 - 1]["bn"][:], op0=ALU.mult, op1=ALU.add,
                        )
                        hnb[tf] = u
                    # S: gelu cluster (gelu table segment)
                    for tf in tiles:
                        if l2 == 1:
                            hn = sh1_map[tf][:, tf % 4, :]
                        else:
                            hn = wpool.tile(
                                [P, 256], BF, tag="hn", bufs=6, name="hnf"
                            )
                            hn = hn[:]
                        nc.scalar.activation(hn, hnb[tf][:], AF.Gelu)
                        hnb[tf] = hn
                        if l2 == 1 and (tf % 4 == 3 or tf == NT - 1):
                            g0 = tf - (tf % 4)
                            nc.sync.dma_start(
                                out=h1_dram[:].rearrange("(t p) c -> p t c", p=P)[
                                    :, g0 : tf + 1, :
                                ],
                                in_=sh1_map[tf][:, : tf - g0 + 1, :],
                            )
                    # PE/V: transpose + hnT copies
                    lhsb = {}
                    for tf in tiles:
                        trp = psB.tile([P, 2, P], BF, tag="tr", bufs=2)
                        for c in range(2):
                            nc.tensor.transpose(
                                out=trp[:, c, :],
                                in_=hnb[tf][:, c * P : (c + 1) * P],
                                identity=idb_sb[:],
                            )
                        hnT = wpool.tile([P, 2, P], BF, tag="hnT", bufs=6)
                        nc.scalar.copy(hnT[:], trp[:])
                        lhsb[tf] = hnT
                    # PE: projections; S: copies (still gelu/copy table)
                    for tf in tiles:
                        if tf % 4 == 0:
                            fstate["stag_hs"] = wpool.tile(
                                [P, 4, 256], BF, tag="stag_hs", bufs=2, name="shs"
                            )
                        hsht = psPair.tile([P, 512], F32, tag="pair", name="hshtf")
                        for c in range(2):
                            nc.tensor.matmul(
                                out=hsht[:],
                                lhsT=lhsb[tf][:, c, :],
                                rhs=s2["wswt"][:, c * 512 : (c + 1) * 512],
                                start=(c == 0),
                                stop=(c == 1),
                            )
                        nc.scalar.copy(fstate["stag_hs"][:, tf % 4, :], hsht[:, 0:256])
                        nc.scalar.copy(
                            ht_all[:, tf * 256 : (tf + 1) * 256], hsht[:, 256:512]
                        )
                        if l2 == 2:
                            rp = psC.tile([P, 512], F32, tag="agg", name="rpf")[:, 0:64]
                            for c in range(2):
                                nc.tensor.matmul(
                                    out=rp,
                                    lhsT=lhsb[tf][:, c, :],
                                    rhs=s2["skw"][:, c * 64 : (c + 1) * 64],
                                    start=(c == 0),
                                    stop=(c == 1),
                                )
                            nc.vector.scalar_tensor_tensor(
                                res2_all[:, tf * 64 : (tf + 1) * 64], rp, 1.0,
                                s2["skb"][:], op0=ALU.mult, op1=ALU.add,
                            )
                        if tf % 4 == 3 or tf == NT - 1:
                            g0 = tf - (tf % 4)
                            nc.sync.dma_start(
                                out=hs_shard[l2][:].rearrange("(t p) c -> p t c", p=P)[
                                    :, g0 : tf + 1, :
                                ],
                                in_=fstate["stag_hs"][:, : tf - g0 + 1, :],
                            )
                        for c in range(NCH):
                            if tf == CHT[c + 1] - 1:
                                ag_chunk(l2, c)
                    fstate["tf"] = t0f + n

                def front(t):
                    j = t // 2
                    if t % 2 == 0:
                        d = {"S": {}, "res": {}, "msgp": {}}
                        info[j] = d
                        d["hsg"] = wpool.tile(
                            [P, 2 * K * 256], BF, tag="hsg", bufs=3, name="hsg"
                        )
                        d["lr2"] = wpool.tile(
                            [P, 2, 1024], BF, tag="lr2", bufs=2, name="lr2"
                        )
                        ea_sb = wpool.tile([16, 2 * KP], BF, tag="ea", bufs=2)
                        nc.sync.dma_start(
                            out=ea_sb[:], in_=ea_T[:, t * KP : (t + 2) * KP]
                        )
                        tr_sb = wpool.tile([P, 2 * KP], BF, tag="tgtr", bufs=1)
                        nc.sync.dma_start(
                            out=tr_sb[:],
                            in_=tgt_r[0:1, t * KP : (t + 2) * KP].to_broadcast(
                                (P, 2 * KP)
                            ),
                        )
                        d["ea"], d["tr"] = ea_sb, tr_sb
                    d = info[j]
                    if l < 2:
                        if t % 4 == 0:
                            nbr = min(4, NT - t)
                            res_sb[0] = wpool.tile(
                                [P, 4, 256], BF, tag="r4x256", bufs=3, name="res_sb"
                            )
                            rdram = res0_dram if l == 0 else h1_dram
                            nc.sync.dma_start(
                                out=res_sb[0][:, :nbr, :],
                                in_=rdram[:].rearrange("(t p) c -> p t c", p=P)[
                                    :, t : t + nbr, :
                                ],
                            )
                        d["res"][t] = res_sb[0][:, t % 4, :]
                    else:
                        d["res"][t] = res2_all[:, t * 64 : (t + 1) * 64]
                    for k in range(K):
                        nc.gpsimd.indirect_dma_start(
                            out=d["hsg"][
                                :, ((t % 2) * K + k) * 256 : ((t % 2) * K + k + 1) * 256
                            ],
                            out_offset=None,
                            in_=hs_full[l][:],
                            in_offset=bass.IndirectOffsetOnAxis(
                                ap=srcs[:, t * K + k : t * K + k + 1], axis=0
                            ),
                        )
                    eoff = (t % 2) * KP
                    ST_all = wpool.tile([P, KP], BF, tag="ST", bufs=3)
                    nc.vector.tensor_scalar(
                        ST_all[:], d["tr"][:, eoff : eoff + KP], iotac_sb[:, 0:1],
                        None, op0=ALU.is_equal,
                    )
                    S_all = wpool.tile([P, KP], BF, tag="S", bufs=7)
                    nc.vector.tensor_tensor(
                        out=S_all[:].rearrange("p (k c) -> p k c", k=K),
                        in0=iotaK_sb[:].rearrange("p (k c) -> p k c", k=K),
                        in1=tgts[:, t * K : (t + 1) * K].to_broadcast((P, K, P)),
                        op=ALU.is_equal,
                    )
                    d["S"][t] = S_all
                    tinfo[t] = (ST_all, d)

                def msg(t):
                    ST_all, d = tinfo.pop(t)
                    j = t // 2
                    eoff = (t % 2) * KP
                    d["msgp"][t] = []
                    for jj in range(2):
                        msgp = psPair.tile([P, 512], F32, tag="pair", name="msgp")
                        cb = ((t % 2) * K + 2 * jj) * 256
                        nc.tensor.matmul(
                            out=msgp[:], lhsT=idb_sb[:],
                            rhs=d["hsg"][:, cb : cb + 512],
                            start=True, stop=False, skip_group_check=True,
                        )
                        for c in range(2):
                            k = 2 * jj + c
                            nc.tensor.matmul(
                                out=msgp[:, c * 256 : (c + 1) * 256],
                                lhsT=d["ea"][:, eoff + k * P : eoff + (k + 1) * P],
                                rhs=s["we"][:],
                                start=False, stop=False, skip_group_check=True,
                            )
                        for c in range(2):
                            k = 2 * jj + c
                            nc.tensor.matmul(
                                out=msgp[:, c * 256 : (c + 1) * 256],
                                lhsT=ST_all[:, k * P : (k + 1) * P],
                                rhs=ht_all[:, t * 256 : (t + 1) * 256],
                                start=False, stop=(c == 1), skip_group_check=True,
                            )
                        d["msgp"][t].append(msgp)

                def prelu(t):
                    j = t // 2
                    d = info[j]
                    for jj in range(2):
                        nc.scalar.activation(
                            d["lr2"][:, t % 2, jj * 512 : (jj + 1) * 512],
                            d["msgp"][t][jj][:],
                            AF.Prelu, alpha=0.2,
                        )
                    del d["msgp"][t]

                def alpha(j):
                    d = info[j]
                    scr2 = wpool.tile([P, 2048], BF, tag="scr2", bufs=1)
                    nc.vector.tensor_tensor(
                        out=scr2[:],
                        in0=d["lr2"][:].rearrange("p a b -> p (a b)"),
                        in1=a2_sb[:],
                        op=ALU.mult,
                    )
                    alph2 = spool.tile([P, 32], F32, tag="alph", bufs=2)
                    nc.vector.tensor_reduce(
                        out=alph2[:],
                        in_=scr2[:].rearrange("p (g d) -> p g d", d=64),
                        axis=mybir.AxisListType.X,
                        op=ALU.add,
                    )
                    expa2 = spool.tile([P, 32], F32, tag="expa", bufs=2)
                    nc.scalar.activation(expa2[:], alph2[:], AF.Exp)
                    d["expa"] = expa2

                def mid(j):
                    d = info[j]
                    d["waug"] = []
                    for i in range(2):
                        w_aug = wpool.tile(
                            [P, K * 260], BF, tag="waug", bufs=2, name="waug"
                        )
                        w4 = w_aug[:].rearrange("p (k h c) -> p k h c", k=K, c=65)
                        hs4 = d["hsg"][
                            :, i * K * 256 : (i + 1) * K * 256
                        ].rearrange("p (k h dd) -> p k h dd", k=K, dd=64)
                        e4 = d["expa"][:, i * 16 : (i + 1) * 16].rearrange(
                            "p (k h) -> p k h", k=K
                        )
                        nc.vector.tensor_tensor(
                            out=w4[:, :, :, 0:64],
                            in0=hs4[:],
                            in1=e4[:].to_broadcast((P, K, H, 64)),
                            op=ALU.mult,
                        )
                        nc.scalar.copy(w4[:, :, :, 64], e4)
                        d["waug"].append(w_aug)

                def tail_pe(j):
                    d = info[j]
                    d["agg"] = []
                    d["den"] = []
                    for i in range(2):
                        agg = psC.tile([P, 512], F32, tag="agg", name="agg")[:, 0:260]
                        for k in range(K):
                            nc.tensor.matmul(
                                out=agg,
                                lhsT=d["S"][2 * j + i][:, k * P : (k + 1) * P],
                                rhs=d["waug"][i][:, k * 260 : (k + 1) * 260],
                                start=(k == 0),
                                stop=(k == K - 1),
                            )
                        d["agg"].append(agg)
                    for i in range(2):
                        aggv = d["agg"][i].rearrange("p (h c) -> p h c", c=65)
                        den = spool.tile([P, 4], F32, tag="den", bufs=4)
                        nc.scalar.activation(den[:], aggv[:, :, 64], AF.Copy, bias=1e-8)
                        d["den"].append(den)

                def tail_vs(j):
                    d = info.pop(j)
                    for i in range(2):
                        tt = 2 * j + i
                        aggv = d["agg"][i].rearrange("p (h c) -> p h c", c=65)
                        rden = spool.tile([P, 4], F32, tag="rden", bufs=2)
                        nc.vector.reciprocal(rden[:], d["den"][i][:])
                        gat = wpool.tile([P, 256], F32, tag="gat", bufs=1)
                        nc.vector.scalar_tensor_tensor(
                            gat[:].rearrange("p (h dd) -> p h dd", h=4),
                            aggv[:, :, 0:64],
                            0.25 if l == 2 else 1.0,
                            rden[:].to_broadcast((P, 4, 64)),
                            op0=ALU.mult,
                            op1=ALU.mult,
                        )
                        if l == 2:
                            g64 = wpool.tile([P, 64], F32, tag="g64", bufs=2)
                            nc.vector.tensor_reduce(
                                out=g64[:],
                                in_=gat[:].rearrange("p (h dd) -> p dd h", h=4),
                                axis=mybir.AxisListType.X,
                                op=ALU.add,
                            )
                            zin = g64[:]
                        else:
                            zin = gat[:]
                        zslot = z_all[:, tt, :outd]
                        nc.vector.scalar_tensor_tensor(
                            zslot, zin, 1.0, d["res"][tt],
                            op0=ALU.mult, op1=ALU.add,
                            accum_out=st["s1"][:, tt : tt + 1],
                        )
                        junk = wpool.tile(
                            [P, 256], BF, tag="junk", bufs=1, name="junke"
                        )[:, :outd]
                        nc.scalar.activation(
                            junk, zslot, AF.Square,
                            accum_out=st["s2"][:, tt : tt + 1],
                        )

                for t in range(NT + 2):
                    if t % 2 == 0:
                        if t >= 4:
                            alpha(t // 2 - 2)
                        if t >= 6:
                            tail_pe(t // 2 - 3)
                    else:
                        if t >= 5:
                            mid(t // 2 - 2)
                        if t >= 7:
                            tail_vs(t // 2 - 3)
                    if t < NT:
                        front(t)
                    if 1 <= t <= NT:
                        msg(t - 1)
                    if 2 <= t <= NT + 1:
                        prelu(t - 2)
                    # fused next-layer f2a in 6-tile clusters
                    if l < 2 and t % 2 == 1 and (t // 2) % 3 == 2 and t >= 13:
                        f2a_block(2 * (t // 2 - 3) + 1, 6)
                NP = NT // 2
                alpha(NP - 1)
                tail_pe(NP - 2)
                mid(NP - 1)
                tail_vs(NP - 2)
                tail_pe(NP - 1)
                tail_vs(NP - 1)
                if l < 2:
                    while fstate["tf"] < NT:
                        f2a_block(NT - 1, 8)

            if DEBUG:
                nc.sync.dma_start(out=dbg_z0[:], in_=z_all[:, :, 0:64])
            if DEBUG:
                nc.sync.dma_start(out=dbg_ht[:], in_=ht_all[:])
            for l in range(3):
                with nc.named_scope(f"edge{l}"):
                    edge_f1(l)
                    if l == 2:
                        sqrt_batch(3, LW[2]["outd"])
                if DEBUG and l == 0:
                    nc.sync.dma_start(out=dbg_z1[:], in_=z_all[:])
                if l == 2:
                    with nc.named_scope("f2a3"):
                        f2a(3)

            sig = ppool.tile([P, NT], F32)
            nc.scalar.activation(sig[:], scores[:], AF.Sigmoid, bias=bh2_val)
            nc.sync.dma_start(out=out[:], in_=sig[:])
    return nc


# ---------------------------------------------------------------- host prep
def _balance_nodes(tgt):
    """Degree-balanced assignment of nodes to NCORES*NT tiles of <=128 slots.
    Returns (gtile[node], slot[node], K)."""
    import heapq

    NTILES = NCORES * NT
    deg = np.bincount(tgt, minlength=N)
    order = np.argsort(-deg, kind="stable")
    gtile = np.empty(N, np.int32)
    slot = np.empty(N, np.int32)
    count = np.zeros(NTILES, np.int32)
    load = np.zeros(NTILES, np.int64)
    heap = [(0, t) for t in range(NTILES)]
    heapq.heapify(heap)
    for node in order:
        while True:
            ld, t = heapq.heappop(heap)
            if count[t] < P and ld == load[t]:
                break
        gtile[node] = t
        slot[node] = count[t]
        count[t] += 1
        load[t] += deg[node]
        if count[t] < P:
            heapq.heappush(heap, (int(load[t]), t))
    K = int(np.ceil(load.max() / P))
    return gtile, slot, K


def _prep(inputs):
    ei = np.asarray(inputs["edge_index"]).astype(np.int64)
    src, tgt = ei[0], ei[1]
    ea = np.asarray(inputs["edge_attr"], np.float32)

    gtile, slot, K = _balance_nodes(tgt)
    core_of = gtile // NT
    lt_of = gtile % NT

    lt = lt_of.astype(np.int64)
    chunk = np.searchsorted(np.array(CHT[1:-1]), lt, side="right")
    chrows = np.array(CHROWS)[chunk]
    chbase = np.array(CHBASE)[chunk]
    chtile0 = np.array(CHT[:-1])[chunk]
    row_id = chbase + core_of.astype(np.int64) * chrows + (lt - chtile0) * P + slot

    NTK = NT * K
    ES = NTK * P

    e_core = core_of[tgt]
    e_lt = lt_of[tgt]
    e_p = slot[tgt]  # target's slot within its tile
    order = np.lexsort((e_lt, e_core))
    src_s = src[order]
    ea_s = ea[order]
    e_core_s, e_lt_s, e_p_s = e_core[order], e_lt[order], e_p[order]

    grp = e_core_s * NT + e_lt_s
    idx_in_grp = np.zeros(len(grp), np.int64)
    _, first_pos, cnt = np.unique(grp, return_index=True, return_counts=True)
    for fp, c in zip(first_pos, cnt):
        idx_in_grp[fp : fp + c] = np.arange(c)
    assert cnt.max() <= K * P, (cnt.max(), K)

    src_cols = np.zeros((NCORES, P, NTK), np.int32)
    tgt_cols = np.full((NCORES, P, NTK), -1.0, np.float32)
    tgt_rows = np.full((NCORES, 1, ES), -1.0, np.float32)
    ea_T = np.zeros((NCORES, 16, ES), np.float32)
    eslot = e_lt_s * (K * P) + idx_in_grp
    col = eslot // P
    row = eslot % P
    src_cols[e_core_s, row, col] = row_id[src_s].astype(np.int32)
    tgt_cols[e_core_s, row, col] = e_p_s.astype(np.float32)
    tgt_rows[e_core_s, 0, eslot] = e_p_s.astype(np.float32)
    ea_T[e_core_s[:, None], np.arange(ED)[None, :], eslot[:, None]] = ea_s

    x = np.asarray(inputs["x"], np.float32)
    x_T = np.zeros((NCORES, 384, NPAD), np.float32)
    pos = lt * P + slot  # position within core [0, NPAD)
    x_T[core_of, :FN, pos] = x
    x_T[core_of, FN, pos] = 1.0  # ones-row carries ctx@Wp+bp via wp1

    rep = lambda v: np.broadcast_to(
        np.asarray(v, np.float32)[None, :], (P, len(np.asarray(v)))
    ).copy()
    bf = lambda a: np.asarray(a).astype(ml_dtypes.bfloat16)

    Wp = np.asarray(inputs["Wp"], np.float32)
    cb = (
        np.asarray(inputs["context_vector"], np.float32) @ Wp[FN:]
        + np.asarray(inputs["bp"], np.float32)
    )
    wp1 = np.zeros((384, 64), np.float32)
    wp1[:FN] = Wp[:FN]
    wp1[FN] = cb
    wp1 = wp1.astype(ml_dtypes.bfloat16)

    common = {
        "wp1": wp1,
        "iota2d": np.broadcast_to(
            np.arange(P, dtype=np.float32)[None, :], (P, P)
        ).astype(ml_dtypes.bfloat16),
        "iota_col": np.arange(P, dtype=np.float32)[:, None].copy(),
        "ident": np.eye(P, dtype=np.float32).astype(ml_dtypes.bfloat16),
        "wh1": np.asarray(inputs["Wh1"], np.float32),
        "bh1_rep": rep(inputs["bh1"]),
        "wh2_rep": rep(np.asarray(inputs["Wh2"], np.float32)[:, 0]),
    }
    g_in = np.asarray(inputs["g_in"], np.float32)
    b_in = np.asarray(inputs["b_in"], np.float32)
    for l in range(3):
        sfx = str(l)
        ws = np.asarray(inputs["Ws" + sfx], np.float32)
        wt = np.asarray(inputs["Wt" + sfx], np.float32)
        wswt = np.concatenate([ws, wt], axis=1)
        if l == 0:
            wswt = np.concatenate(
                [g_in[:, None] * wswt, (b_in @ wswt)[None, :]], axis=0
            )
        common[f"wswt{l}"] = bf(wswt)
        we = np.zeros((16, 256), np.float32)
        we[:ED] = np.asarray(inputs["We" + sfx], np.float32)
        common[f"we{l}"] = bf(we)
        a1 = np.asarray(inputs["A" + sfx], np.float32).reshape(-1)
        common[f"a_rep2_{l}"] = bf(rep(np.tile(a1, 2 * K)))
        if l != 1:
            skw = np.asarray(inputs[f"Sk{l}W"], np.float32)
            if l == 0:
                skw = np.concatenate(
                    [
                        g_in[:, None] * skw,
                        (b_in @ skw + np.asarray(inputs["Sk0b"], np.float32))[
                            None, :
                        ],
                    ],
                    axis=0,
                )
            common[f"skw{l}"] = bf(skw)
            common[f"skb_rep{l}"] = bf(rep(inputs[f"Sk{l}b"]))
        common[f"gn_rep{l}"] = bf(rep(inputs["gn" + sfx]))
        common[f"bn_rep{l}"] = bf(rep(inputs["bn" + sfx]))

    in_maps = []
    for c in range(NCORES):
        m = dict(common)
        m["x_T"] = x_T[c].astype(ml_dtypes.bfloat16)
        m["src_c"] = src_cols[c]
        m["tgt_c"] = tgt_cols[c].astype(ml_dtypes.bfloat16)
        m["tgt_r"] = tgt_rows[c].astype(ml_dtypes.bfloat16)
        m["ea_T"] = ea_T[c].astype(ml_dtypes.bfloat16)
        in_maps.append(m)
    bh2_val = float(np.asarray(inputs["bh2"]).reshape(-1)[0])
    return in_maps, K, bh2_val, (core_of, lt_of, slot)


def kernel(**inputs):
    in_maps, K, bh2_val, (core_of, lt_of, slot) = _prep(inputs)
    nc = build_nc(K, bh2_val)
    res = run_bass_kernel_spmd(
        nc, in_maps, core_ids=list(range(NCORES)), trace=TRACE
    )
    LAST_RESULT["exec_time_ns"] = res.exec_time_ns
    LAST_RESULT["res"] = res
    if DEBUG:
        LAST_RESULT["dbg"] = res.results
        LAST_RESULT["layout"] = (core_of, lt_of, slot)
    outs = np.stack([res.results[c]["out"] for c in range(NCORES)])  # [8, P, NT]
    return outs[core_of, slot, lt_of].astype(np.float32)


# revision 42
# speedup vs baseline: 1.0581x; 1.0151x over previous
"""Bass/Trainium2 kernel for nn_MemoryGAT (3-layer GATv2 + MLP head), 8 NeuronCores.

Nodes are degree-balanced into 8x98 tiles of 128 (K edge-tiles per node tile,
K~4). hs rows are written straight into a device-shared hs_full buffer with
batched indirect scatters; a 1-element AllGather acts as the cross-core
barrier (no bulk collective). Edge loop gathers hs[src] in multi-tile batched
indirect DMAs (SWDGE fixed cost amortized), builds the one-hot S / S^T
selection masks on DVE+Pool without PE transposes, accumulates msg in paired
PSUM banks, and keeps LN stats via accum_out. z stays in SBUF end to end.
"""

import sys
import types

sys.path.insert(0, "/opt/trn_rl_repo")

import ml_dtypes
import numpy as np
import orjson

# ---------------------------------------------------------------- shims

_counter = [0]


def _legalize_module(m, maxw=1):
    """This walrus build accepts only ONE sync-wait per instruction; hoist
    overflow waits onto NoOps inserted just before, on the same engine."""
    for f in m.get("functions", []):
        for b in f.get("blocks", []):
            insts = b.get("instructions")
            if not insts:
                continue
            out = []
            for inst in insts:
                si = inst.get("sync_info")
                waits = (si or {}).get("on_wait") or []
                if si is not None and len(waits) > maxw:
                    keep = waits[-maxw:]
                    extra = waits[: len(waits) - maxw]
                    for j in range(0, len(extra), maxw):
                        _counter[0] += 1
                        out.append(
                            {
                                "name": f"ant-wsplit-{_counter[0]}",
                                "opcode": "NoOp",
                                "engine": inst.get("engine"),
                                "ins": [],
                                "outs": [],
                                "sync_info": {
                                    "on_wait": extra[j : j + maxw],
                                    "on_update": [],
                                },
                            }
                        )
                    si["on_wait"] = keep
                out.append(inst)
            b["instructions"] = out
    return m


def _install_shims():
    import antenv

    if "antenv.axon_hooks" not in sys.modules:
        try:
            from trn_agent_boot.trn_boot import _ntff_profile_via_ctypes

            hooks = types.ModuleType("antenv.axon_hooks")
            hook = _ntff_profile_via_ctypes("/opt/axon/libaxon_pjrt.so")
            hooks.get_axon_ntff_profile_hook = lambda: hook
            hooks.set_axon_ntff_profile_hook = lambda h: None
            sys.modules["antenv.axon_hooks"] = hooks
            antenv.axon_hooks = hooks
        except Exception:
            pass

    import concourse.bass as bass
    from concourse import bass_utils

    bass_utils.upload_artifacts = lambda tmpdir: tmpdir

    if not getattr(bass.Bass, "_waitfix_installed", False):
        base = bass.Bass.to_json_bytes

        def patched(self):
            return orjson.dumps(_legalize_module(orjson.loads(base(self))))

        bass.Bass.to_json_bytes = patched
        bass.Bass._waitfix_installed = True


_install_shims()

import concourse.bass as bass
import concourse.tile as tile
from concourse import mybir
from concourse.bass_utils import run_bass_kernel_spmd

F32 = mybir.dt.float32
BF = mybir.dt.bfloat16
AF = mybir.ActivationFunctionType
ALU = mybir.AluOpType

# ---------------------------------------------------------------- sizes
N = 100_000
E = 400_000
FN = 267
DC = 256
H, D = 4, 64
HD = 256
ED = 11
NCORES = 8
P = 128
NT = 98
NPAD = NT * P  # 12544
NFULL = NCORES * NPAD
# AllGather chunk boundaries (in node tiles) and hs_full region bases
CHT = [0, 40, 72, 92, 98]
NCH = len(CHT) - 1
CHROWS = [(CHT[i + 1] - CHT[i]) * P for i in range(NCH)]
CHBASE = [0]
for i in range(NCH - 1):
    CHBASE.append(CHBASE[-1] + NCORES * CHROWS[i])

TRACE = False
DEBUG = False
LAST_RESULT = {}


# ---------------------------------------------------------------- builder
def build_nc(K, bh2_val):
    NTK = NT * K
    ES = NTK * P
    KP = K * P

    nc = bass.Bass()
    dp = nc.declare_dram_parameter

    x_T = dp("x_T", [384, NPAD], BF, isOutput=False)
    src_c = dp("src_c", [P, NTK], mybir.dt.int32, isOutput=False)
    tgt_c = dp("tgt_c", [P, NTK], BF, isOutput=False)
    tgt_r = dp("tgt_r", [1, ES], BF, isOutput=False)
    ea_T = dp("ea_T", [16, ES], BF, isOutput=False)
    wp1 = dp("wp1", [384, 64], BF, isOutput=False)
    iota2d = dp("iota2d", [P, P], BF, isOutput=False)
    iota_col = dp("iota_col", [P, 1], F32, isOutput=False)
    ident = dp("ident", [P, P], BF, isOutput=False)
    wh1 = dp("wh1", [64, 32], F32, isOutput=False)
    bh1_rep = dp("bh1_rep", [P, 32], F32, isOutput=False)
    wh2_rep = dp("wh2_rep", [P, 32], F32, isOutput=False)

    LW = []
    for l, ind in ((0, 65), (1, 256), (2, 256)):
        d = {"ind": ind, "outd": 64 if l == 2 else 256}
        d["wswt"] = dp(f"wswt{l}", [ind, 512], BF, isOutput=False)
        d["we"] = dp(f"we{l}", [16, 256], BF, isOutput=False)
        d["a_rep2"] = dp(f"a_rep2_{l}", [P, 2 * KP * 2], BF, isOutput=False)
        if l != 1:
            d["skw"] = dp(f"skw{l}", [ind, d["outd"]], BF, isOutput=False)
            d["skb_rep"] = dp(f"skb_rep{l}", [P, d["outd"]], BF, isOutput=False)
        d["gn_rep"] = dp(f"gn_rep{l}", [P, d["outd"]], BF, isOutput=False)
        d["bn_rep"] = dp(f"bn_rep{l}", [P, d["outd"]], BF, isOutput=False)
        LW.append(d)

    out = dp("out", [P, NT], F32, isOutput=True)
    if DEBUG:
        dbg_z0 = dp("dbg_z0", [P, NT, 64], BF, isOutput=True)
        dbg_ht = dp("dbg_ht", [P, NT * 256], BF, isOutput=True)
        dbg_z1 = dp("dbg_z1", [P, NT, 256], BF, isOutput=True)
        dbg_lr = dp("dbg_lr", [P, 2, 1024], BF, isOutput=True)
        dbg_st = dp("dbg_st", [P, 512], BF, isOutput=True)

    hs_shard = [nc.dram_tensor(f"hs_shard{l}", [NPAD, 256], BF) for l in range(3)]
    hs_full = [
        nc.dram_tensor(f"hs_full{l}", [NFULL, 256], BF, addr_space="Shared")
        for l in range(3)
    ]
    res0_dram = nc.dram_tensor("res0_dram", [NPAD, 256], BF)
    h1_dram = nc.dram_tensor("h1_dram", [NPAD, 256], BF)

    with tile.TileContext(nc) as tc:
        with (
            tc.tile_pool(name="const", bufs=1) as cpool,
            tc.tile_pool(name="work", bufs=2) as wpool,
            tc.tile_pool(name="small", bufs=2) as spool,
            tc.tile_pool(name="persist", bufs=1) as ppool,
            tc.tile_pool(name="psPair", bufs=4, space="PSUM") as psPair,
            tc.tile_pool(name="psB", bufs=2, space="PSUM") as psB,
            tc.tile_pool(name="psC", bufs=2, space="PSUM") as psC,
        ):
            for v in {1e-5, 1e-8, float(bh2_val)}:
                ct = cpool.tile([P, 1], F32, tag=f"k{v}", name=f"k{_counter[0]}")
                _counter[0] += 1
                nc.vector.memset(ct[:], v)
                nc.const_aps.aps[(F32, float(v))] = ct[:]

            _cn = [0]

            def c_load(ap, shape, dt=F32):
                _cn[0] += 1
                t = cpool.tile(shape, dt, tag=f"c{_cn[0]}", name=f"c{_cn[0]}")
                nc.sync.dma_start(out=t[:], in_=ap[:])
                return t

            def c_load_chunks(ap, kk, ck, n, dt=F32):
                _cn[0] += 1
                t = cpool.tile([kk, ck * n], dt, tag=f"c{_cn[0]}", name=f"c{_cn[0]}")
                for c in range(ck):
                    nc.sync.dma_start(
                        out=t[:, c * n : (c + 1) * n],
                        in_=ap[c * kk : (c + 1) * kk, :],
                    )
                return t

            iota_sb = c_load(iota2d, [P, P], BF)
            idb_sb = c_load(ident, [P, P], BF)
            iotac_sb = c_load(iota_col, [P, 1], F32)
            iotaK_sb = cpool.tile([P, KP], BF, tag="iotaK", name="iotaK")
            for k in range(K):
                nc.vector.tensor_copy(iotaK_sb[:, k * P : (k + 1) * P], iota_sb[:])
            ones1p = cpool.tile([1, P], BF, tag="ones1p", name="ones1p")
            nc.vector.memset(ones1p[:], 1.0)
            wp1_sb = c_load_chunks(wp1, P, 3, 64, BF)
            wh1_sb = c_load(wh1, [64, 32])
            bh1_sb = c_load(bh1_rep, [P, 32])
            wh2_sb = c_load(wh2_rep, [P, 32])
            lws = []
            for l, d in enumerate(LW):
                s = {}
                ck = max(d["ind"] // P, 1)
                kk = min(d["ind"], P)
                s["wswt"] = c_load_chunks(d["wswt"], kk, ck, 512, BF)
                s["we"] = c_load(d["we"], [16, 256], BF)
                if "skw" in d:
                    s["skw"] = c_load_chunks(d["skw"], kk, ck, d["outd"], BF)
                    s["skb"] = c_load(d["skb_rep"], [P, d["outd"]], BF)
                s["gn"] = c_load(d["gn_rep"], [P, d["outd"]], BF)
                s["bn"] = c_load(d["bn_rep"], [P, d["outd"]], BF)
                s["ck"], s["kk"] = ck, kk
                lws.append(s)

            srcs = ppool.tile([P, NTK], mybir.dt.int32)
            nc.sync.dma_start(out=srcs[:], in_=src_c[:])
            tgts = ppool.tile([P, NTK], BF)
            nc.sync.dma_start(out=tgts[:], in_=tgt_c[:])

            ht_all = ppool.tile([P, NT * 256], BF)
            z_all = ppool.tile([P, NT, 256], BF)
            res2_all = ppool.tile([P, NT * 64], BF)
            scores = ppool.tile([P, NT], F32)

            # one shared LN-stat set; stages are strictly phased so WAR
            # deps keep this safe
            _st = {}
            for nm in ("s1", "s2", "m", "va", "rstd"):
                _st[nm] = ppool.tile([P, NT], F32, tag=f"st{nm}", name=f"st{nm}")
            stats = [_st] * 4

            def sqrt_batch(i, dim, t0=0, t1=NT):
                st = stats[i]
                sl = slice(t0, t1)
                nc.vector.tensor_scalar_mul(st["m"][:, sl], st["s1"][:, sl], 1.0 / dim)
                nc.vector.tensor_scalar_mul(st["va"][:, sl], st["s2"][:, sl], 1.0 / dim)
                nm2 = spool.tile([P, NT], F32, tag="nm2", name="nm2")[:, sl]
                nc.vector.scalar_tensor_tensor(
                    nm2, st["m"][:, sl], -1.0, st["m"][:, sl],
                    op0=ALU.mult, op1=ALU.mult,
                )
                nc.vector.tensor_add(st["va"][:, sl], st["va"][:, sl], nm2)
                sd = spool.tile([P, NT], F32, tag="sd", name="sd")[:, sl]
                nc.scalar.activation(sd, st["va"][:, sl], AF.Sqrt, bias=1e-5)
                nc.vector.reciprocal(st["rstd"][:, sl], sd)
                nc.vector.scalar_tensor_tensor(
                    st["va"][:, sl], st["m"][:, sl], -1.0, st["rstd"][:, sl],
                    op0=ALU.mult, op1=ALU.mult,
                )

            def ag_chunk(l, c):
                nc.gpsimd.collective_compute(
                    "AllGather",
                    ALU.bypass,
                    ins=[hs_shard[l][CHT[c] * P : CHT[c + 1] * P, :]],
                    outs=[
                        hs_full[l][CHBASE[c] : CHBASE[c] + NCORES * CHROWS[c], :]
                    ],
                    replica_groups=[list(range(NCORES))],
                )

            # ---------------- fused phase 0 + f2a0 pipeline -------------------
            # p0(t): x@Wp -> gelu -> z0, stats; sqrt per 4-block;
            # f2a0 stages trail: hn(t-6) | transpose(t-7) | proj(t-8) | copies(t-9)
            def p0_f2a0():
                st = stats[0]
                s = lws[0]
                hns = {}
                lhss = {}
                hshts = {}
                rps = {}
                stag_hs = [None]
                stag_res = [None]
                LAG = 6
                for step in range(NT + LAG + 3):
                    if step < NT:
                        t = step
                        if t % 4 == 0:
                            nbt = min(4, NT - t)
                            xt = wpool.tile([P, 3, 4 * P], BF, tag="hsg", bufs=3)
                            for c in range(3):
                                nc.sync.dma_start(
                                    out=xt[:, c, : nbt * P],
                                    in_=x_T[
                                        c * P : (c + 1) * P, t * P : (t + nbt) * P
                                    ],
                                )
                        xoff = (t % 4) * P
                        h0p = psPair.tile([P, 512], F32, tag="pair", name="h0p")[:, 0:64]
                        for c in range(3):
                            nc.tensor.matmul(
                                out=h0p,
                                lhsT=xt[:, c, xoff : xoff + P],
                                rhs=wp1_sb[:, c * 64 : (c + 1) * 64],
                                start=(c == 0),
                                stop=(c == 2),
                            )
                        zsl = z_all[:, t, 0:64]
                        nc.scalar.activation(
                            zsl, h0p, AF.Gelu, accum_out=st["s1"][:, t : t + 1]
                        )
                        junk = wpool.tile([P, 256], BF, tag="junk", bufs=1, name="junk0")[:, 0:64]
                        nc.vector.scalar_tensor_tensor(
                            junk, zsl, 1.0, zsl,
                            op0=ALU.mult, op1=ALU.mult,
                            accum_out=st["s2"][:, t : t + 1],
                        )
                        if t % 4 == 3 or t == NT - 1:
                            sqrt_batch(0, 64, t - (t % 4), t + 1)
                    # stage A: hn(t) via identity
                    tA = step - LAG
                    if 0 <= tA < NT:
                        hn = wpool.tile([P, 256], BF, tag="hn", bufs=6, name="hn0")[:, 0:64]
                        nc.scalar.activation(
                            hn, z_all[:, tA, :64], AF.Identity,
                            bias=st["va"][:, tA : tA + 1],
                            scale=st["rstd"][:, tA : tA + 1],
                        )
                        hns[tA] = hn
                    # stage B: transpose(t-LAG-1)
                    tB = step - LAG - 1
                    if 0 <= tB < NT:
                        hn = hns.pop(tB)
                        h0t = wpool.tile([65, P], BF, tag="h0t", bufs=4)
                        if tB < 4:
                            nc.vector.memset(h0t[64:65, :], 1.0)
                        trp = psB.tile([P, 2, P], BF, tag="tr", bufs=2)
                        nc.tensor.transpose(
                            out=trp[0:64, 0, :], in_=hn, identity=idb_sb[:]
                        )
                        nc.vector.tensor_copy(h0t[0:64, :], trp[0:64, 0, :])
                        lhss[tB] = h0t
                    # stage C: projections(t-LAG-2)
                    tC = step - LAG - 2
                    if 0 <= tC < NT:
                        h0t = lhss.pop(tC)
                        hsht = psPair.tile([P, 512], F32, tag="pair", name="hsht")
                        nc.tensor.matmul(
                            out=hsht[:], lhsT=h0t[:, :], rhs=s["wswt"][:, 0:512],
                            start=True, stop=True,
                        )
                        hshts[tC] = hsht
                        rp = psC.tile([P, 512], F32, tag="agg", name="rp")[:, 0:256]
                        nc.tensor.matmul(
                            out=rp, lhsT=h0t[:, :], rhs=s["skw"][:, 0:256],
                            start=True, stop=True,
                        )
                        rps[tC] = rp
                    # stage D: copies + stores(t-LAG-3)
                    tD = step - LAG - 3
                    if 0 <= tD < NT:
                        t = tD
                        if t % 4 == 0:
                            stag_hs[0] = wpool.tile(
                                [P, 4, 256], BF, tag="stag_hs", bufs=2, name="shs"
                            )
                            stag_res[0] = wpool.tile(
                                [P, 4, 256], BF, tag="r4x256", bufs=3, name="sres"
                            )
                        hsht = hshts.pop(t)
                        nc.vector.tensor_copy(stag_hs[0][:, t % 4, :], hsht[:, 0:256])
                        nc.scalar.copy(
                            ht_all[:, t * 256 : (t + 1) * 256], hsht[:, 256:512]
                        )
                        rp = rps.pop(t)
                        nc.vector.tensor_copy(stag_res[0][:, t % 4, :], rp)
                        if t % 4 == 3 or t == NT - 1:
                            t0 = t - (t % 4)
                            nbt = t - t0 + 1
                            nc.sync.dma_start(
                                out=hs_shard[0][:].rearrange("(t p) c -> p t c", p=P)[
                                    :, t0 : t0 + nbt, :
                                ],
                                in_=stag_hs[0][:, :nbt, :],
                            )
                            nc.sync.dma_start(
                                out=res0_dram[:].rearrange("(t p) c -> p t c", p=P)[
                                    :, t0 : t0 + nbt, :
                                ],
                                in_=stag_res[0][:, :nbt, :],
                            )
                        for c in range(NCH):
                            if t == CHT[c + 1] - 1:
                                ag_chunk(0, c)

            with nc.named_scope("p0"):
                p0_f2a0()

            # ---------------- F2A(l): finalize h_l, project, scatter+barrier
            # Software-pipelined: hn(t) | transpose(t-1) | proj+copies(t-2)
            def f2a(l):
                st = stats[l]
                ind = 64 if l == 0 else (256 if l < 3 else 64)
                s = lws[l] if l < 3 else None
                hns = {}
                lhss = {}
                h3Ts = {}
                hshts = {}
                rps = {}
                stag_h1 = None
                stag_hs = None
                stag_res = None
                for step in range(NT + 3):
                    # ---- stage A: produce hn(step)
                    if step < NT:
                        t = step
                        if l == 1 and t % 4 == 0:
                            stag_h1 = wpool.tile(
                                [P, 4, 256], BF, tag="sh1", bufs=2, name="sh1"
                            )
                        if l == 0:
                            hn = wpool.tile(
                                [P, 256], BF, tag="hn", bufs=6, name="hn0"
                            )[:, :ind]
                            nc.scalar.activation(
                                hn, z_all[:, t, :ind], AF.Identity,
                                bias=st["va"][:, t : t + 1],
                                scale=st["rstd"][:, t : t + 1],
                            )
                        else:
                            if l == 1:
                                hn = stag_h1[:, t % 4, :]
                            else:
                                hn = wpool.tile(
                                    [P, 256], BF, tag="hn", bufs=6, name="hnl"
                                )[:, :ind]
                            g_sb = lws[l - 1]["gn"]
                            b_sb = lws[l - 1]["bn"]
                            t1 = wpool.tile(
                                [P, 256], F32, tag="t1", bufs=1, name="t1"
                            )[:, :ind]
                            nc.vector.scalar_tensor_tensor(
                                t1, z_all[:, t, :ind], st["m"][:, t : t + 1],
                                g_sb[:, :ind], op0=ALU.subtract, op1=ALU.mult,
                            )
                            u = wpool.tile(
                                [P, 256], BF, tag="u", bufs=6, name="u"
                            )[:, :ind]
                            nc.vector.scalar_tensor_tensor(
                                u, t1, st["rstd"][:, t : t + 1], b_sb[:, :ind],
                                op0=ALU.mult, op1=ALU.add,
                            )
                            nc.scalar.activation(hn, u, AF.Gelu)
                        hns[t] = hn
                        if l == 1 and (t % 4 == 3 or t == NT - 1):
                            t0 = t - (t % 4)
                            nc.sync.dma_start(
                                out=h1_dram[:].rearrange("(t p) c -> p t c", p=P)[
                                    :, t0 : t + 1, :
                                ],
                                in_=stag_h1[:, : t - t0 + 1, :],
                            )
                    # ---- stage B: transpose hn(step-1)
                    if 1 <= step <= NT:
                        t = step - 1
                        hn = hns[t]
                        if l == 3:
                            trp = psB.tile([P, 2, P], BF, tag="tr", bufs=2)
                            nc.tensor.transpose(
                                out=trp[0:64, 0, :], in_=hn, identity=idb_sb[:]
                            )
                            h3T = wpool.tile([64, P], F32, tag="h3T", bufs=3)
                            nc.scalar.copy(h3T[:], trp[0:64, 0, :])
                            h3Ts[t] = h3T
                        elif l == 0:
                            h0t = wpool.tile([65, P], BF, tag="h0t", bufs=4)
                            if t < 4:
                                nc.vector.memset(h0t[64:65, :], 1.0)
                            trp = psB.tile([P, 2, P], BF, tag="tr", bufs=2)
                            nc.tensor.transpose(
                                out=trp[0:64, 0, :], in_=hn, identity=idb_sb[:]
                            )
                            nc.vector.tensor_copy(h0t[0:64, :], trp[0:64, 0, :])
                            lhss[t] = [h0t[:, :]]
                        else:
                            trp = psB.tile([P, 2, P], BF, tag="tr", bufs=2)
                            for c in range(2):
                                nc.tensor.transpose(
                                    out=trp[:, c, :],
                                    in_=hn[:, c * P : (c + 1) * P],
                                    identity=idb_sb[:],
                                )
                            hnT = wpool.tile([P, 2, P], BF, tag="hnT", bufs=6)
                            nc.vector.tensor_copy(hnT[:], trp[:])
                            lhss[t] = [hnT[:, c, :] for c in range(2)]
                    # ---- stage C: project for tile step-2
                    if 2 <= step <= NT + 1:
                        t = step - 2
                        if l == 3:
                            pass
                        else:
                            lhs = lhss.pop(t)
                            hns.pop(t, None)
                            ck = s["ck"]
                            hsht = psPair.tile([P, 512], F32, tag="pair", name="hsht")
                            for c in range(ck):
                                nc.tensor.matmul(
                                    out=hsht[:],
                                    lhsT=lhs[c],
                                    rhs=s["wswt"][:, c * 512 : (c + 1) * 512],
                                    start=(c == 0),
                                    stop=(c == ck - 1),
                                )
                            hshts[t] = hsht
                            if l != 1:
                                outd = LW[l]["outd"]
                                rp = psC.tile([P, 512], F32, tag="agg", name="rp")[
                                    :, :outd
                                ]
                                for c in range(ck):
                                    nc.tensor.matmul(
                                        out=rp,
                                        lhsT=lhs[c],
                                        rhs=s["skw"][:, c * outd : (c + 1) * outd],
                                        start=(c == 0),
                                        stop=(c == ck - 1),
                                    )
                                rps[t] = rp
                    # ---- stage D: copies + stores for tile step-3
                    if step < 3:
                        continue
                    t = step - 3
                    if l == 3:
                        h3T = h3Ts.pop(t)
                        sp1 = psC.tile([P, 512], F32, tag="agg", name="sp1")[:, :32]
                        nc.tensor.matmul(
                            out=sp1, lhsT=h3T[:], rhs=wh1_sb[:], start=True, stop=True
                        )
                        u1 = wpool.tile([P, 32], F32, tag="u1", bufs=2)
                        nc.vector.tensor_add(u1[:], sp1, bh1_sb[:])
                        g1 = wpool.tile([P, 32], F32, tag="g1", bufs=2)
                        nc.scalar.activation(g1[:], u1[:], AF.Gelu)
                        j32 = wpool.tile([P, 32], BF, tag="j32", bufs=2)
                        nc.vector.scalar_tensor_tensor(
                            j32[:], g1[:], 1.0, wh2_sb[:],
                            op0=ALU.mult, op1=ALU.mult,
                            accum_out=scores[:, t : t + 1],
                        )
                        hns.pop(t, None)
                        continue
                    if t % 4 == 0:
                        stag_hs = wpool.tile(
                            [P, 4, 256], BF, tag="stag_hs", bufs=2, name="shs"
                        )
                        if l == 0:
                            stag_res = wpool.tile(
                                [P, 4, 256], BF, tag="r4x256", bufs=3, name="sres"
                            )
                    hsht = hshts.pop(t)
                    nc.scalar.copy(stag_hs[:, t % 4, :], hsht[:, 0:256])
                    nc.scalar.copy(
                        ht_all[:, t * 256 : (t + 1) * 256], hsht[:, 256:512]
                    )
                    if l != 1:
                        rp = rps.pop(t)
                        if l == 0:
                            nc.scalar.copy(stag_res[:, t % 4, :], rp)
                        else:
                            nc.vector.scalar_tensor_tensor(
                                res2_all[:, t * 64 : (t + 1) * 64], rp, 1.0,
                                s["skb"][:], op0=ALU.mult, op1=ALU.add,
                            )
                    # batched stores + AG chunks
                    if t % 4 == 3 or t == NT - 1:
                        t0 = t - (t % 4)
                        nbt = t - t0 + 1
                        nc.sync.dma_start(
                            out=hs_shard[l][:].rearrange("(t p) c -> p t c", p=P)[
                                :, t0 : t0 + nbt, :
                            ],
                            in_=stag_hs[:, :nbt, :],
                        )
                        if l == 0:
                            nc.sync.dma_start(
                                out=res0_dram[:].rearrange("(t p) c -> p t c", p=P)[
                                    :, t0 : t0 + nbt, :
                                ],
                                in_=stag_res[:, :nbt, :],
                            )
                    for c in range(NCH):
                        if t == CHT[c + 1] - 1:
                            ag_chunk(l, c)

            # ---------------- edge + F1 loop --------------------------------
            # Deep pipeline: every cross-engine dep is >=1 tile old.
            #  front(t):   loads, 4 gathers(t) [Pool], ST/S masks(t) [V]
            #  msg(t-1):   10 matmuls [PE] + 2 Prelu(t-2) [S]
            #  alpha(j):   at t=2j+4: scr2/alph2 [V], exp [S]
            #  mid(j):     at t=2j+5: w4 [V], w4col [S]
            #  tail_pe(j): at t=2j+6: agg [PE], den [S]
            #  tail_vs(j): at t=2j+7: rden/gat/z [V], square [S]
            def edge_f1(l):
                s = lws[l]
                outd = LW[l]["outd"]
                st = stats[l + 1]
                a2_sb = wpool.tile(
                    [P, 2 * KP * 2], BF, tag="arep", bufs=1, name=f"arep{l}"
                )
                nc.sync.dma_start(out=a2_sb[:], in_=LW[l]["a_rep2"][:])
                info = {}
                tinfo = {}
                res_sb = [None]
                # ---- fused next-layer f2a state (l2 = l+1, only for l < 2) ----
                l2 = l + 1
                s2 = lws[l2] if l2 < 3 else None
                fstate = {"tf": 0, "sq": 0, "stag_hs": None, "stag_h1": None}

                def f2a_block(zready, nmax):
                    # finalize LN stats in 4-blocks as they become available
                    while fstate["sq"] + 4 <= zready + 1 or (
                        zready == NT - 1 and fstate["sq"] < NT
                    ):
                        b0 = fstate["sq"]
                        b1 = min(b0 + 4, NT)
                        sqrt_batch(l2, LW[l]["outd"], b0, b1)
                        fstate["sq"] = b1
                    n = min(nmax, fstate["sq"] - fstate["tf"])
                    if n <= 0:
                        return
                    t0f = fstate["tf"]
                    tiles = range(t0f, t0f + n)
                    stf = stats[l2]
                    hnb = {}
                    sh1_map = {}
                    # V: LN finalize
                    for tf in tiles:
                        if l2 == 1 and tf % 4 == 0:
                            fstate["stag_h1"] = wpool.tile(
                                [P, 4, 256], BF, tag="sh1", bufs=2, name="sh1"
                            )
                        if l2 == 1:
                            sh1_map[tf] = fstate["stag_h1"]
                        t1 = wpool.tile([P, 256], F32, tag="t1", bufs=1, name="t1")
                        nc.vector.scalar_tensor_tensor(
                            t1[:], z_all[:, tf, :], stf["m"][:, tf : tf + 1],
                            lws[l2 - 1]["gn"][:], op0=ALU.subtract, op1=ALU.mult,
                        )
                        u = wpool.tile([P, 256], BF, tag="u", bufs=6, name="u")
                        nc.vector.scalar_tensor_tensor(
                            u[:], t1[:], stf["rstd"][:, tf : tf + 1],
                            lws[l2 - 1]["bn"][:], op0=ALU.mult, op1=ALU.add,
                        )
                        hnb[tf] = u
                    # S: gelu cluster (gelu table segment)
                    for tf in tiles:
                        if l2 == 1:
                            hn = sh1_map[tf][:, tf % 4, :]
                        else:
                            hn = wpool.tile(
                                [P, 256], BF, tag="hn", bufs=6, name="hnf"
                            )
                            hn = hn[:]
                        nc.scalar.activation(hn, hnb[tf][:], AF.Gelu)
                        hnb[tf] = hn
                        if l2 == 1 and (tf % 4 == 3 or tf == NT - 1):
                            g0 = tf - (tf % 4)
                            nc.sync.dma_start(
                                out=h1_dram[:].rearrange("(t p) c -> p t c", p=P)[
                                    :, g0 : tf + 1, :
                                ],
                                in_=sh1_map[tf][:, : tf - g0 + 1, :],
                            )
                    # PE/V: transpose + hnT copies
                    lhsb = {}
                    for tf in tiles:
                        trp = psB.tile([P, 2, P], BF, tag="tr", bufs=2)
                        for c in range(2):
                            nc.tensor.transpose(
                                out=trp[:, c, :],
                                in_=hnb[tf][:, c * P : (c + 1) * P],
                                identity=idb_sb[:],
                            )
                        hnT = wpool.tile([P, 2, P], BF, tag="hnT", bufs=6)
                        nc.scalar.copy(hnT[:], trp[:])
                        lhsb[tf] = hnT
                    # PE: projections; S: copies (still gelu/copy table)
                    for tf in tiles:
                        if tf % 4 == 0:
                            fstate["stag_hs"] = wpool.tile(
                                [P, 4, 256], BF, tag="stag_hs", bufs=2, name="shs"
                            )
                        hsht = psPair.tile([P, 512], F32, tag="pair", name="hshtf")
                        for c in range(2):
                            nc.tensor.matmul(
                                out=hsht[:],
                                lhsT=lhsb[tf][:, c, :],
                                rhs=s2["wswt"][:, c * 512 : (c + 1) * 512],
                                start=(c == 0),
                                stop=(c == 1),
                            )
                        nc.scalar.copy(fstate["stag_hs"][:, tf % 4, :], hsht[:, 0:256])
                        nc.scalar.copy(
                            ht_all[:, tf * 256 : (tf + 1) * 256], hsht[:, 256:512]
                        )
                        if l2 == 2:
                            rp = psC.tile([P, 512], F32, tag="agg", name="rpf")[:, 0:64]
                            for c in range(2):
                                nc.tensor.matmul(
                                    out=rp,
                                    lhsT=lhsb[tf][:, c, :],
                                    rhs=s2["skw"][:, c * 64 : (c + 1) * 64],
                                    start=(c == 0),
                                    stop=(c == 1),
                                )
                            nc.vector.scalar_tensor_tensor(
                                res2_all[:, tf * 64 : (tf + 1) * 64], rp, 1.0,
                                s2["skb"][:], op0=ALU.mult, op1=ALU.add,
                            )
                        if tf % 4 == 3 or tf == NT - 1:
                            g0 = tf - (tf % 4)
                            nc.sync.dma_start(
                                out=hs_shard[l2][:].rearrange("(t p) c -> p t c", p=P)[
                                    :, g0 : tf + 1, :
                                ],
                                in_=fstate["stag_hs"][:, : tf - g0 + 1, :],
                            )
                        for c in range(NCH):
                            if tf == CHT[c + 1] - 1:
                                ag_chunk(l2, c)
                    fstate["tf"] = t0f + n

                def front(t):
                    j = t // 2
                    if t % 2 == 0:
                        d = {"S": {}, "res": {}, "msgp": {}}
                        info[j] = d
                        d["hsg"] = wpool.tile(
                            [P, 2 * K * 256], BF, tag="hsg", bufs=3, name="hsg"
                        )
                        d["lr2"] = wpool.tile(
                            [P, 2, 1024], BF, tag="lr2", bufs=2, name="lr2"
                        )
                        ea_sb = wpool.tile([16, 2 * KP], BF, tag="ea", bufs=2)
                        nc.sync.dma_start(
                            out=ea_sb[:], in_=ea_T[:, t * KP : (t + 2) * KP]
                        )
                        tr_sb = wpool.tile([P, 2 * KP], BF, tag="tgtr", bufs=1)
                        nc.sync.dma_start(
                            out=tr_sb[:],
                            in_=tgt_r[0:1, t * KP : (t + 2) * KP].to_broadcast(
                                (P, 2 * KP)
                            ),
                        )
                        d["ea"], d["tr"] = ea_sb, tr_sb
                    d = info[j]
                    if l < 2:
                        if t % 4 == 0:
                            nbr = min(4, NT - t)
                            res_sb[0] = wpool.tile(
                                [P, 4, 256], BF, tag="r4x256", bufs=3, name="res_sb"
                            )
                            rdram = res0_dram if l == 0 else h1_dram
                            nc.sync.dma_start(
                                out=res_sb[0][:, :nbr, :],
                                in_=rdram[:].rearrange("(t p) c -> p t c", p=P)[
                                    :, t : t + nbr, :
                                ],
                            )
                        d["res"][t] = res_sb[0][:, t % 4, :]
                    else:
                        d["res"][t] = res2_all[:, t * 64 : (t + 1) * 64]
                    for k in range(K):
                        nc.gpsimd.indirect_dma_start(
                            out=d["hsg"][
                                :, ((t % 2) * K + k) * 256 : ((t % 2) * K + k + 1) * 256
                            ],
                            out_offset=None,
                            in_=hs_full[l][:],
                            in_offset=bass.IndirectOffsetOnAxis(
                                ap=srcs[:, t * K + k : t * K + k + 1], axis=0
                            ),
                        )
                    eoff = (t % 2) * KP
                    ST_all = wpool.tile([P, KP], BF, tag="ST", bufs=3)
                    nc.vector.tensor_scalar(
                        ST_all[:], d["tr"][:, eoff : eoff + KP], iotac_sb[:, 0:1],
                        None, op0=ALU.is_equal,
                    )
                    S_all = wpool.tile([P, KP], BF, tag="S", bufs=7)
                    nc.vector.tensor_tensor(
                        out=S_all[:].rearrange("p (k c) -> p k c", k=K),
                        in0=iotaK_sb[:].rearrange("p (k c) -> p k c", k=K),
                        in1=tgts[:, t * K : (t + 1) * K].to_broadcast((P, K, P)),
                        op=ALU.is_equal,
                    )
                    d["S"][t] = S_all
                    tinfo[t] = (ST_all, d)

                def msg(t):
                    ST_all, d = tinfo.pop(t)
                    j = t // 2
                    eoff = (t % 2) * KP
                    d["msgp"][t] = []
                    for jj in range(2):
                        msgp = psPair.tile([P, 512], F32, tag="pair", name="msgp")
                        cb = ((t % 2) * K + 2 * jj) * 256
                        nc.tensor.matmul(
                            out=msgp[:], lhsT=idb_sb[:],
                            rhs=d["hsg"][:, cb : cb + 512],
                            start=True, stop=False, skip_group_check=True,
                        )
                        for c in range(2):
                            k = 2 * jj + c
                            nc.tensor.matmul(
                                out=msgp[:, c * 256 : (c + 1) * 256],
                                lhsT=d["ea"][:, eoff + k * P : eoff + (k + 1) * P],
                                rhs=s["we"][:],
                                start=False, stop=False, skip_group_check=True,
                            )
                        for c in range(2):
                            k = 2 * jj + c
                            nc.tensor.matmul(
                                out=msgp[:, c * 256 : (c + 1) * 256],
                                lhsT=ST_all[:, k * P : (k + 1) * P],
                                rhs=ht_all[:, t * 256 : (t + 1) * 256],
                                start=False, stop=(c == 1), skip_group_check=True,
                            )
                        d["msgp"][t].append(msgp)

                def prelu(t):
                    j = t // 2
                    d = info[j]
                    for jj in range(2):
                        nc.scalar.activation(
                            d["lr2"][:, t % 2, jj * 512 : (jj + 1) * 512],
                            d["msgp"][t][jj][:],
                            AF.Prelu, alpha=0.2,
                        )
                    del d["msgp"][t]

                def alpha(j):
                    d = info[j]
                    scr2 = wpool.tile([P, 2048], BF, tag="scr2", bufs=1)
                    nc.vector.tensor_tensor(
                        out=scr2[:],
                        in0=d["lr2"][:].rearrange("p a b -> p (a b)"),
                        in1=a2_sb[:],
                        op=ALU.mult,
                    )
                    alph2 = spool.tile([P, 32], F32, tag="alph", bufs=2)
                    nc.vector.tensor_reduce(
                        out=alph2[:],
                        in_=scr2[:].rearrange("p (g d) -> p g d", d=64),
                        axis=mybir.AxisListType.X,
                        op=ALU.add,
                    )
                    expa2 = spool.tile([P, 32], F32, tag="expa", bufs=2)
                    nc.scalar.activation(expa2[:], alph2[:], AF.Exp)
                    d["expa"] = expa2

                def mid(j):
                    d = info[j]
                    d["waug"] = []
                    for i in range(2):
                        w_aug = wpool.tile(
                            [P, K * 260], BF, tag="waug", bufs=2, name="waug"
                        )
                        w4 = w_aug[:].rearrange("p (k h c) -> p k h c", k=K, c=65)
                        hs4 = d["hsg"][
                            :, i * K * 256 : (i + 1) * K * 256
                        ].rearrange("p (k h dd) -> p k h dd", k=K, dd=64)
                        e4 = d["expa"][:, i * 16 : (i + 1) * 16].rearrange(
                            "p (k h) -> p k h", k=K
                        )
                        nc.vector.tensor_tensor(
                            out=w4[:, :, :, 0:64],
                            in0=hs4[:],
                            in1=e4[:].to_broadcast((P, K, H, 64)),
                            op=ALU.mult,
                        )
                        nc.scalar.copy(w4[:, :, :, 64], e4)
                        d["waug"].append(w_aug)

                def tail_pe(j):
                    d = info[j]
                    d["agg"] = []
                    d["den"] = []
                    for i in range(2):
                        agg = psC.tile([P, 512], F32, tag="agg", name="agg")[:, 0:260]
                        for k in range(K):
                            nc.tensor.matmul(
                                out=agg,
                                lhsT=d["S"][2 * j + i][:, k * P : (k + 1) * P],
                                rhs=d["waug"][i][:, k * 260 : (k + 1) * 260],
                                start=(k == 0),
                                stop=(k == K - 1),
                            )
                        d["agg"].append(agg)
                    for i in range(2):
                        aggv = d["agg"][i].rearrange("p (h c) -> p h c", c=65)
                        den = spool.tile([P, 4], F32, tag="den", bufs=4)
                        nc.scalar.activation(den[:], aggv[:, :, 64], AF.Copy, bias=1e-8)
                        d["den"].append(den)

                def tail_vs(j):
                    d = info.pop(j)
                    for i in range(2):
                        tt = 2 * j + i
                        aggv = d["agg"][i].rearrange("p (h c) -> p h c", c=65)
                        rden = spool.tile([P, 4], F32, tag="rden", bufs=2)
                        nc.vector.reciprocal(rden[:], d["den"][i][:])
                        gat = wpool.tile([P, 256], F32, tag="gat", bufs=1)
                        nc.vector.scalar_tensor_tensor(
                            gat[:].rearrange("p (h dd) -> p h dd", h=4),
                            aggv[:, :, 0:64],
                            0.25 if l == 2 else 1.0,
                            rden[:].to_broadcast((P, 4, 64)),
                            op0=ALU.mult,
                            op1=ALU.mult,
                        )
                        if l == 2:
                            g64 = wpool.tile([P, 64], F32, tag="g64", bufs=2)
                            nc.vector.tensor_reduce(
                                out=g64[:],
                                in_=gat[:].rearrange("p (h dd) -> p dd h", h=4),
                                axis=mybir.AxisListType.X,
                                op=ALU.add,
                            )
                            zin = g64[:]
                        else:
                            zin = gat[:]
                        zslot = z_all[:, tt, :outd]
                        nc.vector.scalar_tensor_tensor(
                            zslot, zin, 1.0, d["res"][tt],
                            op0=ALU.mult, op1=ALU.add,
                            accum_out=st["s1"][:, tt : tt + 1],
                        )
                        junk = wpool.tile(
                            [P, 256], BF, tag="junk", bufs=1, name="junke"
                        )[:, :outd]
                        nc.scalar.activation(
                            junk, zslot, AF.Square,
                            accum_out=st["s2"][:, tt : tt + 1],
                        )

                for t in range(NT + 2):
                    if t % 2 == 0:
                        if t >= 4:
                            alpha(t // 2 - 2)
                        if t >= 6:
                            tail_pe(t // 2 - 3)
                    else:
                        if t >= 5:
                            mid(t // 2 - 2)
                        if t >= 7:
                            tail_vs(t // 2 - 3)
                    if t < NT:
                        front(t)
                    if 1 <= t <= NT:
                        msg(t - 1)
                    if 2 <= t <= NT + 1:
                        prelu(t - 2)
                    # fused next-layer f2a in 6-tile clusters
                    if l < 2 and t % 2 == 1 and (t // 2) % 3 == 2 and t >= 13:
                        f2a_block(2 * (t // 2 - 3) + 1, 6)
                NP = NT // 2
                alpha(NP - 1)
                tail_pe(NP - 2)
                mid(NP - 1)
                tail_vs(NP - 2)
                tail_pe(NP - 1)
                tail_vs(NP - 1)
                if l < 2:
                    while fstate["tf"] < NT:
                        f2a_block(NT - 1, 8)

            if DEBUG:
                nc.sync.dma_start(out=dbg_z0[:], in_=z_all[:, :, 0:64])
            if DEBUG:
                nc.sync.dma_start(out=dbg_ht[:], in_=ht_all[:])
            for l in range(3):
                with nc.named_scope(f"edge{l}"):
                    edge_f1(l)
                    if l == 2:
                        sqrt_batch(3, LW[2]["outd"])
                if DEBUG and l == 0:
                    nc.sync.dma_start(out=dbg_z1[:], in_=z_all[:])
                if l == 2:
                    with nc.named_scope("f2a3"):
                        f2a(3)

            sig = ppool.tile([P, NT], F32)
            nc.scalar.activation(sig[:], scores[:], AF.Sigmoid, bias=bh2_val)
            nc.sync.dma_start(out=out[:], in_=sig[:])
    return nc


# ---------------------------------------------------------------- host prep
def _balance_nodes(tgt):
    """Degree-balanced assignment of nodes to NCORES*NT tiles of <=128 slots.
    Returns (gtile[node], slot[node], K)."""
    import heapq

    NTILES = NCORES * NT
    deg = np.bincount(tgt, minlength=N)
    order = np.argsort(-deg, kind="stable")
    gtile = np.empty(N, np.int32)
    slot = np.empty(N, np.int32)
    count = np.zeros(NTILES, np.int32)
    load = np.zeros(NTILES, np.int64)
    heap = [(0, t) for t in range(NTILES)]
    heapq.heapify(heap)
    for node in order:
        while True:
            ld, t = heapq.heappop(heap)
            if count[t] < P and ld == load[t]:
                break
        gtile[node] = t
        slot[node] = count[t]
        count[t] += 1
        load[t] += deg[node]
        if count[t] < P:
            heapq.heappush(heap, (int(load[t]), t))
    K = int(np.ceil(load.max() / P))
    return gtile, slot, K


def _prep(inputs):
    ei = np.asarray(inputs["edge_index"]).astype(np.int64)
    src, tgt = ei[0], ei[1]
    ea = np.asarray(inputs["edge_attr"], np.float32)

    gtile, slot, K = _balance_nodes(tgt)
    core_of = gtile // NT
    lt_of = gtile % NT

    lt = lt_of.astype(np.int64)
    chunk = np.searchsorted(np.array(CHT[1:-1]), lt, side="right")
    chrows = np.array(CHROWS)[chunk]
    chbase = np.array(CHBASE)[chunk]
    chtile0 = np.array(CHT[:-1])[chunk]
    row_id = chbase + core_of.astype(np.int64) * chrows + (lt - chtile0) * P + slot

    NTK = NT * K
    ES = NTK * P

    e_core = core_of[tgt]
    e_lt = lt_of[tgt]
    e_p = slot[tgt]  # target's slot within its tile
    order = np.lexsort((e_lt, e_core))
    src_s = src[order]
    ea_s = ea[order]
    e_core_s, e_lt_s, e_p_s = e_core[order], e_lt[order], e_p[order]

    grp = e_core_s * NT + e_lt_s
    idx_in_grp = np.zeros(len(grp), np.int64)
    _, first_pos, cnt = np.unique(grp, return_index=True, return_counts=True)
    for fp, c in zip(first_pos, cnt):
        idx_in_grp[fp : fp + c] = np.arange(c)
    assert cnt.max() <= K * P, (cnt.max(), K)

    src_cols = np.zeros((NCORES, P, NTK), np.int32)
    tgt_cols = np.full((NCORES, P, NTK), -1.0, np.float32)
    tgt_rows = np.full((NCORES, 1, ES), -1.0, np.float32)
    ea_T = np.zeros((NCORES, 16, ES), np.float32)
    eslot = e_lt_s * (K * P) + idx_in_grp
    col = eslot // P
    row = eslot % P
    src_cols[e_core_s, row, col] = row_id[src_s].astype(np.int32)
    tgt_cols[e_core_s, row, col] = e_p_s.astype(np.float32)
    tgt_rows[e_core_s, 0, eslot] = e_p_s.astype(np.float32)
    ea_T[e_core_s[:, None], np.arange(ED)[None, :], eslot[:, None]] = ea_s

    x = np.asarray(inputs["x"], np.float32)
    x_T = np.zeros((NCORES, 384, NPAD), np.float32)
    pos = lt * P + slot  # position within core [0, NPAD)
    x_T[core_of, :FN, pos] = x
    x_T[core_of, FN, pos] = 1.0  # ones-row carries ctx@Wp+bp via wp1

    rep = lambda v: np.broadcast_to(
        np.asarray(v, np.float32)[None, :], (P, len(np.asarray(v)))
    ).copy()
    bf = lambda a: np.asarray(a).astype(ml_dtypes.bfloat16)

    Wp = np.asarray(inputs["Wp"], np.float32)
    cb = (
        np.asarray(inputs["context_vector"], np.float32) @ Wp[FN:]
        + np.asarray(inputs["bp"], np.float32)
    )
    wp1 = np.zeros((384, 64), np.float32)
    wp1[:FN] = Wp[:FN]
    wp1[FN] = cb
    wp1 = wp1.astype(ml_dtypes.bfloat16)

    common = {
        "wp1": wp1,
        "iota2d": np.broadcast_to(
            np.arange(P, dtype=np.float32)[None, :], (P, P)
        ).astype(ml_dtypes.bfloat16),
        "iota_col": np.arange(P, dtype=np.float32)[:, None].copy(),
        "ident": np.eye(P, dtype=np.float32).astype(ml_dtypes.bfloat16),
        "wh1": np.asarray(inputs["Wh1"], np.float32),
        "bh1_rep": rep(inputs["bh1"]),
        "wh2_rep": rep(np.asarray(inputs["Wh2"], np.float32)[:, 0]),
    }
    g_in = np.asarray(inputs["g_in"], np.float32)
    b_in = np.asarray(inputs["b_in"], np.float32)
    for l in range(3):
        sfx = str(l)
        ws = np.asarray(inputs["Ws" + sfx], np.float32)
        wt = np.asarray(inputs["Wt" + sfx], np.float32)
        wswt = np.concatenate([ws, wt], axis=1)
        if l == 0:
            wswt = np.concatenate(
                [g_in[:, None] * wswt, (b_in @ wswt)[None, :]], axis=0
            )
        common[f"wswt{l}"] = bf(wswt)
        we = np.zeros((16, 256), np.float32)
        we[:ED] = np.asarray(inputs["We" + sfx], np.float32)
        common[f"we{l}"] = bf(we)
        a1 = np.asarray(inputs["A" + sfx], np.float32).reshape(-1)
        common[f"a_rep2_{l}"] = bf(rep(np.tile(a1, 2 * K)))
        if l != 1:
            skw = np.asarray(inputs[f"Sk{l}W"], np.float32)
            if l == 0:
                skw = np.concatenate(
                    [
                        g_in[:, None] * skw,
                        (b_in @ skw + np.asarray(inputs["Sk0b"], np.float32))[
                            None, :
                        ],
                    ],
                    axis=0,
                )
            common[f"skw{l}"] = bf(skw)
            common[f"skb_rep{l}"] = bf(rep(inputs[f"Sk{l}b"]))
        common[f"gn_rep{l}"] = bf(rep(inputs["gn" + sfx]))
        common[f"bn_rep{l}"] = bf(rep(inputs["bn" + sfx]))

    in_maps = []
    for c in range(NCORES):
        m = dict(common)
        m["x_T"] = x_T[c].astype(ml_dtypes.bfloat16)
        m["src_c"] = src_cols[c]
        m["tgt_c"] = tgt_cols[c].astype(ml_dtypes.bfloat16)
        m["tgt_r"] = tgt_rows[c].astype(ml_dtypes.bfloat16)
        m["ea_T"] = ea_T[c].astype(ml_dtypes.bfloat16)
        in_maps.append(m)
    bh2_val = float(np.asarray(inputs["bh2"]).reshape(-1)[0])
    return in_maps, K, bh2_val, (core_of, lt_of, slot)


def kernel(**inputs):
    in_maps, K, bh2_val, (core_of, lt_of, slot) = _prep(inputs)
    nc = build_nc(K, bh2_val)
    res = run_bass_kernel_spmd(
        nc, in_maps, core_ids=list(range(NCORES)), trace=TRACE
    )
    LAST_RESULT["exec_time_ns"] = res.exec_time_ns
    LAST_RESULT["res"] = res
    if DEBUG:
        LAST_RESULT["dbg"] = res.results
        LAST_RESULT["layout"] = (core_of, lt_of, slot)
    outs = np.stack([res.results[c]["out"] for c in range(NCORES)])  # [8, P, NT]
    return outs[core_of, slot, lt_of].astype(np.float32)
